# revision 1
# baseline (speedup 1.0000x reference)
"""BigBird attention (B=4, N=4096, D=1024, H=16, BS=64) on 8 TRN2 NeuronCores.

Sharding: batch (4-way) x head-group (2-way).  Core c handles batch c//2 and
heads [hg*8, hg*8+8) where hg = c%2 (d_model slice [hg*512, hg*512+512)).
Per core:
  pass A: QKV projections.  x.T tiles produced with PE transposes; q/k emitted
          transposed (qT/kT: [dl, n], head dim on partitions), v natural.
          The 1/sqrt(dh) score scale is folded into Wq/bq on the host.
  pass B: per-head BigBird attention (local sliding window + global-column
          softmax summed, then global-row full attention overwrite), writing
          ctx transposed into an SBUF-resident accumulator.
  pass C: row-parallel output projection -> partial outT [d_model, n].
Host combines: out[b] = outT(core 2b).T + outT(core 2b+1).T + bo.

The kernel is specialized (compiled) per global_indices value.
"""

import functools
import sys

import numpy as np

P = 128
BS = 64
NEG = -1e9


def _ensure_path():
    try:
        import concourse.bass  # noqa: F401
    except ImportError:
        sys.path.insert(0, "/opt/trn_rl_repo")


def _build(n, dmodel, dl, g0, g1):
    """Build the per-core Bass program.

    n: sequence length per core, dmodel: model dim (contraction for QKV,
    output dim for out-proj), dl: local (per-core) head dims = hpc*64.
    g0, g1: global block indices (compile-time constants).
    """
    _ensure_path()
    from contextlib import ExitStack

    import concourse.bass as bass  # noqa: F401
    import concourse.tile as tile
    from concourse import bacc, mybir
    from concourse.masks import make_identity

    f32 = mybir.dt.float32
    bf16 = mybir.dt.bfloat16
    AF = mybir.ActivationFunctionType
    AX = mybir.AxisListType.X

    nch = n // 512   # 512-column chunks of the sequence
    ndc = dmodel // P  # contraction chunks for QKV proj
    njt = dl // P      # row tiles of qT/kT
    hpc = dl // BS     # heads per core
    nt = n // P        # query tiles (2 blocks each)
    nkc = n // 512     # key chunks for global-row attention
    ndc2 = dl // P     # contraction chunks for out proj
    npad = (n + 2 * BS) // P  # padded v blocks

    nc = bacc.Bacc(None, target_bir_lowering=False, debug=False)

    x_d = nc.dram_tensor("x", [n, dmodel], bf16, kind="ExternalInput")
    wq_d = nc.dram_tensor("wqT", [dmodel, dl], bf16, kind="ExternalInput")
    wk_d = nc.dram_tensor("wkT", [dmodel, dl], bf16, kind="ExternalInput")
    wv_d = nc.dram_tensor("wvT", [dmodel, dl], bf16, kind="ExternalInput")
    wo_d = nc.dram_tensor("woT", [dl, dmodel], bf16, kind="ExternalInput")
    bq_d = nc.dram_tensor("bq", [dl], f32, kind="ExternalInput")
    bk_d = nc.dram_tensor("bk", [dl], f32, kind="ExternalInput")
    bv_d = nc.dram_tensor("bv", [dl], f32, kind="ExternalInput")
    out_d = nc.dram_tensor("outT", [dmodel, n], f32, kind="ExternalOutput")

    with tile.TileContext(nc) as tc, ExitStack() as top:
        dram = top.enter_context(tc.tile_pool(name="dram", bufs=1, space="DRAM"))
        qT_d = dram.tile([dl, n], bf16)
        kT_d = dram.tile([dl, n], bf16)
        v_d = dram.tile([n, dl], bf16)

        const = top.enter_context(tc.tile_pool(name="const", bufs=1))
        ident = const.tile([P, P], bf16)
        make_identity(nc, ident)
        identf = const.tile([P, P], f32)
        make_identity(nc, identf)
        ones1 = const.tile([1, BS], f32)
        nc.gpsimd.memset(ones1, 1.0)
        onesP = const.tile([1, P], f32)
        nc.gpsimd.memset(onesP, 1.0)
        # additive mask for the 2-block query tile vs 4-block key window
        mask = const.tile([P, 256], f32)
        nc.gpsimd.memset(mask, 0.0)
        nc.gpsimd.memset(mask[0:BS, 192:256], NEG)
        nc.gpsimd.memset(mask[BS:P, 0:BS], NEG)

        # ctx transposed accumulator: row (grp*128+p) = local head dim,
        # lives in SBUF through passes B and C.
        ctx_pool = top.enter_context(tc.tile_pool(name="ctx", bufs=1))
        ctxT = ctx_pool.tile([P, ndc2, n], bf16)

        # ---------------- pass A: projections ----------------
        with ExitStack() as ps:
            wpool = ps.enter_context(tc.tile_pool(name="wpool", bufs=1))
            wq_sb = wpool.tile([P, ndc, dl], bf16)
            wk_sb = wpool.tile([P, ndc, dl], bf16)
            wv_sb = wpool.tile([P, ndc, dl], bf16)
            nc.sync.dma_start(wq_sb, wq_d.rearrange("(a p) j -> p a j", p=P))
            nc.sync.dma_start(wk_sb, wk_d.rearrange("(a p) j -> p a j", p=P))
            nc.sync.dma_start(wv_sb, wv_d.rearrange("(a p) j -> p a j", p=P))
            bq_sb = wpool.tile([P, njt], f32)
            bk_sb = wpool.tile([P, njt], f32)
            nc.sync.dma_start(bq_sb, bq_d.rearrange("(a p) -> p a", p=P))
            nc.sync.dma_start(bk_sb, bk_d.rearrange("(a p) -> p a", p=P))
            bv_row = wpool.tile([1, dl], f32)
            nc.sync.dma_start(bv_row, bv_d.rearrange("(a j) -> a j", a=1))

            psA = ps.enter_context(tc.tile_pool(name="psA", bufs=4, space="PSUM"))

            # bv broadcast to [P, dl] via ones-matmul
            bvp = psA.tile([P, dl], f32, tag="ps_a")
            nc.tensor.matmul(bvp, onesP, bv_row, start=True, stop=True)
            bv_bc = wpool.tile([P, dl], f32)
            nc.vector.tensor_copy(bv_bc, bvp)

            xtpool = ps.enter_context(tc.tile_pool(name="xtpool", bufs=2))
            aout = ps.enter_context(tc.tile_pool(name="aout", bufs=4))

            for ch in range(nch):
                n0 = ch * 512
                xT = xtpool.tile([P, ndc, 512], bf16, tag="xT")
                for dc in range(ndc):
                    nc.sync.dma_start(
                        xT[:, dc, :],
                        x_d[n0 : n0 + 512, dc * P : (dc + 1) * P],
                        transpose=True,
                    )
                # qT / kT (transposed outputs, bias per-partition)
                for w_sb, b_sb, dst in ((wq_sb, bq_sb, qT_d), (wk_sb, bk_sb, kT_d)):
                    for jt in range(njt):
                        pp = psA.tile([P, 512], f32, tag="ps_a")
                        for dc in range(ndc):
                            nc.tensor.matmul(
                                pp,
                                w_sb[:, dc, jt * P : (jt + 1) * P],
                                xT[:, dc, :],
                                start=(dc == 0),
                                stop=(dc == ndc - 1),
                            )
                        ot = aout.tile([P, 512], bf16, tag="aout")
                        nc.scalar.activation(
                            ot, pp, AF.Identity, bias=b_sb[:, jt : jt + 1]
                        )
                        nc.sync.dma_start(
                            dst[jt * P : (jt + 1) * P, n0 : n0 + 512], ot
                        )
                # v (natural layout, bias broadcast along free dim)
                for ns in range(4):
                    pp = psA.tile([P, dl], f32, tag="ps_a")
                    for dc in range(ndc):
                        nc.tensor.matmul(
                            pp,
                            xT[:, dc, ns * P : (ns + 1) * P],
                            wv_sb[:, dc, :],
                            start=(dc == 0),
                            stop=(dc == ndc - 1),
                        )
                    ot = aout.tile([P, dl], bf16, tag="aout_v")
                    nc.vector.tensor_add(ot, pp, bv_bc)
                    nc.sync.dma_start(v_d[n0 + ns * P : n0 + (ns + 1) * P, :], ot)

        # ---------------- pass B: attention ----------------
        with ExitStack() as ps:
            hq = ps.enter_context(tc.tile_pool(name="hq", bufs=2))
            hk = ps.enter_context(tc.tile_pool(name="hk", bufs=2))
            hv = ps.enter_context(tc.tile_pool(name="hv", bufs=2))
            hvu = ps.enter_context(tc.tile_pool(name="hvu", bufs=2))
            gpool = ps.enter_context(tc.tile_pool(name="gpool", bufs=2))
            apool = ps.enter_context(tc.tile_pool(name="apool", bufs=4))
            atpool = ps.enter_context(tc.tile_pool(name="atpool", bufs=4))
            stat = ps.enter_context(tc.tile_pool(name="stat", bufs=4))
            psS = ps.enter_context(tc.tile_pool(name="psS", bufs=3, space="PSUM"))
            psT = ps.enter_context(tc.tile_pool(name="psT", bufs=2, space="PSUM"))
            psC = ps.enter_context(tc.tile_pool(name="psC", bufs=2, space="PSUM"))

            for h in range(hpc):
                r0 = h * BS
                p0, grp = (h % 2) * BS, h // 2
                qT_sb = hq.tile([BS, n], bf16, tag="hq")
                nc.sync.dma_start(qT_sb, qT_d[r0 : r0 + BS, :])
                kT_sb = hk.tile([BS, n + 2 * BS], bf16, tag="hk")
                nc.sync.dma_start(kT_sb[:, BS : BS + n], kT_d[r0 : r0 + BS, :])
                nc.sync.dma_start(kT_sb[:, 0:BS], kT_d[r0 : r0 + BS, n - BS : n])
                nc.sync.dma_start(kT_sb[:, BS + n :], kT_d[r0 : r0 + BS, 0:BS])
                vs = v_d[:, r0 : r0 + BS]
                v_sb = hv.tile([P, npad, BS], bf16, tag="hv")
                nc.sync.dma_start(v_sb[0:BS, 0, :], vs[n - BS : n, :])
                nc.sync.dma_start(v_sb[BS:P, 0, :], vs[0:BS, :])
                nc.sync.dma_start(
                    v_sb[:, 1 : npad - 1, :],
                    vs[BS : n - BS, :].rearrange("(a p) c -> p a c", p=P),
                )
                nc.sync.dma_start(v_sb[0:BS, npad - 1, :], vs[n - BS : n, :])
                nc.sync.dma_start(v_sb[BS:P, npad - 1, :], vs[0:BS, :])
                vu_sb = hvu.tile([P, n // P, BS], bf16, tag="hvu")
                nc.sync.dma_start(vu_sb, vs.rearrange("(a p) c -> p a c", p=P))
                # global key/value blocks (compact)
                kTg = gpool.tile([BS, 2 * BS], bf16, tag="kTg")
                vg = gpool.tile([P, BS], bf16, tag="vg")
                qg = gpool.tile([BS, P], bf16, tag="qg")
                for gi, gv in enumerate((g0, g1)):
                    nc.vector.tensor_copy(
                        kTg[:, gi * BS : (gi + 1) * BS],
                        kT_sb[:, BS + gv * BS : BS + (gv + 1) * BS],
                    )
                    nc.sync.dma_start(
                        vg[gi * BS : (gi + 1) * BS, :], vs[gv * BS : (gv + 1) * BS, :]
                    )
                    nc.vector.tensor_copy(
                        qg[:, gi * BS : (gi + 1) * BS],
                        qT_sb[:, gv * BS : (gv + 1) * BS],
                    )

                # ---- local window + global columns ----
                for t in range(nt):
                    sps = psS.tile([P, 512], f32, tag="sps")
                    qsl = qT_sb[:, t * P : (t + 1) * P]
                    nc.tensor.matmul(
                        sps[:, 0:256],
                        qsl,
                        kT_sb[:, t * P : t * P + 256],
                        start=True,
                        stop=True,
                    )
                    nc.tensor.matmul(sps[:, 256:384], qsl, kTg, start=True, stop=True)
                    nc.vector.tensor_add(sps[:, 0:256], sps[:, 0:256], mask)
                    negm = stat.tile([P, 2], f32, tag="negm")
                    nc.vector.reduce_max(
                        negm[:, 0:1], sps[:, 0:256], axis=AX, negate=True
                    )
                    nc.vector.reduce_max(
                        negm[:, 1:2], sps[:, 256:384], axis=AX, negate=True
                    )
                    s2 = stat.tile([P, 2], f32, tag="s2")
                    a_sb = apool.tile([P, 384], bf16, tag="a")
                    nc.scalar.activation(
                        a_sb[:, 0:256],
                        sps[:, 0:256],
                        AF.Exp,
                        bias=negm[:, 0:1],
                        accum_out=s2[:, 0:1],
                    )
                    nc.scalar.activation(
                        a_sb[:, 256:384],
                        sps[:, 256:384],
                        AF.Exp,
                        bias=negm[:, 1:2],
                        accum_out=s2[:, 1:2],
                    )
                    r2 = stat.tile([P, 2], f32, tag="r2")
                    nc.vector.reciprocal(r2, s2)
                    nc.vector.tensor_scalar_mul(
                        a_sb[:, 0:256], a_sb[:, 0:256], r2[:, 0:1]
                    )
                    nc.vector.tensor_scalar_mul(
                        a_sb[:, 256:384], a_sb[:, 256:384], r2[:, 1:2]
                    )
                    at_sb = atpool.tile([P, 3, P], bf16, tag="at")
                    for cc in range(3):
                        tp = psT.tile([P, P], bf16, tag="tp")
                        nc.tensor.transpose(tp, a_sb[:, cc * P : (cc + 1) * P], ident)
                        nc.vector.tensor_copy(at_sb[:, cc, :], tp)
                    cps = psC.tile([BS, P], f32, tag="cps")
                    nc.tensor.matmul(
                        cps, v_sb[:, t, :], at_sb[:, 0, :], start=True, stop=False
                    )
                    nc.tensor.matmul(
                        cps, v_sb[:, t + 1, :], at_sb[:, 1, :], start=False, stop=False
                    )
                    nc.tensor.matmul(cps, vg, at_sb[:, 2, :], start=False, stop=True)
                    nc.scalar.copy(ctxT[p0 : p0 + BS, grp, t * P : (t + 1) * P], cps)

                # ---- global rows: full attention, overwrite ----
                mr = stat.tile([P, nkc], f32, tag="mr")
                for kc in range(nkc):
                    sps = psS.tile([P, 512], f32, tag="sps")
                    nc.tensor.matmul(
                        sps,
                        qg,
                        kT_sb[:, BS + kc * 512 : BS + (kc + 1) * 512],
                        start=True,
                        stop=True,
                    )
                    nc.vector.reduce_max(mr[:, kc : kc + 1], sps, axis=AX)
                negmr = stat.tile([P, 1], f32, tag="negmr")
                nc.vector.reduce_max(negmr, mr, axis=AX, negate=True)
                sr = stat.tile([P, nkc], f32, tag="sr")
                crp = psC.tile([BS, P], f32, tag="crp", bufs=1)
                for kc in range(nkc):
                    sps = psS.tile([P, 512], f32, tag="sps")
                    nc.tensor.matmul(
                        sps,
                        qg,
                        kT_sb[:, BS + kc * 512 : BS + (kc + 1) * 512],
                        start=True,
                        stop=True,
                    )
                    ar = apool.tile([P, 512], bf16, tag="ar")
                    nc.scalar.activation(
                        ar, sps, AF.Exp, bias=negmr, accum_out=sr[:, kc : kc + 1]
                    )
                    for cc in range(4):
                        tp = psT.tile([P, P], bf16, tag="tp")
                        nc.tensor.transpose(tp, ar[:, cc * P : (cc + 1) * P], ident)
                        atr = atpool.tile([P, P], bf16, tag="atr")
                        nc.vector.tensor_copy(atr, tp)
                        nc.tensor.matmul(
                            crp,
                            vu_sb[:, kc * 4 + cc, :],
                            atr,
                            start=(kc == 0 and cc == 0),
                            stop=(kc == nkc - 1 and cc == 3),
                        )
                srf = stat.tile([P, 1], f32, tag="srf")
                nc.vector.reduce_sum(srf, sr, axis=AX)
                rr = stat.tile([P, 1], f32, tag="rr")
                nc.vector.reciprocal(rr, srf)
                tpr = psT.tile([P, P], f32, tag="tp")
                nc.tensor.transpose(tpr[0:1, :], rr, identf)
                rrT = stat.tile([1, P], f32, tag="rrT")
                nc.vector.tensor_copy(rrT, tpr[0:1, :])
                rbc = psT.tile([P, P], f32, tag="tp")
                nc.tensor.matmul(rbc[0:BS, :], ones1, rrT, start=True, stop=True)
                rbc_sb = stat.tile([BS, P], f32, tag="rbc_sb")
                nc.scalar.copy(rbc_sb, rbc[0:BS, :])
                for gi, gv in enumerate((g0, g1)):
                    nc.vector.tensor_mul(
                        ctxT[p0 : p0 + BS, grp, gv * BS : (gv + 1) * BS],
                        crp[:, gi * BS : (gi + 1) * BS],
                        rbc_sb[:, gi * BS : (gi + 1) * BS],
                    )

        # ---------------- pass C: output projection ----------------
        with ExitStack() as ps:
            wop = ps.enter_context(tc.tile_pool(name="wop", bufs=1))
            wo_sb = wop.tile([P, ndc2, dmodel], bf16)
            nc.sync.dma_start(wo_sb, wo_d.rearrange("(a p) o -> p a o", p=P))
            copool = ps.enter_context(tc.tile_pool(name="co", bufs=4))
            psO = ps.enter_context(tc.tile_pool(name="psO", bufs=4, space="PSUM"))
            for ot in range(dmodel // P):
                for ncc in range(n // 512):
                    pp = psO.tile([P, 512], f32, tag="pso")
                    for dc in range(ndc2):
                        nc.tensor.matmul(
                            pp,
                            wo_sb[:, dc, ot * P : (ot + 1) * P],
                            ctxT[:, dc, ncc * 512 : (ncc + 1) * 512],
                            start=(dc == 0),
                            stop=(dc == ndc2 - 1),
                        )
                    ob = copool.tile([P, 512], f32, tag="ob")
                    nc.vector.tensor_copy(ob, pp)
                    nc.sync.dma_start(
                        out_d[ot * P : (ot + 1) * P, ncc * 512 : (ncc + 1) * 512], ob
                    )

    nc.finalize()
    return nc


@functools.lru_cache(maxsize=8)
def _get(n, dmodel, dl, g0, g1):
    return _build(n, dmodel, dl, g0, g1)


def _prepare(inputs):
    """Build (nc, in_maps, meta) for the SPMD run from full unsharded inputs."""
    x = np.asarray(inputs["x"], np.float32)
    Wq = np.asarray(inputs["Wq"], np.float32)
    Wk = np.asarray(inputs["Wk"], np.float32)
    Wv = np.asarray(inputs["Wv"], np.float32)
    Wo = np.asarray(inputs["Wo"], np.float32)
    bq = np.asarray(inputs["bq"], np.float32)
    bk = np.asarray(inputs["bk"], np.float32)
    bv = np.asarray(inputs["bv"], np.float32)
    bo = np.asarray(inputs["bo"], np.float32)
    gi = np.asarray(inputs["global_indices"]).astype(np.int64)
    g0, g1 = int(gi[0]), int(gi[1])

    b_, n_, d_ = x.shape
    dl = d_ // 2
    scale = 1.0 / np.sqrt(np.float32(64.0)).astype(np.float32)

    nc = _get(n_, d_, dl, g0, g1)

    import ml_dtypes

    bf = ml_dtypes.bfloat16
    in_maps = []
    for c in range(8):
        b, hg = divmod(c, 2)
        S = slice(hg * dl, (hg + 1) * dl)
        in_maps.append(
            {
                "x": np.ascontiguousarray(x[b]).astype(bf),
                "wqT": np.ascontiguousarray((Wq[S, :] * scale).T).astype(bf),
                "wkT": np.ascontiguousarray(Wk[S, :].T).astype(bf),
                "wvT": np.ascontiguousarray(Wv[S, :].T).astype(bf),
                "woT": np.ascontiguousarray(Wo[:, S].T).astype(bf),
                "bq": np.ascontiguousarray(bq[S] * scale),
                "bk": np.ascontiguousarray(bk[S]),
                "bv": np.ascontiguousarray(bv[S]),
            }
        )

    return nc, in_maps, (b_, n_, d_, bo)


def _combine(res, meta):
    b_, n_, d_, bo = meta
    out = np.empty((b_, n_, d_), np.float32)
    for b in range(b_):
        out[b] = res[2 * b]["outT"].T + res[2 * b + 1]["outT"].T + bo[None, :]
    return out


def kernel(**inputs):
    _ensure_path()
    from concourse.bass_utils import run_bass_kernel_spmd

    nc, in_maps, meta = _prepare(inputs)
    res = run_bass_kernel_spmd(nc, in_maps, list(range(8))).results
    return _combine(res, meta)



# revision 13
# speedup vs baseline: 1.3417x; 1.3417x over previous
"""BigBird attention (B=4, N=4096, D=1024, H=16, BS=64) on 8 TRN2 NeuronCores.

Sharding: batch (4-way) x head-group (2-way).  Core c handles batch c//2 and
heads [hg*8, hg*8+8) where hg = c%2 (d_model slice [hg*512, hg*512+512)).

Per core:
  pass A: QKV projections.  x.T tiles produced with DMA transposes; q/k
          emitted transposed (qT/kT: [dl, n], head dim on partitions), v
          natural.  score scale folded into Wq/bq on the host; bv dropped
          entirely (attention is affine in v: host adds c(q)*bv@Wo.T).
  pass B: per-head BigBird attention, all scores computed transposed
          (S^T = K_chunk^T Q, keys on partitions) so probabilities feed the
          AV matmuls directly as stationary operands -- no PE transposes.
          The sliding-window mask is folded into 4 extra contraction rows
          (rank-2 outer product of periodic 0/1 q-patterns and -1e9
          k-patterns), so exp() yields exact zeros in the masked corners.
          No max subtraction (scores bounded ~|3|).  V carries a ones
          column so each AV matmul also emits the softmax denominator
          per-partition; normalization is a per-partition reciprocal.
  pass C: transpose ctx with the PE, then row-parallel output projection
          -> partial outT [d_model, n] (f32).
Host combines: out[b] = outT(core 2b).T + outT(core 2b+1).T + bo + c(q)*bv@Wo.T
with c(q) = 1 for rows in global blocks else 2.

The kernel is specialized (compiled) per global_indices value.
"""

import functools
import sys

import numpy as np

P = 128
BS = 64
NEG = -1e9


def _ensure_path():
    try:
        import concourse.bass  # noqa: F401
    except ImportError:
        sys.path.insert(0, "/opt/trn_rl_repo")


def _build(n, dmodel, dl, g0, g1, dbg=0):
    """Build the per-core Bass program.

    n: sequence length per core, dmodel: model dim, dl: local head dims =
    hpc*64.  g0, g1: global block indices (compile-time constants).
    """
    _ensure_path()
    from contextlib import ExitStack

    import concourse.bass as bass  # noqa: F401
    import concourse.tile as tile
    from concourse import bacc, mybir
    from concourse.masks import make_identity

    f32 = mybir.dt.float32
    bf16 = mybir.dt.bfloat16
    AF = mybir.ActivationFunctionType
    OP = mybir.AluOpType

    nch = n // 512     # 512-column chunks of the sequence
    ndc = dmodel // P  # contraction chunks for QKV proj
    njt = dl // P      # row tiles of qT/kT
    hpc = dl // BS     # heads per core
    nt = n // P        # query tiles (2 blocks each)
    nkc = nt + 1       # padded key chunks (128 keys each, shifted by -BS)
    ndc2 = dl // P     # contraction chunks for out proj

    nc = bacc.Bacc(None, target_bir_lowering=False, debug=False)

    x_d = nc.dram_tensor("x", [n, dmodel], bf16, kind="ExternalInput")
    wq_d = nc.dram_tensor("wqT", [dmodel, dl], bf16, kind="ExternalInput")
    wk_d = nc.dram_tensor("wkT", [dmodel, dl], bf16, kind="ExternalInput")
    wv_d = nc.dram_tensor("wvT", [dmodel, dl], bf16, kind="ExternalInput")
    wo_d = nc.dram_tensor("woT", [dl, dmodel], bf16, kind="ExternalInput")
    bq_d = nc.dram_tensor("bq", [dl], f32, kind="ExternalInput")
    bk_d = nc.dram_tensor("bk", [dl], f32, kind="ExternalInput")
    qm_d = nc.dram_tensor("qmask", [4, n], bf16, kind="ExternalInput")
    km_d = nc.dram_tensor("kmask", [4, n + 2 * BS], bf16, kind="ExternalInput")
    out_d = nc.dram_tensor("outT", [dmodel, n], f32, kind="ExternalOutput")
    if dbg:
        qTo_d = nc.dram_tensor("qTo", [dl, n], bf16, kind="ExternalOutput")
        kTo_d = nc.dram_tensor("kTo", [dl, n], bf16, kind="ExternalOutput")
        vo_d = nc.dram_tensor("vo", [n, dl], bf16, kind="ExternalOutput")
        ctxo_d = nc.dram_tensor("ctxo", [P, n // P, dl], bf16, kind="ExternalOutput")

    with tile.TileContext(nc) as tc, ExitStack() as top:
        dram = top.enter_context(tc.tile_pool(name="dram", bufs=1, space="DRAM"))
        qT_d = dram.tile([dl, n], bf16)
        kT_d = dram.tile([dl, n], bf16)
        v_d = dram.tile([n, dl], bf16)

        const = top.enter_context(tc.tile_pool(name="const", bufs=1))
        ident = const.tile([P, P], bf16)
        make_identity(nc, ident)

        # ctx natural accumulator: [q mod 128, tile, head*64+dh], SBUF-resident
        ctx_pool = top.enter_context(tc.tile_pool(name="ctx", bufs=1))
        ctx_nat = ctx_pool.tile([P, nt, dl], bf16)

        # pass-B per-head slots (manual ping-pong).  Allocated at top level so
        # their memory is disjoint from the pass-A pools: the constant regions
        # (mask rows, ones columns) are written once, up front.
        slot = top.enter_context(tc.tile_pool(name="slot", bufs=1))
        qz_s = [slot.tile([68, n], bf16, tag=f"qz{i}", name=f"qz{i}") for i in range(2)]
        kp_s = [slot.tile([68, n + 2 * BS], bf16, tag=f"kp{i}", name=f"kp{i}") for i in range(2)]
        va_s = [slot.tile([P, nkc, BS + 1], bf16, tag=f"va{i}", name=f"va{i}") for i in range(2)]
        kg_s = [slot.tile([68, P], bf16, tag=f"kg{i}", name=f"kg{i}") for i in range(2)]
        vg_s = [slot.tile([P, BS + 1], bf16, tag=f"vg{i}", name=f"vg{i}") for i in range(2)]
        qg_s = [slot.tile([68, P], bf16, tag=f"qg{i}", name=f"qg{i}") for i in range(2)]
        for qz in qz_s:
            nc.sync.dma_start(qz[64:68, :], qm_d[:, :])
        for kp in kp_s:
            nc.sync.dma_start(kp[64:68, :], km_d[:, :])
        for kg in kg_s:
            nc.gpsimd.memset(kg[64:68, :], 0.0)
        for qg in qg_s:
            nc.gpsimd.memset(qg[64:68, :], 0.0)
        for va in va_s:
            nc.gpsimd.memset(va[:, :, BS : BS + 1], 1.0)
        for vg in vg_s:
            nc.gpsimd.memset(vg[:, BS : BS + 1], 1.0)

        # ---------------- pass A: projections ----------------
        with ExitStack() as ps:
            wpool = ps.enter_context(tc.tile_pool(name="wpool", bufs=1))
            wq_sb = wpool.tile([P, ndc, dl], bf16)
            wk_sb = wpool.tile([P, ndc, dl], bf16)
            wv_sb = wpool.tile([P, ndc, dl], bf16)
            nc.sync.dma_start(wq_sb, wq_d.rearrange("(a p) j -> p a j", p=P))
            nc.sync.dma_start(wk_sb, wk_d.rearrange("(a p) j -> p a j", p=P))
            nc.sync.dma_start(wv_sb, wv_d.rearrange("(a p) j -> p a j", p=P))
            bq_sb = wpool.tile([P, njt], f32)
            bk_sb = wpool.tile([P, njt], f32)
            nc.sync.dma_start(bq_sb, bq_d.rearrange("(a p) -> p a", p=P))
            nc.sync.dma_start(bk_sb, bk_d.rearrange("(a p) -> p a", p=P))

            psA = ps.enter_context(tc.tile_pool(name="psA", bufs=4, space="PSUM"))
            xtpool = ps.enter_context(tc.tile_pool(name="xtpool", bufs=3))
            aout = ps.enter_context(tc.tile_pool(name="aout", bufs=4))

            for ch in range(nch):
                n0 = ch * 512
                xT = xtpool.tile([P, ndc, 512], bf16, tag="xT")
                for dc in range(ndc):
                    nc.sync.dma_start(
                        xT[:, dc, :],
                        x_d[n0 : n0 + 512, dc * P : (dc + 1) * P],
                        transpose=True,
                    )
                # qT / kT (transposed outputs, bias per-partition)
                for w_sb, b_sb, dst in ((wq_sb, bq_sb, qT_d), (wk_sb, bk_sb, kT_d)):
                    for jt in range(njt):
                        pp = psA.tile([P, 512], f32, tag="ps_a")
                        for dc in range(ndc):
                            nc.tensor.matmul(
                                pp,
                                w_sb[:, dc, jt * P : (jt + 1) * P],
                                xT[:, dc, :],
                                start=(dc == 0),
                                stop=(dc == ndc - 1),
                            )
                        ot = aout.tile([P, 512], bf16, tag="aout")
                        nc.scalar.activation(
                            ot, pp, AF.Identity, bias=b_sb[:, jt : jt + 1]
                        )
                        nc.sync.dma_start(
                            dst[jt * P : (jt + 1) * P, n0 : n0 + 512], ot
                        )
                # v (natural layout, no bias -- folded to host)
                for ns in range(4):
                    pp = psA.tile([P, dl], f32, tag="ps_a")
                    for dc in range(ndc):
                        nc.tensor.matmul(
                            pp,
                            xT[:, dc, ns * P : (ns + 1) * P],
                            wv_sb[:, dc, :],
                            start=(dc == 0),
                            stop=(dc == ndc - 1),
                        )
                    ot = aout.tile([P, dl], bf16, tag="aout_v")
                    nc.scalar.copy(ot, pp)
                    nc.sync.dma_start(v_d[n0 + ns * P : n0 + (ns + 1) * P, :], ot)

        # ---------------- pass B: attention ----------------
        with ExitStack() as ps:
            apool = ps.enter_context(tc.tile_pool(name="apool", bufs=4))
            agp = ps.enter_context(tc.tile_pool(name="agp", bufs=2))
            agr = ps.enter_context(tc.tile_pool(name="agr", bufs=2))
            stat = ps.enter_context(tc.tile_pool(name="stat", bufs=4))
            tgp = ps.enter_context(tc.tile_pool(name="tgp", bufs=4))
            psS = ps.enter_context(tc.tile_pool(name="psS", bufs=2, space="PSUM"))
            psG = ps.enter_context(tc.tile_pool(name="psG", bufs=1, space="PSUM"))
            psC = ps.enter_context(tc.tile_pool(name="psC", bufs=2, space="PSUM"))
            psR = ps.enter_context(tc.tile_pool(name="psR", bufs=1, space="PSUM"))
            psQ = ps.enter_context(tc.tile_pool(name="psQ", bufs=1, space="PSUM"))

            p0s = (g0 % 2) * BS
            p1s = (g1 % 2) * BS

            for h in range(hpc):
                r0 = h * BS
                qz, kp, va = qz_s[h % 2], kp_s[h % 2], va_s[h % 2]
                kg, vg, qg = kg_s[h % 2], vg_s[h % 2], qg_s[h % 2]

                # -- per-head DMAs (overlap previous head's compute) --
                nc.sync.dma_start(qz[0:64, :], qT_d[r0 : r0 + BS, :])
                nc.sync.dma_start(kp[0:64, BS : BS + n], kT_d[r0 : r0 + BS, :])
                nc.sync.dma_start(kp[0:64, 0:BS], kT_d[r0 : r0 + BS, n - BS : n])
                nc.sync.dma_start(kp[0:64, BS + n :], kT_d[r0 : r0 + BS, 0:BS])
                vs = v_d[:, r0 : r0 + BS]
                nc.sync.dma_start(va[0:BS, 0, 0:BS], vs[n - BS : n, :])
                nc.sync.dma_start(va[BS:P, 0, 0:BS], vs[0:BS, :])
                nc.sync.dma_start(
                    va[:, 1 : nkc - 1, 0:BS],
                    vs[BS : n - BS, :].rearrange("(a p) c -> p a c", p=P),
                )
                nc.sync.dma_start(va[0:BS, nkc - 1, 0:BS], vs[n - BS : n, :])
                nc.sync.dma_start(va[BS:P, nkc - 1, 0:BS], vs[0:BS, :])
                for gi, gv in enumerate((g0, g1)):
                    nc.sync.dma_start(
                        kg[0:64, gi * BS : (gi + 1) * BS],
                        kT_d[r0 : r0 + BS, gv * BS : (gv + 1) * BS],
                    )
                    nc.sync.dma_start(
                        vg[gi * BS : (gi + 1) * BS, 0:BS],
                        vs[gv * BS : (gv + 1) * BS, :],
                    )
                    nc.sync.dma_start(
                        qg[0:64, gi * BS : (gi + 1) * BS],
                        qT_d[r0 : r0 + BS, gv * BS : (gv + 1) * BS],
                    )

                # -- local + global-col scores (S^T layout), exp, AV --
                def sc_chunk(c, tag="a"):
                    """scores+exp for padded key chunk c; returns (a_sb, off0)"""
                    lo = max(0, (c - 1)) * P
                    hi = min(nt, c + 1) * P
                    nw = hi - lo
                    sps = psS.tile([P, 256], f32, tag="sps")
                    nc.tensor.matmul(
                        sps[:, 0:nw],
                        kp[:, c * P : (c + 1) * P],
                        qz[:, lo:hi],
                        start=True,
                        stop=True,
                    )
                    a_sb = apool.tile([P, 256], bf16, tag=tag)
                    nc.scalar.activation(a_sb[:, 0:nw], sps[:, 0:nw], AF.Exp)
                    return a_sb

                def gc_group(j):
                    spg = psG.tile([P, 512], f32, tag="spg")
                    nc.tensor.matmul(
                        spg, kg, qz[:, j * 512 : (j + 1) * 512], start=True, stop=True
                    )
                    ag = agp.tile([P, 512], bf16, tag="ag")
                    nc.scalar.activation(ag, spg, AF.Exp)
                    return ag

                ag_cur = gc_group(0)
                a_buf = [sc_chunk(0), sc_chunk(1), sc_chunk(2)]
                for t in range(nt):
                    if t % 4 == 0 and t > 0:
                        ag_cur = gc_group(t // 4)
                    if t + 3 <= nt:
                        a_buf.append(sc_chunk(t + 3))
                    a_lo = a_buf[t]
                    off = 0 if t == 0 else P
                    a_up = a_buf[t + 1]
                    cps = psC.tile([P, 130], f32, tag="cps")
                    nc.tensor.matmul(
                        cps[:, 0:65],
                        a_lo[:, off : off + P],
                        va[:, t, :],
                        start=True,
                        stop=False,
                    )
                    nc.tensor.matmul(
                        cps[:, 0:65],
                        a_up[:, 0:P],
                        va[:, t + 1, :],
                        start=False,
                        stop=True,
                    )
                    nc.tensor.matmul(
                        cps[:, 65:130],
                        ag_cur[:, (t % 4) * P : (t % 4 + 1) * P],
                        vg,
                        start=True,
                        stop=True,
                    )
                    a_buf[t] = None  # release
                    # normalization: per-partition (per-query) reciprocals
                    r2 = stat.tile([P, 2], f32, tag="r2")
                    nc.vector.reciprocal(r2[:, 0:1], cps[:, 64:65])
                    nc.vector.reciprocal(r2[:, 1:2], cps[:, 129:130])
                    tg = tgp.tile([P, BS], f32, tag="tg")
                    nc.scalar.activation(
                        tg, cps[:, 65:129], AF.Copy, scale=r2[:, 1:2]
                    )
                    nc.vector.scalar_tensor_tensor(
                        ctx_nat[:, t, r0 : r0 + BS],
                        cps[:, 0:64],
                        r2[:, 0:1],
                        tg,
                        OP.mult,
                        OP.add,
                    )

                # -- global rows: full attention for the 2 global q blocks --
                cpr0 = psQ.tile([P, 65], f32, tag="cpr0")
                cpr1 = psQ.tile([P, 65], f32, tag="cpr1")
                for j in range(8):
                    spr = psR.tile([P, 4, P], f32, tag="spr")
                    for i in range(4):
                        c = 1 + 4 * j + i
                        nc.tensor.matmul(
                            spr[:, i, :],
                            kp[:, c * P : (c + 1) * P],
                            qg,
                            start=True,
                            stop=True,
                        )
                    ar = agr.tile([P, 4, P], bf16, tag="ar")
                    nc.scalar.activation(ar, spr, AF.Exp)
                    for i in range(4):
                        c = 1 + 4 * j + i
                        nc.tensor.matmul(
                            cpr0[p0s : p0s + BS, :],
                            ar[:, i, 0:BS],
                            va[:, c, :],
                            start=(c == 1),
                            stop=(c == nkc - 1),
                        )
                        nc.tensor.matmul(
                            cpr1[p1s : p1s + BS, :],
                            ar[:, i, BS:P],
                            va[:, c, :],
                            start=(c == 1),
                            stop=(c == nkc - 1),
                        )
                for gi, gv, pb, cpr in ((0, g0, p0s, cpr0), (1, g1, p1s, cpr1)):
                    rg = stat.tile([P, 1], f32, tag=f"rg{gi}")
                    nc.vector.reciprocal(
                        rg[pb : pb + BS, :], cpr[pb : pb + BS, 64:65]
                    )
                    nc.vector.tensor_scalar_mul(
                        ctx_nat[pb : pb + BS, gv // 2, r0 : r0 + BS],
                        cpr[pb : pb + BS, 0:64],
                        rg[pb : pb + BS, :],
                    )

        # ---------------- pass C: output projection ----------------
        with ExitStack() as ps:
            wop = ps.enter_context(tc.tile_pool(name="wop", bufs=1))
            wo_sb = wop.tile([P, ndc2, dmodel], bf16)
            nc.sync.dma_start(wo_sb, wo_d.rearrange("(a p) o -> p a o", p=P))
            ctp = ps.enter_context(tc.tile_pool(name="ctp", bufs=2))
            copool = ps.enter_context(tc.tile_pool(name="co", bufs=4))
            psT = ps.enter_context(tc.tile_pool(name="psT", bufs=2, space="PSUM"))
            psO = ps.enter_context(tc.tile_pool(name="psO", bufs=4, space="PSUM"))
            for ncc in range(nch):
                ctxT = ctp.tile([P, ndc2, 512], bf16, tag="ctxT")
                for ti in range(4):
                    t = ncc * 4 + ti
                    for dc in range(ndc2):
                        tp = psT.tile([P, P], bf16, tag="tp")
                        nc.tensor.transpose(
                            tp, ctx_nat[:, t, dc * P : (dc + 1) * P], ident
                        )
                        nc.vector.tensor_copy(
                            ctxT[:, dc, ti * P : (ti + 1) * P], tp
                        )
                for ot in range(dmodel // P):
                    pp = psO.tile([P, 512], f32, tag="pso")
                    for dc in range(ndc2):
                        nc.tensor.matmul(
                            pp,
                            wo_sb[:, dc, ot * P : (ot + 1) * P],
                            ctxT[:, dc, :],
                            start=(dc == 0),
                            stop=(dc == ndc2 - 1),
                        )
                    ob = copool.tile([P, 512], f32, tag="ob")
                    if ot % 2 == 0:
                        nc.scalar.copy(ob, pp)
                    else:
                        nc.vector.tensor_copy(ob, pp)
                    nc.sync.dma_start(
                        out_d[ot * P : (ot + 1) * P, ncc * 512 : (ncc + 1) * 512], ob
                    )

        if dbg:
            nc.sync.dma_start(qTo_d[:, :], qT_d)
            nc.sync.dma_start(kTo_d[:, :], kT_d)
            nc.sync.dma_start(vo_d[:, :], v_d)
            nc.sync.dma_start(ctxo_d[:, :, :], ctx_nat)

    nc.finalize()
    return nc


@functools.lru_cache(maxsize=8)
def _get(n, dmodel, dl, g0, g1):
    return _build(n, dmodel, dl, g0, g1)


def _prepare(inputs):
    """Build (nc, in_maps, meta) for the SPMD run from full unsharded inputs."""
    x = np.asarray(inputs["x"], np.float32)
    Wq = np.asarray(inputs["Wq"], np.float32)
    Wk = np.asarray(inputs["Wk"], np.float32)
    Wv = np.asarray(inputs["Wv"], np.float32)
    Wo = np.asarray(inputs["Wo"], np.float32)
    bq = np.asarray(inputs["bq"], np.float32)
    bk = np.asarray(inputs["bk"], np.float32)
    bv = np.asarray(inputs["bv"], np.float32)
    bo = np.asarray(inputs["bo"], np.float32)
    gi = np.asarray(inputs["global_indices"]).astype(np.int64)
    g0, g1 = int(gi[0]), int(gi[1])

    b_, n_, d_ = x.shape
    dl = d_ // 2
    scale = 1.0 / np.sqrt(np.float32(64.0)).astype(np.float32)

    nc = _get(n_, d_, dl, g0, g1)

    import ml_dtypes

    bf = ml_dtypes.bfloat16
    # mask pattern rows (periodic in the column index, see pass B docstring)
    NEGf = np.float32(-1e9)
    j = np.arange(n_) % 256
    qmask = np.zeros((4, n_), np.float32)
    qmask[0, (j >= 128) & (j < 192)] = 1.0  # w1e
    qmask[1, (j >= 64) & (j < 128)] = 1.0   # w2e
    qmask[2, j < 64] = 1.0                  # w1o
    qmask[3, j >= 192] = 1.0                # w2o
    qmask = np.ascontiguousarray(qmask).astype(bf)
    jk = np.arange(n_ + 128) % 256
    kmask = np.zeros((4, n_ + 128), np.float32)
    kmask[0, (jk >= 64) & (jk < 128)] = NEGf   # u1e
    kmask[1, jk < 64] = NEGf                   # u2e
    kmask[2, jk >= 192] = NEGf                 # u1o
    kmask[3, (jk >= 128) & (jk < 192)] = NEGf  # u2o
    kmask = np.ascontiguousarray(kmask).astype(bf)
    in_maps = []
    for c in range(8):
        b, hg = divmod(c, 2)
        S = slice(hg * dl, (hg + 1) * dl)
        in_maps.append(
            {
                "x": np.ascontiguousarray(x[b]).astype(bf),
                "qmask": qmask,
                "kmask": kmask,
                "wqT": np.ascontiguousarray((Wq[S, :] * scale).T).astype(bf),
                "wkT": np.ascontiguousarray(Wk[S, :].T).astype(bf),
                "wvT": np.ascontiguousarray(Wv[S, :].T).astype(bf),
                "woT": np.ascontiguousarray(Wo[:, S].T).astype(bf),
                "bq": np.ascontiguousarray(bq[S] * scale),
                "bk": np.ascontiguousarray(bk[S]),
            }
        )

    # host-side bv correction: out[q] += c(q) * bv @ Wo.T, c(q)=1 on global
    # blocks (overwritten by full-attention rows), else 2.
    bvWo = bv @ Wo.T  # [d_model]
    coef = np.full((n_, 1), 2.0, np.float32)
    bs = 64
    coef[g0 * bs : (g0 + 1) * bs] = 1.0
    coef[g1 * bs : (g1 + 1) * bs] = 1.0
    corr = (coef * bvWo[None, :] + bo[None, :]).astype(np.float32)

    return nc, in_maps, (b_, n_, d_, corr)


def _combine(res, meta):
    b_, n_, d_, corr = meta
    out = np.empty((b_, n_, d_), np.float32)
    for b in range(b_):
        out[b] = res[2 * b]["outT"].T + res[2 * b + 1]["outT"].T + corr
    return out


def kernel(**inputs):
    _ensure_path()
    from concourse.bass_utils import run_bass_kernel_spmd

    nc, in_maps, meta = _prepare(inputs)
    res = run_bass_kernel_spmd(nc, in_maps, list(range(8))).results
    return _combine(res, meta)


# revision 14
# speedup vs baseline: 1.7136x; 1.2772x over previous
"""BigBird attention (B=4, N=4096, D=1024, H=16, BS=64) on 8 TRN2 NeuronCores.

Sharding: batch (4-way) x head-group (2-way).  Core c handles batch c//2 and
heads [hg*8, hg*8+8) where hg = c%2 (d_model slice [hg*512, hg*512+512)).

Per core:
  pass A: QKV projections.  x.T tiles produced with DMA transposes; q/k
          emitted transposed (qT/kT: [dl, n], head dim on partitions), v
          natural.  score scale folded into Wq/bq on the host; bv dropped
          entirely (attention is affine in v: host adds c(q)*bv@Wo.T).
  pass B: per-head BigBird attention, all scores computed transposed
          (S^T = K_chunk^T Q, keys on partitions) so probabilities feed the
          AV matmuls directly as stationary operands -- no PE transposes.
          The sliding-window mask is folded into 4 extra contraction rows
          (rank-2 outer product of periodic 0/1 q-patterns and -1e9
          k-patterns), so exp() yields exact zeros in the masked corners.
          No max subtraction (scores bounded ~|3|).  V carries a ones
          column so each AV matmul also emits the softmax denominator
          per-partition; normalization is a per-partition reciprocal.
  pass C: transpose ctx with the PE, then row-parallel output projection
          -> partial outT [d_model, n] (f32).
Host combines: out[b] = outT(core 2b).T + outT(core 2b+1).T + bo + c(q)*bv@Wo.T
with c(q) = 1 for rows in global blocks else 2.

The kernel is specialized (compiled) per global_indices value.
"""

import functools
import sys

import numpy as np

P = 128
BS = 64
NEG = -1e9


def _ensure_path():
    try:
        import concourse.bass  # noqa: F401
    except ImportError:
        sys.path.insert(0, "/opt/trn_rl_repo")


def _build(n, dmodel, dl, g0, g1, dbg=0):
    """Build the per-core Bass program.

    n: sequence length per core, dmodel: model dim, dl: local head dims =
    hpc*64.  g0, g1: global block indices (compile-time constants).
    """
    _ensure_path()
    from contextlib import ExitStack

    import concourse.bass as bass  # noqa: F401
    import concourse.tile as tile
    from concourse import bacc, mybir
    from concourse.masks import make_identity

    f32 = mybir.dt.float32
    bf16 = mybir.dt.bfloat16
    AF = mybir.ActivationFunctionType
    OP = mybir.AluOpType

    nch = n // 512     # 512-column chunks of the sequence
    ndc = dmodel // P  # contraction chunks for QKV proj
    njt = dl // P      # row tiles of qT/kT
    hpc = dl // BS     # heads per core
    nt = n // P        # query tiles (2 blocks each)
    nkc = nt + 1       # padded key chunks (128 keys each, shifted by -BS)
    ndc2 = dl // P     # contraction chunks for out proj

    nc = bacc.Bacc(None, target_bir_lowering=False, debug=False)

    x_d = nc.dram_tensor("x", [n, dmodel], bf16, kind="ExternalInput")
    wq_d = nc.dram_tensor("wqT", [dmodel, dl], bf16, kind="ExternalInput")
    wk_d = nc.dram_tensor("wkT", [dmodel, dl], bf16, kind="ExternalInput")
    wv_d = nc.dram_tensor("wvT", [dmodel, dl], bf16, kind="ExternalInput")
    wo_d = nc.dram_tensor("woT", [dl, dmodel], bf16, kind="ExternalInput")
    bq_d = nc.dram_tensor("bq", [dl], f32, kind="ExternalInput")
    bk_d = nc.dram_tensor("bk", [dl], f32, kind="ExternalInput")
    qm_d = nc.dram_tensor("qmask", [4, n], bf16, kind="ExternalInput")
    km_d = nc.dram_tensor("kmask", [4, n + 2 * BS], bf16, kind="ExternalInput")
    out_d = nc.dram_tensor("outT", [dmodel, n], f32, kind="ExternalOutput")
    if dbg:
        qTo_d = nc.dram_tensor("qTo", [dl, n], bf16, kind="ExternalOutput")
        kTo_d = nc.dram_tensor("kTo", [dl, n], bf16, kind="ExternalOutput")
        vo_d = nc.dram_tensor("vo", [n, dl], bf16, kind="ExternalOutput")
        ctxo_d = nc.dram_tensor("ctxo", [P, n // P, dl], bf16, kind="ExternalOutput")

    with tile.TileContext(nc) as tc, ExitStack() as top:
        dram = top.enter_context(tc.tile_pool(name="dram", bufs=1, space="DRAM"))
        qT_d = dram.tile([dl, n], bf16)
        kT_d = dram.tile([dl, n], bf16)
        v_d = dram.tile([n, dl], bf16)

        const = top.enter_context(tc.tile_pool(name="const", bufs=1))
        ident = const.tile([P, P], bf16)
        make_identity(nc, ident)

        # ctx natural accumulator: [q mod 128, tile, head*64+dh], SBUF-resident
        ctx_pool = top.enter_context(tc.tile_pool(name="ctx", bufs=1))
        ctx_nat = ctx_pool.tile([P, nt, dl], bf16)

        # pass-B per-head slots (manual ping-pong).  Allocated at top level so
        # their memory is disjoint from the pass-A pools: the constant regions
        # (mask rows, ones columns) are written once, up front.
        slot = top.enter_context(tc.tile_pool(name="slot", bufs=1))
        qz_s = [slot.tile([68, n], bf16, tag=f"qz{i}", name=f"qz{i}") for i in range(2)]
        kp_s = [slot.tile([68, n + 2 * BS], bf16, tag=f"kp{i}", name=f"kp{i}") for i in range(2)]
        va_s = [slot.tile([P, nkc, BS + 1], bf16, tag=f"va{i}", name=f"va{i}") for i in range(2)]
        kg_s = [slot.tile([68, P], bf16, tag=f"kg{i}", name=f"kg{i}") for i in range(2)]
        vg_s = [slot.tile([P, BS + 1], bf16, tag=f"vg{i}", name=f"vg{i}") for i in range(2)]
        qg_s = [slot.tile([68, P], bf16, tag=f"qg{i}", name=f"qg{i}") for i in range(2)]
        for qz in qz_s:
            nc.sync.dma_start(qz[64:68, :], qm_d[:, :])
        for kp in kp_s:
            nc.sync.dma_start(kp[64:68, :], km_d[:, :])
        for kg in kg_s:
            nc.gpsimd.memset(kg[64:68, :], 0.0)
        for qg in qg_s:
            nc.gpsimd.memset(qg[64:68, :], 0.0)
        for va in va_s:
            nc.gpsimd.memset(va[:, :, BS : BS + 1], 1.0)
        for vg in vg_s:
            nc.gpsimd.memset(vg[:, BS : BS + 1], 1.0)

        # ---------------- pass A: projections ----------------
        with ExitStack() as ps:
            wpool = ps.enter_context(tc.tile_pool(name="wpool", bufs=1))
            wq_sb = wpool.tile([P, ndc, dl], bf16)
            wk_sb = wpool.tile([P, ndc, dl], bf16)
            wv_sb = wpool.tile([P, ndc, dl], bf16)
            nc.sync.dma_start(wq_sb, wq_d.rearrange("(a p) j -> p a j", p=P))
            nc.sync.dma_start(wk_sb, wk_d.rearrange("(a p) j -> p a j", p=P))
            nc.sync.dma_start(wv_sb, wv_d.rearrange("(a p) j -> p a j", p=P))
            bq_sb = wpool.tile([P, njt], f32)
            bk_sb = wpool.tile([P, njt], f32)
            nc.sync.dma_start(bq_sb, bq_d.rearrange("(a p) -> p a", p=P))
            nc.sync.dma_start(bk_sb, bk_d.rearrange("(a p) -> p a", p=P))

            psA = ps.enter_context(tc.tile_pool(name="psA", bufs=4, space="PSUM"))
            xtpool = ps.enter_context(tc.tile_pool(name="xtpool", bufs=3))
            aout = ps.enter_context(tc.tile_pool(name="aout", bufs=4))

            for ch in range(nch):
                n0 = ch * 512
                xT = xtpool.tile([P, ndc, 512], bf16, tag="xT")
                for dc in range(ndc):
                    nc.sync.dma_start(
                        xT[:, dc, :],
                        x_d[n0 : n0 + 512, dc * P : (dc + 1) * P],
                        transpose=True,
                    )
                # qT / kT (transposed outputs, bias per-partition)
                for w_sb, b_sb, dst in ((wq_sb, bq_sb, qT_d), (wk_sb, bk_sb, kT_d)):
                    for jt in range(njt):
                        pp = psA.tile([P, 512], f32, tag="ps_a")
                        for dc in range(ndc):
                            nc.tensor.matmul(
                                pp,
                                w_sb[:, dc, jt * P : (jt + 1) * P],
                                xT[:, dc, :],
                                start=(dc == 0),
                                stop=(dc == ndc - 1),
                            )
                        ot = aout.tile([P, 512], bf16, tag="aout")
                        nc.scalar.activation(
                            ot, pp, AF.Identity, bias=b_sb[:, jt : jt + 1]
                        )
                        nc.scalar.dma_start(
                            dst[jt * P : (jt + 1) * P, n0 : n0 + 512], ot
                        )
                # v (natural layout, no bias -- folded to host)
                for ns in range(4):
                    pp = psA.tile([P, dl], f32, tag="ps_a")
                    for dc in range(ndc):
                        nc.tensor.matmul(
                            pp,
                            xT[:, dc, ns * P : (ns + 1) * P],
                            wv_sb[:, dc, :],
                            start=(dc == 0),
                            stop=(dc == ndc - 1),
                        )
                    ot = aout.tile([P, dl], bf16, tag="aout_v")
                    nc.scalar.copy(ot, pp)
                    nc.scalar.dma_start(v_d[n0 + ns * P : n0 + (ns + 1) * P, :], ot)

        # ---------------- pass B: attention ----------------
        with ExitStack() as ps:
            apool = ps.enter_context(tc.tile_pool(name="apool", bufs=4))
            agp = ps.enter_context(tc.tile_pool(name="agp", bufs=2))
            agr = ps.enter_context(tc.tile_pool(name="agr", bufs=2))
            stat = ps.enter_context(tc.tile_pool(name="stat", bufs=4))
            tgp = ps.enter_context(tc.tile_pool(name="tgp", bufs=4))
            psS = ps.enter_context(tc.tile_pool(name="psS", bufs=2, space="PSUM"))
            psG = ps.enter_context(tc.tile_pool(name="psG", bufs=1, space="PSUM"))
            psC = ps.enter_context(tc.tile_pool(name="psC", bufs=2, space="PSUM"))
            psR = ps.enter_context(tc.tile_pool(name="psR", bufs=1, space="PSUM"))
            psQ = ps.enter_context(tc.tile_pool(name="psQ", bufs=1, space="PSUM"))

            p0s = (g0 % 2) * BS
            p1s = (g1 % 2) * BS

            for h in range(hpc):
                r0 = h * BS
                qz, kp, va = qz_s[h % 2], kp_s[h % 2], va_s[h % 2]
                kg, vg, qg = kg_s[h % 2], vg_s[h % 2], qg_s[h % 2]

                # -- per-head DMAs (overlap previous head's compute) --
                nc.sync.dma_start(qz[0:64, :], qT_d[r0 : r0 + BS, :])
                nc.sync.dma_start(kp[0:64, BS : BS + n], kT_d[r0 : r0 + BS, :])
                nc.sync.dma_start(kp[0:64, 0:BS], kT_d[r0 : r0 + BS, n - BS : n])
                nc.sync.dma_start(kp[0:64, BS + n :], kT_d[r0 : r0 + BS, 0:BS])
                vs = v_d[:, r0 : r0 + BS]
                nc.sync.dma_start(va[0:BS, 0, 0:BS], vs[n - BS : n, :])
                nc.sync.dma_start(va[BS:P, 0, 0:BS], vs[0:BS, :])
                nc.sync.dma_start(
                    va[:, 1 : nkc - 1, 0:BS],
                    vs[BS : n - BS, :].rearrange("(a p) c -> p a c", p=P),
                )
                nc.sync.dma_start(va[0:BS, nkc - 1, 0:BS], vs[n - BS : n, :])
                nc.sync.dma_start(va[BS:P, nkc - 1, 0:BS], vs[0:BS, :])
                for gi, gv in enumerate((g0, g1)):
                    nc.sync.dma_start(
                        kg[0:64, gi * BS : (gi + 1) * BS],
                        kT_d[r0 : r0 + BS, gv * BS : (gv + 1) * BS],
                    )
                    nc.sync.dma_start(
                        vg[gi * BS : (gi + 1) * BS, 0:BS],
                        vs[gv * BS : (gv + 1) * BS, :],
                    )
                    nc.sync.dma_start(
                        qg[0:64, gi * BS : (gi + 1) * BS],
                        qT_d[r0 : r0 + BS, gv * BS : (gv + 1) * BS],
                    )

                # -- local + global-col scores (S^T layout), exp, AV --
                def sc_chunk(c, tag="a"):
                    """scores+exp for padded key chunk c; returns (a_sb, off0)"""
                    lo = max(0, (c - 1)) * P
                    hi = min(nt, c + 1) * P
                    nw = hi - lo
                    sps = psS.tile([P, 256], f32, tag="sps")
                    nc.tensor.matmul(
                        sps[:, 0:nw],
                        kp[:, c * P : (c + 1) * P],
                        qz[:, lo:hi],
                        start=True,
                        stop=True,
                    )
                    a_sb = apool.tile([P, 256], bf16, tag=tag)
                    nc.scalar.activation(a_sb[:, 0:nw], sps[:, 0:nw], AF.Exp)
                    return a_sb

                def gc_group(j):
                    spg = psG.tile([P, 512], f32, tag="spg")
                    nc.tensor.matmul(
                        spg, kg, qz[:, j * 512 : (j + 1) * 512], start=True, stop=True
                    )
                    ag = agp.tile([P, 512], bf16, tag="ag")
                    nc.scalar.activation(ag, spg, AF.Exp)
                    return ag

                ag_cur = gc_group(0)
                a_buf = [sc_chunk(0), sc_chunk(1), sc_chunk(2)]
                for t in range(nt):
                    if t % 4 == 0 and t > 0:
                        ag_cur = gc_group(t // 4)
                    if t + 3 <= nt:
                        a_buf.append(sc_chunk(t + 3))
                    a_lo = a_buf[t]
                    off = 0 if t == 0 else P
                    a_up = a_buf[t + 1]
                    cps = psC.tile([P, 130], f32, tag="cps")
                    nc.tensor.matmul(
                        cps[:, 0:65],
                        a_lo[:, off : off + P],
                        va[:, t, :],
                        start=True,
                        stop=False,
                    )
                    nc.tensor.matmul(
                        cps[:, 0:65],
                        a_up[:, 0:P],
                        va[:, t + 1, :],
                        start=False,
                        stop=True,
                    )
                    nc.tensor.matmul(
                        cps[:, 65:130],
                        ag_cur[:, (t % 4) * P : (t % 4 + 1) * P],
                        vg,
                        start=True,
                        stop=True,
                    )
                    a_buf[t] = None  # release
                    # normalization: per-partition (per-query) reciprocals
                    r2 = stat.tile([P, 2], f32, tag="r2")
                    nc.vector.reciprocal(r2, cps[:, 64:130:65])
                    tg = tgp.tile([P, BS], f32, tag="tg")
                    nc.vector.tensor_scalar_mul(tg, cps[:, 65:129], r2[:, 1:2])
                    nc.vector.scalar_tensor_tensor(
                        ctx_nat[:, t, r0 : r0 + BS],
                        cps[:, 0:64],
                        r2[:, 0:1],
                        tg,
                        OP.mult,
                        OP.add,
                    )

                # -- global rows: full attention for the 2 global q blocks --
                cpr0 = psQ.tile([P, 65], f32, tag="cpr0")
                cpr1 = psQ.tile([P, 65], f32, tag="cpr1")
                for j in range(8):
                    spr = psR.tile([P, 4, P], f32, tag="spr")
                    for i in range(4):
                        c = 1 + 4 * j + i
                        nc.tensor.matmul(
                            spr[:, i, :],
                            kp[:, c * P : (c + 1) * P],
                            qg,
                            start=True,
                            stop=True,
                        )
                    ar = agr.tile([P, 4, P], bf16, tag="ar")
                    nc.scalar.activation(ar, spr, AF.Exp)
                    for i in range(4):
                        c = 1 + 4 * j + i
                        nc.tensor.matmul(
                            cpr0[p0s : p0s + BS, :],
                            ar[:, i, 0:BS],
                            va[:, c, :],
                            start=(c == 1),
                            stop=(c == nkc - 1),
                        )
                        nc.tensor.matmul(
                            cpr1[p1s : p1s + BS, :],
                            ar[:, i, BS:P],
                            va[:, c, :],
                            start=(c == 1),
                            stop=(c == nkc - 1),
                        )
                for gi, gv, pb, cpr in ((0, g0, p0s, cpr0), (1, g1, p1s, cpr1)):
                    rg = stat.tile([P, 1], f32, tag=f"rg{gi}")
                    nc.vector.reciprocal(
                        rg[pb : pb + BS, :], cpr[pb : pb + BS, 64:65]
                    )
                    nc.vector.tensor_scalar_mul(
                        ctx_nat[pb : pb + BS, gv // 2, r0 : r0 + BS],
                        cpr[pb : pb + BS, 0:64],
                        rg[pb : pb + BS, :],
                    )

        # ---------------- pass C: output projection ----------------
        with ExitStack() as ps:
            wop = ps.enter_context(tc.tile_pool(name="wop", bufs=1))
            wo_sb = wop.tile([P, ndc2, dmodel], bf16)
            nc.sync.dma_start(wo_sb, wo_d.rearrange("(a p) o -> p a o", p=P))
            ctp = ps.enter_context(tc.tile_pool(name="ctp", bufs=2))
            copool = ps.enter_context(tc.tile_pool(name="co", bufs=4))
            psT = ps.enter_context(tc.tile_pool(name="psT", bufs=2, space="PSUM"))
            psO = ps.enter_context(tc.tile_pool(name="psO", bufs=4, space="PSUM"))
            for ncc in range(nch):
                ctxT = ctp.tile([P, ndc2, 512], bf16, tag="ctxT")
                for ti in range(4):
                    t = ncc * 4 + ti
                    for dc in range(ndc2):
                        tp = psT.tile([P, P], bf16, tag="tp")
                        nc.tensor.transpose(
                            tp, ctx_nat[:, t, dc * P : (dc + 1) * P], ident
                        )
                        nc.scalar.copy(ctxT[:, dc, ti * P : (ti + 1) * P], tp)
                for ot in range(dmodel // P):
                    pp = psO.tile([P, 512], f32, tag="pso")
                    for dc in range(ndc2):
                        nc.tensor.matmul(
                            pp,
                            wo_sb[:, dc, ot * P : (ot + 1) * P],
                            ctxT[:, dc, :],
                            start=(dc == 0),
                            stop=(dc == ndc2 - 1),
                        )
                    ob = copool.tile([P, 512], f32, tag="ob")
                    nc.vector.tensor_copy(ob, pp)
                    nc.sync.dma_start(
                        out_d[ot * P : (ot + 1) * P, ncc * 512 : (ncc + 1) * 512], ob
                    )

        if dbg:
            nc.sync.dma_start(qTo_d[:, :], qT_d)
            nc.sync.dma_start(kTo_d[:, :], kT_d)
            nc.sync.dma_start(vo_d[:, :], v_d)
            nc.sync.dma_start(ctxo_d[:, :, :], ctx_nat)

    nc.finalize()
    return nc


@functools.lru_cache(maxsize=8)
def _get(n, dmodel, dl, g0, g1):
    return _build(n, dmodel, dl, g0, g1)


def _prepare(inputs):
    """Build (nc, in_maps, meta) for the SPMD run from full unsharded inputs."""
    x = np.asarray(inputs["x"], np.float32)
    Wq = np.asarray(inputs["Wq"], np.float32)
    Wk = np.asarray(inputs["Wk"], np.float32)
    Wv = np.asarray(inputs["Wv"], np.float32)
    Wo = np.asarray(inputs["Wo"], np.float32)
    bq = np.asarray(inputs["bq"], np.float32)
    bk = np.asarray(inputs["bk"], np.float32)
    bv = np.asarray(inputs["bv"], np.float32)
    bo = np.asarray(inputs["bo"], np.float32)
    gi = np.asarray(inputs["global_indices"]).astype(np.int64)
    g0, g1 = int(gi[0]), int(gi[1])

    b_, n_, d_ = x.shape
    dl = d_ // 2
    scale = 1.0 / np.sqrt(np.float32(64.0)).astype(np.float32)

    nc = _get(n_, d_, dl, g0, g1)

    import ml_dtypes

    bf = ml_dtypes.bfloat16
    # mask pattern rows (periodic in the column index, see pass B docstring)
    NEGf = np.float32(-1e9)
    j = np.arange(n_) % 256
    qmask = np.zeros((4, n_), np.float32)
    qmask[0, (j >= 128) & (j < 192)] = 1.0  # w1e
    qmask[1, (j >= 64) & (j < 128)] = 1.0   # w2e
    qmask[2, j < 64] = 1.0                  # w1o
    qmask[3, j >= 192] = 1.0                # w2o
    qmask = np.ascontiguousarray(qmask).astype(bf)
    jk = np.arange(n_ + 128) % 256
    kmask = np.zeros((4, n_ + 128), np.float32)
    kmask[0, (jk >= 64) & (jk < 128)] = NEGf   # u1e
    kmask[1, jk < 64] = NEGf                   # u2e
    kmask[2, jk >= 192] = NEGf                 # u1o
    kmask[3, (jk >= 128) & (jk < 192)] = NEGf  # u2o
    kmask = np.ascontiguousarray(kmask).astype(bf)
    in_maps = []
    for c in range(8):
        b, hg = divmod(c, 2)
        S = slice(hg * dl, (hg + 1) * dl)
        in_maps.append(
            {
                "x": np.ascontiguousarray(x[b]).astype(bf),
                "qmask": qmask,
                "kmask": kmask,
                "wqT": np.ascontiguousarray((Wq[S, :] * scale).T).astype(bf),
                "wkT": np.ascontiguousarray(Wk[S, :].T).astype(bf),
                "wvT": np.ascontiguousarray(Wv[S, :].T).astype(bf),
                "woT": np.ascontiguousarray(Wo[:, S].T).astype(bf),
                "bq": np.ascontiguousarray(bq[S] * scale),
                "bk": np.ascontiguousarray(bk[S]),
            }
        )

    # host-side bv correction: out[q] += c(q) * bv @ Wo.T, c(q)=1 on global
    # blocks (overwritten by full-attention rows), else 2.
    bvWo = bv @ Wo.T  # [d_model]
    coef = np.full((n_, 1), 2.0, np.float32)
    bs = 64
    coef[g0 * bs : (g0 + 1) * bs] = 1.0
    coef[g1 * bs : (g1 + 1) * bs] = 1.0
    corr = (coef * bvWo[None, :] + bo[None, :]).astype(np.float32)

    return nc, in_maps, (b_, n_, d_, corr)


def _combine(res, meta):
    b_, n_, d_, corr = meta
    out = np.empty((b_, n_, d_), np.float32)
    for b in range(b_):
        out[b] = res[2 * b]["outT"].T + res[2 * b + 1]["outT"].T + corr
    return out


def kernel(**inputs):
    _ensure_path()
    from concourse.bass_utils import run_bass_kernel_spmd

    nc, in_maps, meta = _prepare(inputs)
    res = run_bass_kernel_spmd(nc, in_maps, list(range(8))).results
    return _combine(res, meta)


# revision 15
# speedup vs baseline: 2.0442x; 1.1929x over previous
"""BigBird attention (B=4, N=4096, D=1024, H=16, BS=64) on 8 TRN2 NeuronCores.

Sharding: batch (4-way) x head-group (2-way).  Core c handles batch c//2 and
heads [hg*8, hg*8+8) where hg = c%2 (d_model slice [hg*512, hg*512+512)).

Per core:
  pass A: QKV projections.  x.T tiles produced with DMA transposes; q/k
          emitted transposed (qT/kT: [dl, n], head dim on partitions), v
          natural.  score scale folded into Wq/bq on the host; bv dropped
          entirely (attention is affine in v: host adds c(q)*bv@Wo.T).
  pass B: per-head BigBird attention, all scores computed transposed
          (S^T = K_chunk^T Q, keys on partitions) so probabilities feed the
          AV matmuls directly as stationary operands -- no PE transposes.
          The sliding-window mask is folded into 4 extra contraction rows
          (rank-2 outer product of periodic 0/1 q-patterns and -1e9
          k-patterns), so exp() yields exact zeros in the masked corners.
          No max subtraction (scores bounded ~|3|).  V carries a ones
          column so each AV matmul also emits the softmax denominator
          per-partition; normalization is a per-partition reciprocal.
  pass C: transpose ctx with the PE, then row-parallel output projection
          -> partial outT [d_model, n] (f32).
Host combines: out[b] = outT(core 2b).T + outT(core 2b+1).T + bo + c(q)*bv@Wo.T
with c(q) = 1 for rows in global blocks else 2.

The kernel is specialized (compiled) per global_indices value.
"""

import functools
import sys

import numpy as np

P = 128
BS = 64
NEG = -1e9


def _ensure_path():
    try:
        import concourse.bass  # noqa: F401
    except ImportError:
        sys.path.insert(0, "/opt/trn_rl_repo")


def _build(n, dmodel, dl, g0, g1, dbg=0):
    """Build the per-core Bass program.

    n: sequence length per core, dmodel: model dim, dl: local head dims =
    hpc*64.  g0, g1: global block indices (compile-time constants).
    """
    _ensure_path()
    from contextlib import ExitStack

    import concourse.bass as bass  # noqa: F401
    import concourse.tile as tile
    from concourse import bacc, mybir
    from concourse.masks import make_identity

    f32 = mybir.dt.float32
    bf16 = mybir.dt.bfloat16
    AF = mybir.ActivationFunctionType
    OP = mybir.AluOpType

    nch = n // 512     # 512-column chunks of the sequence
    ndc = dmodel // P  # contraction chunks for QKV proj
    njt = dl // P      # row tiles of qT/kT
    hpc = dl // BS     # heads per core
    nt = n // P        # query tiles (2 blocks each)
    nkc = nt + 1       # padded key chunks (128 keys each, shifted by -BS)
    ndc2 = dl // P     # contraction chunks for out proj

    nc = bacc.Bacc(None, target_bir_lowering=False, debug=False)

    xT_d = nc.dram_tensor("xT", [dmodel, n], bf16, kind="ExternalInput")
    wq_d = nc.dram_tensor("wqT", [dmodel, dl], bf16, kind="ExternalInput")
    wk_d = nc.dram_tensor("wkT", [dmodel, dl], bf16, kind="ExternalInput")
    wv_d = nc.dram_tensor("wvT", [dmodel, dl], bf16, kind="ExternalInput")
    wo_d = nc.dram_tensor("woT", [dl, dmodel], bf16, kind="ExternalInput")
    bq_d = nc.dram_tensor("bq", [dl], f32, kind="ExternalInput")
    bk_d = nc.dram_tensor("bk", [dl], f32, kind="ExternalInput")
    qm_d = nc.dram_tensor("qmask", [4, n], bf16, kind="ExternalInput")
    km_d = nc.dram_tensor("kmask", [4, n + 2 * BS], bf16, kind="ExternalInput")
    out_d = nc.dram_tensor("outT", [dmodel, n], f32, kind="ExternalOutput")
    if dbg:
        qTo_d = nc.dram_tensor("qTo", [dl, n], bf16, kind="ExternalOutput")
        kTo_d = nc.dram_tensor("kTo", [dl, n], bf16, kind="ExternalOutput")
        vo_d = nc.dram_tensor("vo", [n, dl], bf16, kind="ExternalOutput")
        ctxo_d = nc.dram_tensor("ctxo", [P, n // P, dl], bf16, kind="ExternalOutput")

    with tile.TileContext(nc) as tc, ExitStack() as top:
        dram = top.enter_context(tc.tile_pool(name="dram", bufs=1, space="DRAM"))
        qT_d = dram.tile([dl, n], bf16)
        kT_d = dram.tile([dl, n], bf16)
        v_d = dram.tile([n, dl], bf16)

        const = top.enter_context(tc.tile_pool(name="const", bufs=1))
        ident = const.tile([P, P], bf16)
        make_identity(nc, ident)

        # ctx natural accumulator: [q mod 128, tile, head*64+dh], SBUF-resident
        ctx_pool = top.enter_context(tc.tile_pool(name="ctx", bufs=1))
        ctx_nat = ctx_pool.tile([P, nt, dl], bf16)

        # pass-B per-head slots (manual ping-pong).  Allocated at top level so
        # their memory is disjoint from the pass-A pools: the constant regions
        # (mask rows, ones columns) are written once, up front.
        slot = top.enter_context(tc.tile_pool(name="slot", bufs=1))
        qz_s = [slot.tile([68, n], bf16, tag=f"qz{i}", name=f"qz{i}") for i in range(2)]
        kp_s = [slot.tile([68, n + 2 * BS], bf16, tag=f"kp{i}", name=f"kp{i}") for i in range(2)]
        va_s = [slot.tile([P, nkc, BS + 1], bf16, tag=f"va{i}", name=f"va{i}") for i in range(2)]
        kg_s = [slot.tile([68, P], bf16, tag=f"kg{i}", name=f"kg{i}") for i in range(2)]
        vg_s = [slot.tile([P, BS + 1], bf16, tag=f"vg{i}", name=f"vg{i}") for i in range(2)]
        qg_s = [slot.tile([68, P], bf16, tag=f"qg{i}", name=f"qg{i}") for i in range(2)]
        for qz in qz_s:
            nc.sync.dma_start(qz[64:68, :], qm_d[:, :])
        for kp in kp_s:
            nc.sync.dma_start(kp[64:68, :], km_d[:, :])
        for kg in kg_s:
            nc.gpsimd.memset(kg[64:68, :], 0.0)
        for qg in qg_s:
            nc.gpsimd.memset(qg[64:68, :], 0.0)
        for va in va_s:
            nc.gpsimd.memset(va[:, :, BS : BS + 1], 1.0)
        for vg in vg_s:
            nc.gpsimd.memset(vg[:, BS : BS + 1], 1.0)

        # ---------------- pass A: projections ----------------
        with ExitStack() as ps:
            wpool = ps.enter_context(tc.tile_pool(name="wpool", bufs=1))
            wq_sb = wpool.tile([P, ndc, dl], bf16)
            wk_sb = wpool.tile([P, ndc, dl], bf16)
            wv_sb = wpool.tile([P, ndc, dl], bf16)
            nc.sync.dma_start(wq_sb, wq_d.rearrange("(a p) j -> p a j", p=P))
            nc.sync.dma_start(wk_sb, wk_d.rearrange("(a p) j -> p a j", p=P))
            nc.sync.dma_start(wv_sb, wv_d.rearrange("(a p) j -> p a j", p=P))
            bq_sb = wpool.tile([P, njt], f32)
            bk_sb = wpool.tile([P, njt], f32)
            nc.sync.dma_start(bq_sb, bq_d.rearrange("(a p) -> p a", p=P))
            nc.sync.dma_start(bk_sb, bk_d.rearrange("(a p) -> p a", p=P))

            psA = ps.enter_context(tc.tile_pool(name="psA", bufs=4, space="PSUM"))
            xtpool = ps.enter_context(tc.tile_pool(name="xtpool", bufs=3))
            aout = ps.enter_context(tc.tile_pool(name="aout", bufs=4))

            for ch in range(nch):
                n0 = ch * 512
                xT = xtpool.tile([P, ndc, 512], bf16, tag="xT")
                for dc in range(ndc):
                    nc.sync.dma_start(
                        xT[:, dc, :], xT_d[dc * P : (dc + 1) * P, n0 : n0 + 512]
                    )
                # qT / kT (transposed outputs, bias per-partition)
                for w_sb, b_sb, dst in ((wq_sb, bq_sb, qT_d), (wk_sb, bk_sb, kT_d)):
                    for jt in range(njt):
                        pp = psA.tile([P, 512], f32, tag="ps_a")
                        for dc in range(ndc):
                            nc.tensor.matmul(
                                pp,
                                w_sb[:, dc, jt * P : (jt + 1) * P],
                                xT[:, dc, :],
                                start=(dc == 0),
                                stop=(dc == ndc - 1),
                            )
                        ot = aout.tile([P, 512], bf16, tag="aout")
                        nc.scalar.activation(
                            ot, pp, AF.Identity, bias=b_sb[:, jt : jt + 1]
                        )
                        nc.scalar.dma_start(
                            dst[jt * P : (jt + 1) * P, n0 : n0 + 512], ot
                        )
                # v (natural layout, no bias -- folded to host)
                for ns in range(4):
                    pp = psA.tile([P, dl], f32, tag="ps_a")
                    for dc in range(ndc):
                        nc.tensor.matmul(
                            pp,
                            xT[:, dc, ns * P : (ns + 1) * P],
                            wv_sb[:, dc, :],
                            start=(dc == 0),
                            stop=(dc == ndc - 1),
                        )
                    ot = aout.tile([P, dl], bf16, tag="aout_v")
                    nc.scalar.copy(ot, pp)
                    nc.scalar.dma_start(v_d[n0 + ns * P : n0 + (ns + 1) * P, :], ot)

        # ---------------- pass B: attention ----------------
        with ExitStack() as ps:
            apool = ps.enter_context(tc.tile_pool(name="apool", bufs=4))
            agp = ps.enter_context(tc.tile_pool(name="agp", bufs=2))
            agr = ps.enter_context(tc.tile_pool(name="agr", bufs=2))
            stat = ps.enter_context(tc.tile_pool(name="stat", bufs=4))
            tgp = ps.enter_context(tc.tile_pool(name="tgp", bufs=4))
            psS = ps.enter_context(tc.tile_pool(name="psS", bufs=2, space="PSUM"))
            psG = ps.enter_context(tc.tile_pool(name="psG", bufs=1, space="PSUM"))
            psC = ps.enter_context(tc.tile_pool(name="psC", bufs=2, space="PSUM"))
            psR = ps.enter_context(tc.tile_pool(name="psR", bufs=1, space="PSUM"))
            psQ = ps.enter_context(tc.tile_pool(name="psQ", bufs=1, space="PSUM"))

            p0s = (g0 % 2) * BS
            p1s = (g1 % 2) * BS

            for h in range(hpc):
                r0 = h * BS
                qz, kp, va = qz_s[h % 2], kp_s[h % 2], va_s[h % 2]
                kg, vg, qg = kg_s[h % 2], vg_s[h % 2], qg_s[h % 2]

                # -- per-head DMAs (overlap previous head's compute) --
                nc.sync.dma_start(qz[0:64, :], qT_d[r0 : r0 + BS, :])
                nc.sync.dma_start(kp[0:64, BS : BS + n], kT_d[r0 : r0 + BS, :])
                nc.sync.dma_start(kp[0:64, 0:BS], kT_d[r0 : r0 + BS, n - BS : n])
                nc.sync.dma_start(kp[0:64, BS + n :], kT_d[r0 : r0 + BS, 0:BS])
                vs = v_d[:, r0 : r0 + BS]
                nc.sync.dma_start(va[0:BS, 0, 0:BS], vs[n - BS : n, :])
                nc.sync.dma_start(va[BS:P, 0, 0:BS], vs[0:BS, :])
                nc.sync.dma_start(
                    va[:, 1 : nkc - 1, 0:BS],
                    vs[BS : n - BS, :].rearrange("(a p) c -> p a c", p=P),
                )
                nc.sync.dma_start(va[0:BS, nkc - 1, 0:BS], vs[n - BS : n, :])
                nc.sync.dma_start(va[BS:P, nkc - 1, 0:BS], vs[0:BS, :])
                for gi, gv in enumerate((g0, g1)):
                    nc.sync.dma_start(
                        kg[0:64, gi * BS : (gi + 1) * BS],
                        kT_d[r0 : r0 + BS, gv * BS : (gv + 1) * BS],
                    )
                    nc.sync.dma_start(
                        vg[gi * BS : (gi + 1) * BS, 0:BS],
                        vs[gv * BS : (gv + 1) * BS, :],
                    )
                    nc.sync.dma_start(
                        qg[0:64, gi * BS : (gi + 1) * BS],
                        qT_d[r0 : r0 + BS, gv * BS : (gv + 1) * BS],
                    )

                # -- local + global-col scores (S^T layout), exp, AV --
                def sc_chunk(c, tag="a"):
                    """scores+exp for padded key chunk c; returns (a_sb, off0)"""
                    lo = max(0, (c - 1)) * P
                    hi = min(nt, c + 1) * P
                    nw = hi - lo
                    sps = psS.tile([P, 256], f32, tag="sps")
                    nc.tensor.matmul(
                        sps[:, 0:nw],
                        kp[:, c * P : (c + 1) * P],
                        qz[:, lo:hi],
                        start=True,
                        stop=True,
                    )
                    a_sb = apool.tile([P, 256], bf16, tag=tag)
                    nc.scalar.activation(a_sb[:, 0:nw], sps[:, 0:nw], AF.Exp)
                    return a_sb

                def gc_group(j):
                    spg = psG.tile([P, 512], f32, tag="spg")
                    nc.tensor.matmul(
                        spg, kg, qz[:, j * 512 : (j + 1) * 512], start=True, stop=True
                    )
                    ag = agp.tile([P, 512], bf16, tag="ag")
                    nc.scalar.activation(ag, spg, AF.Exp)
                    return ag

                ag_cur = gc_group(0)
                a_buf = [sc_chunk(0), sc_chunk(1), sc_chunk(2)]
                for t in range(nt):
                    if t % 4 == 0 and t > 0:
                        ag_cur = gc_group(t // 4)
                    if t + 3 <= nt:
                        a_buf.append(sc_chunk(t + 3))
                    a_lo = a_buf[t]
                    off = 0 if t == 0 else P
                    a_up = a_buf[t + 1]
                    cps = psC.tile([P, 130], f32, tag="cps")
                    nc.tensor.matmul(
                        cps[:, 0:65],
                        a_lo[:, off : off + P],
                        va[:, t, :],
                        start=True,
                        stop=False,
                    )
                    nc.tensor.matmul(
                        cps[:, 0:65],
                        a_up[:, 0:P],
                        va[:, t + 1, :],
                        start=False,
                        stop=True,
                    )
                    nc.tensor.matmul(
                        cps[:, 65:130],
                        ag_cur[:, (t % 4) * P : (t % 4 + 1) * P],
                        vg,
                        start=True,
                        stop=True,
                    )
                    a_buf[t] = None  # release
                    # normalization: per-partition (per-query) reciprocals
                    r2 = stat.tile([P, 2], f32, tag="r2")
                    nc.vector.reciprocal(r2, cps[:, 64:130:65])
                    tg = tgp.tile([P, BS], f32, tag="tg")
                    nc.vector.tensor_scalar_mul(tg, cps[:, 65:129], r2[:, 1:2])
                    nc.vector.scalar_tensor_tensor(
                        ctx_nat[:, t, r0 : r0 + BS],
                        cps[:, 0:64],
                        r2[:, 0:1],
                        tg,
                        OP.mult,
                        OP.add,
                    )

                # -- global rows: full attention for the 2 global q blocks --
                cpr0 = psQ.tile([P, 65], f32, tag="cpr0")
                cpr1 = psQ.tile([P, 65], f32, tag="cpr1")
                for j in range(8):
                    spr = psR.tile([P, 4, P], f32, tag="spr")
                    for i in range(4):
                        c = 1 + 4 * j + i
                        nc.tensor.matmul(
                            spr[:, i, :],
                            kp[:, c * P : (c + 1) * P],
                            qg,
                            start=True,
                            stop=True,
                        )
                    ar = agr.tile([P, 4, P], bf16, tag="ar")
                    nc.scalar.activation(ar, spr, AF.Exp)
                    for i in range(4):
                        c = 1 + 4 * j + i
                        nc.tensor.matmul(
                            cpr0[p0s : p0s + BS, :],
                            ar[:, i, 0:BS],
                            va[:, c, :],
                            start=(c == 1),
                            stop=(c == nkc - 1),
                        )
                        nc.tensor.matmul(
                            cpr1[p1s : p1s + BS, :],
                            ar[:, i, BS:P],
                            va[:, c, :],
                            start=(c == 1),
                            stop=(c == nkc - 1),
                        )
                for gi, gv, pb, cpr in ((0, g0, p0s, cpr0), (1, g1, p1s, cpr1)):
                    rg = stat.tile([P, 1], f32, tag=f"rg{gi}")
                    nc.vector.reciprocal(
                        rg[pb : pb + BS, :], cpr[pb : pb + BS, 64:65]
                    )
                    nc.vector.tensor_scalar_mul(
                        ctx_nat[pb : pb + BS, gv // 2, r0 : r0 + BS],
                        cpr[pb : pb + BS, 0:64],
                        rg[pb : pb + BS, :],
                    )

        # ---------------- pass C: output projection ----------------
        with ExitStack() as ps:
            wop = ps.enter_context(tc.tile_pool(name="wop", bufs=1))
            wo_sb = wop.tile([P, ndc2, dmodel], bf16)
            nc.sync.dma_start(wo_sb, wo_d.rearrange("(a p) o -> p a o", p=P))
            ctp = ps.enter_context(tc.tile_pool(name="ctp", bufs=2))
            copool = ps.enter_context(tc.tile_pool(name="co", bufs=4))
            psT = ps.enter_context(tc.tile_pool(name="psT", bufs=4, space="PSUM"))
            psO = ps.enter_context(tc.tile_pool(name="psO", bufs=4, space="PSUM"))
            for ncc in range(nch):
                ctxT = ctp.tile([P, ndc2, 512], bf16, tag="ctxT")
                for ti in range(4):
                    t = ncc * 4 + ti
                    for dc in range(ndc2):
                        tp = psT.tile([P, P], bf16, tag="tp")
                        nc.tensor.transpose(
                            tp, ctx_nat[:, t, dc * P : (dc + 1) * P], ident
                        )
                        nc.scalar.copy(ctxT[:, dc, ti * P : (ti + 1) * P], tp)
                for ot in range(dmodel // P):
                    pp = psO.tile([P, 512], f32, tag="pso")
                    for dc in range(ndc2):
                        nc.tensor.matmul(
                            pp,
                            wo_sb[:, dc, ot * P : (ot + 1) * P],
                            ctxT[:, dc, :],
                            start=(dc == 0),
                            stop=(dc == ndc2 - 1),
                        )
                    ob = copool.tile([P, 512], f32, tag="ob")
                    nc.vector.tensor_copy(ob, pp)
                    nc.sync.dma_start(
                        out_d[ot * P : (ot + 1) * P, ncc * 512 : (ncc + 1) * 512], ob
                    )

        if dbg:
            nc.sync.dma_start(qTo_d[:, :], qT_d)
            nc.sync.dma_start(kTo_d[:, :], kT_d)
            nc.sync.dma_start(vo_d[:, :], v_d)
            nc.sync.dma_start(ctxo_d[:, :, :], ctx_nat)

    nc.finalize()
    return nc


@functools.lru_cache(maxsize=8)
def _get(n, dmodel, dl, g0, g1):
    return _build(n, dmodel, dl, g0, g1)


def _prepare(inputs):
    """Build (nc, in_maps, meta) for the SPMD run from full unsharded inputs."""
    x = np.asarray(inputs["x"], np.float32)
    Wq = np.asarray(inputs["Wq"], np.float32)
    Wk = np.asarray(inputs["Wk"], np.float32)
    Wv = np.asarray(inputs["Wv"], np.float32)
    Wo = np.asarray(inputs["Wo"], np.float32)
    bq = np.asarray(inputs["bq"], np.float32)
    bk = np.asarray(inputs["bk"], np.float32)
    bv = np.asarray(inputs["bv"], np.float32)
    bo = np.asarray(inputs["bo"], np.float32)
    gi = np.asarray(inputs["global_indices"]).astype(np.int64)
    g0, g1 = int(gi[0]), int(gi[1])

    b_, n_, d_ = x.shape
    dl = d_ // 2
    scale = 1.0 / np.sqrt(np.float32(64.0)).astype(np.float32)

    nc = _get(n_, d_, dl, g0, g1)

    import ml_dtypes

    bf = ml_dtypes.bfloat16
    # mask pattern rows (periodic in the column index, see pass B docstring)
    NEGf = np.float32(-1e9)
    j = np.arange(n_) % 256
    qmask = np.zeros((4, n_), np.float32)
    qmask[0, (j >= 128) & (j < 192)] = 1.0  # w1e
    qmask[1, (j >= 64) & (j < 128)] = 1.0   # w2e
    qmask[2, j < 64] = 1.0                  # w1o
    qmask[3, j >= 192] = 1.0                # w2o
    qmask = np.ascontiguousarray(qmask).astype(bf)
    jk = np.arange(n_ + 128) % 256
    kmask = np.zeros((4, n_ + 128), np.float32)
    kmask[0, (jk >= 64) & (jk < 128)] = NEGf   # u1e
    kmask[1, jk < 64] = NEGf                   # u2e
    kmask[2, jk >= 192] = NEGf                 # u1o
    kmask[3, (jk >= 128) & (jk < 192)] = NEGf  # u2o
    kmask = np.ascontiguousarray(kmask).astype(bf)
    in_maps = []
    for c in range(8):
        b, hg = divmod(c, 2)
        S = slice(hg * dl, (hg + 1) * dl)
        in_maps.append(
            {
                "xT": np.ascontiguousarray(x[b].T).astype(bf),
                "qmask": qmask,
                "kmask": kmask,
                "wqT": np.ascontiguousarray((Wq[S, :] * scale).T).astype(bf),
                "wkT": np.ascontiguousarray(Wk[S, :].T).astype(bf),
                "wvT": np.ascontiguousarray(Wv[S, :].T).astype(bf),
                "woT": np.ascontiguousarray(Wo[:, S].T).astype(bf),
                "bq": np.ascontiguousarray(bq[S] * scale),
                "bk": np.ascontiguousarray(bk[S]),
            }
        )

    # host-side bv correction: out[q] += c(q) * bv @ Wo.T, c(q)=1 on global
    # blocks (overwritten by full-attention rows), else 2.
    bvWo = bv @ Wo.T  # [d_model]
    coef = np.full((n_, 1), 2.0, np.float32)
    bs = 64
    coef[g0 * bs : (g0 + 1) * bs] = 1.0
    coef[g1 * bs : (g1 + 1) * bs] = 1.0
    corr = (coef * bvWo[None, :] + bo[None, :]).astype(np.float32)

    return nc, in_maps, (b_, n_, d_, corr)


def _combine(res, meta):
    b_, n_, d_, corr = meta
    out = np.empty((b_, n_, d_), np.float32)
    for b in range(b_):
        out[b] = res[2 * b]["outT"].T + res[2 * b + 1]["outT"].T + corr
    return out


def kernel(**inputs):
    _ensure_path()
    from concourse.bass_utils import run_bass_kernel_spmd

    nc, in_maps, meta = _prepare(inputs)
    res = run_bass_kernel_spmd(nc, in_maps, list(range(8))).results
    return _combine(res, meta)


# revision 16
# speedup vs baseline: 2.0472x; 1.0015x over previous
"""BigBird attention (B=4, N=4096, D=1024, H=16, BS=64) on 8 TRN2 NeuronCores.

Sharding: batch (4-way) x head-group (2-way).  Core c handles batch c//2 and
heads [hg*8, hg*8+8) where hg = c%2 (d_model slice [hg*512, hg*512+512)).

Per core:
  pass A: QKV projections.  x.T tiles produced with DMA transposes; q/k
          emitted transposed (qT/kT: [dl, n], head dim on partitions), v
          natural.  score scale folded into Wq/bq on the host; bv dropped
          entirely (attention is affine in v: host adds c(q)*bv@Wo.T).
  pass B: per-head BigBird attention, all scores computed transposed
          (S^T = K_chunk^T Q, keys on partitions) so probabilities feed the
          AV matmuls directly as stationary operands -- no PE transposes.
          The sliding-window mask is folded into 4 extra contraction rows
          (rank-2 outer product of periodic 0/1 q-patterns and -1e9
          k-patterns), so exp() yields exact zeros in the masked corners.
          No max subtraction (scores bounded ~|3|).  V carries a ones
          column so each AV matmul also emits the softmax denominator
          per-partition; normalization is a per-partition reciprocal.
  pass C: transpose ctx with the PE, then row-parallel output projection
          -> partial outT [d_model, n] (f32).
Host combines: out[b] = outT(core 2b).T + outT(core 2b+1).T + bo + c(q)*bv@Wo.T
with c(q) = 1 for rows in global blocks else 2.

The kernel is specialized (compiled) per global_indices value.
"""

import functools
import sys

import numpy as np

P = 128
BS = 64
NEG = -1e9


def _ensure_path():
    try:
        import concourse.bass  # noqa: F401
    except ImportError:
        sys.path.insert(0, "/opt/trn_rl_repo")


def _build(n, dmodel, dl, g0, g1, dbg=0):
    """Build the per-core Bass program.

    n: sequence length per core, dmodel: model dim, dl: local head dims =
    hpc*64.  g0, g1: global block indices (compile-time constants).
    """
    _ensure_path()
    from contextlib import ExitStack

    import concourse.bass as bass  # noqa: F401
    import concourse.tile as tile
    from concourse import bacc, mybir
    from concourse.masks import make_identity

    f32 = mybir.dt.float32
    bf16 = mybir.dt.bfloat16
    AF = mybir.ActivationFunctionType
    OP = mybir.AluOpType

    nch = n // 512     # 512-column chunks of the sequence
    ndc = dmodel // P  # contraction chunks for QKV proj
    njt = dl // P      # row tiles of qT/kT
    hpc = dl // BS     # heads per core
    nt = n // P        # query tiles (2 blocks each)
    nkc = nt + 1       # padded key chunks (128 keys each, shifted by -BS)
    ndc2 = dl // P     # contraction chunks for out proj

    nc = bacc.Bacc(None, target_bir_lowering=False, debug=False)

    xT_d = nc.dram_tensor("xT", [dmodel, n], bf16, kind="ExternalInput")
    wq_d = nc.dram_tensor("wqT", [dmodel, dl], bf16, kind="ExternalInput")
    wk_d = nc.dram_tensor("wkT", [dmodel, dl], bf16, kind="ExternalInput")
    wv_d = nc.dram_tensor("wvT", [dmodel, dl], bf16, kind="ExternalInput")
    wo_d = nc.dram_tensor("woT", [dl, dmodel], bf16, kind="ExternalInput")
    bq_d = nc.dram_tensor("bq", [dl], f32, kind="ExternalInput")
    bk_d = nc.dram_tensor("bk", [dl], f32, kind="ExternalInput")
    qm_d = nc.dram_tensor("qmask", [4, n], bf16, kind="ExternalInput")
    km_d = nc.dram_tensor("kmask", [4, n + 2 * BS], bf16, kind="ExternalInput")
    out_d = nc.dram_tensor("outT", [dmodel, n], bf16, kind="ExternalOutput")
    if dbg:
        qTo_d = nc.dram_tensor("qTo", [dl, n], bf16, kind="ExternalOutput")
        kTo_d = nc.dram_tensor("kTo", [dl, n], bf16, kind="ExternalOutput")
        vo_d = nc.dram_tensor("vo", [n, dl], bf16, kind="ExternalOutput")
        ctxo_d = nc.dram_tensor("ctxo", [P, n // P, dl], bf16, kind="ExternalOutput")

    with tile.TileContext(nc) as tc, ExitStack() as top:
        dram = top.enter_context(tc.tile_pool(name="dram", bufs=1, space="DRAM"))
        qT_d = dram.tile([dl, n], bf16)
        kT_d = dram.tile([dl, n], bf16)
        v_d = dram.tile([n, dl], bf16)

        const = top.enter_context(tc.tile_pool(name="const", bufs=1))
        ident = const.tile([P, P], bf16)
        make_identity(nc, ident)

        # ctx natural accumulator: [q mod 128, tile, head*64+dh], SBUF-resident
        ctx_pool = top.enter_context(tc.tile_pool(name="ctx", bufs=1))
        ctx_nat = ctx_pool.tile([P, nt, dl], bf16)

        # pass-B per-head slots (manual ping-pong).  Allocated at top level so
        # their memory is disjoint from the pass-A pools: the constant regions
        # (mask rows, ones columns) are written once, up front.
        slot = top.enter_context(tc.tile_pool(name="slot", bufs=1))
        qz_s = [slot.tile([68, n], bf16, tag=f"qz{i}", name=f"qz{i}") for i in range(2)]
        kp_s = [slot.tile([68, n + 2 * BS], bf16, tag=f"kp{i}", name=f"kp{i}") for i in range(2)]
        va_s = [slot.tile([P, nkc, BS + 1], bf16, tag=f"va{i}", name=f"va{i}") for i in range(2)]
        kg_s = [slot.tile([68, P], bf16, tag=f"kg{i}", name=f"kg{i}") for i in range(2)]
        vg_s = [slot.tile([P, BS + 1], bf16, tag=f"vg{i}", name=f"vg{i}") for i in range(2)]
        qg_s = [slot.tile([68, P], bf16, tag=f"qg{i}", name=f"qg{i}") for i in range(2)]
        for qz in qz_s:
            nc.sync.dma_start(qz[64:68, :], qm_d[:, :])
        for kp in kp_s:
            nc.sync.dma_start(kp[64:68, :], km_d[:, :])
        for kg in kg_s:
            nc.gpsimd.memset(kg[64:68, :], 0.0)
        for qg in qg_s:
            nc.gpsimd.memset(qg[64:68, :], 0.0)
        for va in va_s:
            nc.gpsimd.memset(va[:, :, BS : BS + 1], 1.0)
        for vg in vg_s:
            nc.gpsimd.memset(vg[:, BS : BS + 1], 1.0)

        # ---------------- pass A: projections ----------------
        with ExitStack() as ps:
            wpool = ps.enter_context(tc.tile_pool(name="wpool", bufs=1))
            wq_sb = wpool.tile([P, ndc, dl], bf16)
            wk_sb = wpool.tile([P, ndc, dl], bf16)
            wv_sb = wpool.tile([P, ndc, dl], bf16)
            nc.sync.dma_start(wq_sb, wq_d.rearrange("(a p) j -> p a j", p=P))
            nc.sync.dma_start(wk_sb, wk_d.rearrange("(a p) j -> p a j", p=P))
            nc.sync.dma_start(wv_sb, wv_d.rearrange("(a p) j -> p a j", p=P))
            bq_sb = wpool.tile([P, njt], f32)
            bk_sb = wpool.tile([P, njt], f32)
            nc.sync.dma_start(bq_sb, bq_d.rearrange("(a p) -> p a", p=P))
            nc.sync.dma_start(bk_sb, bk_d.rearrange("(a p) -> p a", p=P))

            psA = ps.enter_context(tc.tile_pool(name="psA", bufs=4, space="PSUM"))
            xtpool = ps.enter_context(tc.tile_pool(name="xtpool", bufs=3))
            aout = ps.enter_context(tc.tile_pool(name="aout", bufs=4))

            for ch in range(nch):
                n0 = ch * 512
                xT = xtpool.tile([P, ndc, 512], bf16, tag="xT")
                for dc in range(ndc):
                    nc.sync.dma_start(
                        xT[:, dc, :], xT_d[dc * P : (dc + 1) * P, n0 : n0 + 512]
                    )
                # qT / kT (transposed outputs, bias per-partition)
                for w_sb, b_sb, dst in ((wq_sb, bq_sb, qT_d), (wk_sb, bk_sb, kT_d)):
                    for jt in range(njt):
                        pp = psA.tile([P, 512], f32, tag="ps_a")
                        for dc in range(ndc):
                            nc.tensor.matmul(
                                pp,
                                w_sb[:, dc, jt * P : (jt + 1) * P],
                                xT[:, dc, :],
                                start=(dc == 0),
                                stop=(dc == ndc - 1),
                            )
                        ot = aout.tile([P, 512], bf16, tag="aout")
                        nc.scalar.activation(
                            ot, pp, AF.Identity, bias=b_sb[:, jt : jt + 1]
                        )
                        nc.scalar.dma_start(
                            dst[jt * P : (jt + 1) * P, n0 : n0 + 512], ot
                        )
                # v (natural layout, no bias -- folded to host)
                for ns in range(4):
                    pp = psA.tile([P, dl], f32, tag="ps_a")
                    for dc in range(ndc):
                        nc.tensor.matmul(
                            pp,
                            xT[:, dc, ns * P : (ns + 1) * P],
                            wv_sb[:, dc, :],
                            start=(dc == 0),
                            stop=(dc == ndc - 1),
                        )
                    ot = aout.tile([P, dl], bf16, tag="aout_v")
                    nc.scalar.copy(ot, pp)
                    nc.scalar.dma_start(v_d[n0 + ns * P : n0 + (ns + 1) * P, :], ot)

        # ---------------- pass B: attention ----------------
        with ExitStack() as ps:
            apool = ps.enter_context(tc.tile_pool(name="apool", bufs=4))
            agp = ps.enter_context(tc.tile_pool(name="agp", bufs=2))
            agr = ps.enter_context(tc.tile_pool(name="agr", bufs=2))
            stat = ps.enter_context(tc.tile_pool(name="stat", bufs=4))
            tgp = ps.enter_context(tc.tile_pool(name="tgp", bufs=4))
            psS = ps.enter_context(tc.tile_pool(name="psS", bufs=3, space="PSUM"))
            psG = ps.enter_context(tc.tile_pool(name="psG", bufs=1, space="PSUM"))
            psC = ps.enter_context(tc.tile_pool(name="psC", bufs=2, space="PSUM"))
            psQ = ps.enter_context(tc.tile_pool(name="psQ", bufs=1, space="PSUM"))

            p0s = (g0 % 2) * BS
            p1s = (g1 % 2) * BS

            for h in range(hpc):
                r0 = h * BS
                qz, kp, va = qz_s[h % 2], kp_s[h % 2], va_s[h % 2]
                kg, vg, qg = kg_s[h % 2], vg_s[h % 2], qg_s[h % 2]

                # -- per-head DMAs (overlap previous head's compute) --
                nc.sync.dma_start(qz[0:64, :], qT_d[r0 : r0 + BS, :])
                nc.sync.dma_start(kp[0:64, BS : BS + n], kT_d[r0 : r0 + BS, :])
                nc.sync.dma_start(kp[0:64, 0:BS], kT_d[r0 : r0 + BS, n - BS : n])
                nc.sync.dma_start(kp[0:64, BS + n :], kT_d[r0 : r0 + BS, 0:BS])
                vs = v_d[:, r0 : r0 + BS]
                nc.sync.dma_start(va[0:BS, 0, 0:BS], vs[n - BS : n, :])
                nc.sync.dma_start(va[BS:P, 0, 0:BS], vs[0:BS, :])
                nc.sync.dma_start(
                    va[:, 1 : nkc - 1, 0:BS],
                    vs[BS : n - BS, :].rearrange("(a p) c -> p a c", p=P),
                )
                nc.sync.dma_start(va[0:BS, nkc - 1, 0:BS], vs[n - BS : n, :])
                nc.sync.dma_start(va[BS:P, nkc - 1, 0:BS], vs[0:BS, :])
                for gi, gv in enumerate((g0, g1)):
                    nc.sync.dma_start(
                        kg[0:64, gi * BS : (gi + 1) * BS],
                        kT_d[r0 : r0 + BS, gv * BS : (gv + 1) * BS],
                    )
                    nc.sync.dma_start(
                        vg[gi * BS : (gi + 1) * BS, 0:BS],
                        vs[gv * BS : (gv + 1) * BS, :],
                    )
                    nc.sync.dma_start(
                        qg[0:64, gi * BS : (gi + 1) * BS],
                        qT_d[r0 : r0 + BS, gv * BS : (gv + 1) * BS],
                    )

                # -- local + global-col scores (S^T layout), exp, AV --
                def sc_pair(pr):
                    """scores+exp for padded key chunks 2pr, 2pr+1 (batched)."""
                    sps = psS.tile([P, 2, 256], f32, tag="sps")
                    a_sb = apool.tile([P, 2, 256], bf16, tag="a")
                    nws = []
                    for i in (0, 1):
                        c = 2 * pr + i
                        if c > nt:
                            continue
                        lo = max(0, (c - 1)) * P
                        hi = min(nt, c + 1) * P
                        nws.append(hi - lo)
                        nc.tensor.matmul(
                            sps[:, i, 0 : hi - lo],
                            kp[:, c * P : (c + 1) * P],
                            qz[:, lo:hi],
                            start=True,
                            stop=True,
                        )
                    if nws == [256, 256]:
                        nc.scalar.activation(a_sb, sps, AF.Exp)
                    else:
                        for i, nw in enumerate(nws):
                            nc.scalar.activation(
                                a_sb[:, i, 0:nw], sps[:, i, 0:nw], AF.Exp
                            )
                    return a_sb

                def gc_group(j):
                    spg = psG.tile([P, 512], f32, tag="spg")
                    nc.tensor.matmul(
                        spg, kg, qz[:, j * 512 : (j + 1) * 512], start=True, stop=True
                    )
                    ag = agp.tile([P, 512], bf16, tag="ag")
                    nc.scalar.activation(ag, spg, AF.Exp)
                    return ag

                ag_cur = gc_group(0)
                a_pair = {0: sc_pair(0), 1: sc_pair(1)}
                for t in range(nt):
                    if t % 4 == 0 and t > 0:
                        ag_cur = gc_group(t // 4)
                    want = min(nt // 2, t // 2 + 2)
                    if want not in a_pair:
                        a_pair[want] = sc_pair(want)
                        a_pair.pop(want - 3, None)
                    a_lo = a_pair[t // 2][:, t % 2, :]
                    off = 0 if t == 0 else P
                    a_up = a_pair[(t + 1) // 2][:, (t + 1) % 2, :]
                    cps = psC.tile([P, 130], f32, tag="cps")
                    nc.tensor.matmul(
                        cps[:, 0:65],
                        a_lo[:, off : off + P],
                        va[:, t, :],
                        start=True,
                        stop=False,
                    )
                    nc.tensor.matmul(
                        cps[:, 0:65],
                        a_up[:, 0:P],
                        va[:, t + 1, :],
                        start=False,
                        stop=True,
                    )
                    nc.tensor.matmul(
                        cps[:, 65:130],
                        ag_cur[:, (t % 4) * P : (t % 4 + 1) * P],
                        vg,
                        start=True,
                        stop=True,
                    )
                    # normalization: per-partition (per-query) reciprocals
                    r2 = stat.tile([P, 2], f32, tag="r2")
                    nc.vector.reciprocal(r2, cps[:, 64:130:65])
                    tg = tgp.tile([P, BS], f32, tag="tg")
                    nc.vector.tensor_scalar_mul(tg, cps[:, 65:129], r2[:, 1:2])
                    nc.vector.scalar_tensor_tensor(
                        ctx_nat[:, t, r0 : r0 + BS],
                        cps[:, 0:64],
                        r2[:, 0:1],
                        tg,
                        OP.mult,
                        OP.add,
                    )

                # -- global rows: full attention for the 2 global q blocks --
                cpr0 = psQ.tile([P, 65], f32, tag="cpr0")
                cpr1 = psQ.tile([P, 65], f32, tag="cpr1")
                for j in range(8):
                    spr = psS.tile([P, 4, P], f32, tag="sps")
                    for i in range(4):
                        c = 1 + 4 * j + i
                        nc.tensor.matmul(
                            spr[:, i, :],
                            kp[:, c * P : (c + 1) * P],
                            qg,
                            start=True,
                            stop=True,
                        )
                    ar = agr.tile([P, 4, P], bf16, tag="ar")
                    nc.scalar.activation(ar, spr, AF.Exp)
                    for i in range(4):
                        c = 1 + 4 * j + i
                        nc.tensor.matmul(
                            cpr0[p0s : p0s + BS, :],
                            ar[:, i, 0:BS],
                            va[:, c, :],
                            start=(c == 1),
                            stop=(c == nkc - 1),
                        )
                        nc.tensor.matmul(
                            cpr1[p1s : p1s + BS, :],
                            ar[:, i, BS:P],
                            va[:, c, :],
                            start=(c == 1),
                            stop=(c == nkc - 1),
                        )
                for gi, gv, pb, cpr in ((0, g0, p0s, cpr0), (1, g1, p1s, cpr1)):
                    rg = stat.tile([P, 1], f32, tag=f"rg{gi}")
                    nc.vector.reciprocal(
                        rg[pb : pb + BS, :], cpr[pb : pb + BS, 64:65]
                    )
                    nc.vector.tensor_scalar_mul(
                        ctx_nat[pb : pb + BS, gv // 2, r0 : r0 + BS],
                        cpr[pb : pb + BS, 0:64],
                        rg[pb : pb + BS, :],
                    )

        # ---------------- pass C: output projection ----------------
        with ExitStack() as ps:
            wop = ps.enter_context(tc.tile_pool(name="wop", bufs=1))
            wo_sb = wop.tile([P, ndc2, dmodel], bf16)
            nc.sync.dma_start(wo_sb, wo_d.rearrange("(a p) o -> p a o", p=P))
            ctp = ps.enter_context(tc.tile_pool(name="ctp", bufs=2))
            copool = ps.enter_context(tc.tile_pool(name="co", bufs=4))
            psT = ps.enter_context(tc.tile_pool(name="psT", bufs=4, space="PSUM"))
            psO = ps.enter_context(tc.tile_pool(name="psO", bufs=4, space="PSUM"))
            for ncc in range(nch):
                ctxT = ctp.tile([P, ndc2, 512], bf16, tag="ctxT")
                for ti in range(4):
                    t = ncc * 4 + ti
                    for dc in range(ndc2):
                        tp = psT.tile([P, P], bf16, tag="tp")
                        nc.tensor.transpose(
                            tp, ctx_nat[:, t, dc * P : (dc + 1) * P], ident
                        )
                        nc.scalar.copy(ctxT[:, dc, ti * P : (ti + 1) * P], tp)
                for ot in range(dmodel // P):
                    pp = psO.tile([P, 512], f32, tag="pso")
                    for dc in range(ndc2):
                        nc.tensor.matmul(
                            pp,
                            wo_sb[:, dc, ot * P : (ot + 1) * P],
                            ctxT[:, dc, :],
                            start=(dc == 0),
                            stop=(dc == ndc2 - 1),
                        )
                    ob = copool.tile([P, 512], bf16, tag="ob")
                    nc.vector.tensor_copy(ob, pp)
                    nc.sync.dma_start(
                        out_d[ot * P : (ot + 1) * P, ncc * 512 : (ncc + 1) * 512], ob
                    )

        if dbg:
            nc.sync.dma_start(qTo_d[:, :], qT_d)
            nc.sync.dma_start(kTo_d[:, :], kT_d)
            nc.sync.dma_start(vo_d[:, :], v_d)
            nc.sync.dma_start(ctxo_d[:, :, :], ctx_nat)

    nc.finalize()
    return nc


@functools.lru_cache(maxsize=8)
def _get(n, dmodel, dl, g0, g1):
    return _build(n, dmodel, dl, g0, g1)


def _prepare(inputs):
    """Build (nc, in_maps, meta) for the SPMD run from full unsharded inputs."""
    x = np.asarray(inputs["x"], np.float32)
    Wq = np.asarray(inputs["Wq"], np.float32)
    Wk = np.asarray(inputs["Wk"], np.float32)
    Wv = np.asarray(inputs["Wv"], np.float32)
    Wo = np.asarray(inputs["Wo"], np.float32)
    bq = np.asarray(inputs["bq"], np.float32)
    bk = np.asarray(inputs["bk"], np.float32)
    bv = np.asarray(inputs["bv"], np.float32)
    bo = np.asarray(inputs["bo"], np.float32)
    gi = np.asarray(inputs["global_indices"]).astype(np.int64)
    g0, g1 = int(gi[0]), int(gi[1])

    b_, n_, d_ = x.shape
    dl = d_ // 2
    scale = 1.0 / np.sqrt(np.float32(64.0)).astype(np.float32)

    nc = _get(n_, d_, dl, g0, g1)

    import ml_dtypes

    bf = ml_dtypes.bfloat16
    # mask pattern rows (periodic in the column index, see pass B docstring)
    NEGf = np.float32(-1e9)
    j = np.arange(n_) % 256
    qmask = np.zeros((4, n_), np.float32)
    qmask[0, (j >= 128) & (j < 192)] = 1.0  # w1e
    qmask[1, (j >= 64) & (j < 128)] = 1.0   # w2e
    qmask[2, j < 64] = 1.0                  # w1o
    qmask[3, j >= 192] = 1.0                # w2o
    qmask = np.ascontiguousarray(qmask).astype(bf)
    jk = np.arange(n_ + 128) % 256
    kmask = np.zeros((4, n_ + 128), np.float32)
    kmask[0, (jk >= 64) & (jk < 128)] = NEGf   # u1e
    kmask[1, jk < 64] = NEGf                   # u2e
    kmask[2, jk >= 192] = NEGf                 # u1o
    kmask[3, (jk >= 128) & (jk < 192)] = NEGf  # u2o
    kmask = np.ascontiguousarray(kmask).astype(bf)
    in_maps = []
    for c in range(8):
        b, hg = divmod(c, 2)
        S = slice(hg * dl, (hg + 1) * dl)
        in_maps.append(
            {
                "xT": np.ascontiguousarray(x[b].T).astype(bf),
                "qmask": qmask,
                "kmask": kmask,
                "wqT": np.ascontiguousarray((Wq[S, :] * scale).T).astype(bf),
                "wkT": np.ascontiguousarray(Wk[S, :].T).astype(bf),
                "wvT": np.ascontiguousarray(Wv[S, :].T).astype(bf),
                "woT": np.ascontiguousarray(Wo[:, S].T).astype(bf),
                "bq": np.ascontiguousarray(bq[S] * scale),
                "bk": np.ascontiguousarray(bk[S]),
            }
        )

    # host-side bv correction: out[q] += c(q) * bv @ Wo.T, c(q)=1 on global
    # blocks (overwritten by full-attention rows), else 2.
    bvWo = bv @ Wo.T  # [d_model]
    coef = np.full((n_, 1), 2.0, np.float32)
    bs = 64
    coef[g0 * bs : (g0 + 1) * bs] = 1.0
    coef[g1 * bs : (g1 + 1) * bs] = 1.0
    corr = (coef * bvWo[None, :] + bo[None, :]).astype(np.float32)

    return nc, in_maps, (b_, n_, d_, corr)


def _combine(res, meta):
    b_, n_, d_, corr = meta
    out = np.empty((b_, n_, d_), np.float32)
    for b in range(b_):
        out[b] = (
            res[2 * b]["outT"].T.astype(np.float32)
            + res[2 * b + 1]["outT"].T.astype(np.float32)
            + corr
        )
    return out


def kernel(**inputs):
    _ensure_path()
    from concourse.bass_utils import run_bass_kernel_spmd

    nc, in_maps, meta = _prepare(inputs)
    res = run_bass_kernel_spmd(nc, in_maps, list(range(8))).results
    return _combine(res, meta)


# revision 17
# speedup vs baseline: 2.1418x; 1.0462x over previous
"""BigBird attention (B=4, N=4096, D=1024, H=16, BS=64) on 8 TRN2 NeuronCores.

Sharding: batch (4-way) x head-group (2-way).  Core c handles batch c//2 and
heads [hg*8, hg*8+8) where hg = c%2 (d_model slice [hg*512, hg*512+512)).

Per core:
  pass A: QKV projections.  x.T tiles produced with DMA transposes; q/k
          emitted transposed (qT/kT: [dl, n], head dim on partitions), v
          natural.  score scale folded into Wq/bq on the host; bv dropped
          entirely (attention is affine in v: host adds c(q)*bv@Wo.T).
  pass B: per-head BigBird attention, all scores computed transposed
          (S^T = K_chunk^T Q, keys on partitions) so probabilities feed the
          AV matmuls directly as stationary operands -- no PE transposes.
          The sliding-window mask is folded into 4 extra contraction rows
          (rank-2 outer product of periodic 0/1 q-patterns and -1e9
          k-patterns), so exp() yields exact zeros in the masked corners.
          No max subtraction (scores bounded ~|3|).  V carries a ones
          column so each AV matmul also emits the softmax denominator
          per-partition; normalization is a per-partition reciprocal.
  pass C: transpose ctx with the PE, then row-parallel output projection
          -> partial outT [d_model, n] (f32).
Host combines: out[b] = outT(core 2b).T + outT(core 2b+1).T + bo + c(q)*bv@Wo.T
with c(q) = 1 for rows in global blocks else 2.

The kernel is specialized (compiled) per global_indices value.
"""

import functools
import sys

import numpy as np

P = 128
BS = 64
NEG = -1e9


def _ensure_path():
    try:
        import concourse.bass  # noqa: F401
    except ImportError:
        sys.path.insert(0, "/opt/trn_rl_repo")


def _build(n, dmodel, dl, g0, g1, dbg=0):
    """Build the per-core Bass program.

    n: sequence length per core, dmodel: model dim, dl: local head dims =
    hpc*64.  g0, g1: global block indices (compile-time constants).
    """
    _ensure_path()
    from contextlib import ExitStack

    import concourse.bass as bass  # noqa: F401
    import concourse.tile as tile
    from concourse import bacc, mybir
    from concourse.masks import make_identity

    f32 = mybir.dt.float32
    bf16 = mybir.dt.bfloat16
    AF = mybir.ActivationFunctionType
    OP = mybir.AluOpType

    nch = n // 512     # 512-column chunks of the sequence
    ndc = dmodel // P  # contraction chunks for QKV proj
    njt = dl // P      # row tiles of qT/kT
    hpc = dl // BS     # heads per core
    nt = n // P        # query tiles (2 blocks each)
    nkc = nt + 1       # padded key chunks (128 keys each, shifted by -BS)
    ndc2 = dl // P     # contraction chunks for out proj

    nc = bacc.Bacc(None, target_bir_lowering=False, debug=False)

    xT_d = nc.dram_tensor("xT", [dmodel, n], bf16, kind="ExternalInput")
    wq_d = nc.dram_tensor("wqT", [dmodel, dl], bf16, kind="ExternalInput")
    wk_d = nc.dram_tensor("wkT", [dmodel, dl], bf16, kind="ExternalInput")
    wv_d = nc.dram_tensor("wvT", [dmodel, dl], bf16, kind="ExternalInput")
    wo_d = nc.dram_tensor("woT", [dl, dmodel], bf16, kind="ExternalInput")
    bq_d = nc.dram_tensor("bq", [dl], f32, kind="ExternalInput")
    bk_d = nc.dram_tensor("bk", [dl], f32, kind="ExternalInput")
    qm_d = nc.dram_tensor("qmask", [4, n], bf16, kind="ExternalInput")
    km_d = nc.dram_tensor("kmask", [4, n + 2 * BS], bf16, kind="ExternalInput")
    out_d = nc.dram_tensor("outT", [dmodel, n], bf16, kind="ExternalOutput")
    if dbg:
        qTo_d = nc.dram_tensor("qTo", [dl, n], bf16, kind="ExternalOutput")
        kTo_d = nc.dram_tensor("kTo", [dl, n], bf16, kind="ExternalOutput")
        vo_d = nc.dram_tensor("vo", [n, dl], bf16, kind="ExternalOutput")
        ctxo_d = nc.dram_tensor("ctxo", [P, n // P, dl], bf16, kind="ExternalOutput")

    with tile.TileContext(nc) as tc, ExitStack() as top:
        dram = top.enter_context(tc.tile_pool(name="dram", bufs=1, space="DRAM"))
        qT_d = dram.tile([dl, n], bf16)
        kT_d = dram.tile([dl, n], bf16)
        v_d = dram.tile([n, dl], bf16)

        const = top.enter_context(tc.tile_pool(name="const", bufs=1))
        ident = const.tile([P, P], bf16)
        make_identity(nc, ident)

        # ctx natural accumulator: [q mod 128, tile, head*64+dh], SBUF-resident
        ctx_pool = top.enter_context(tc.tile_pool(name="ctx", bufs=1))
        ctx_nat = ctx_pool.tile([P, nt, dl], bf16)

        # pass-B per-head slots (manual ping-pong).  Allocated at top level so
        # their memory is disjoint from the pass-A pools: the constant regions
        # (mask rows, ones columns) are written once, up front.
        slot = top.enter_context(tc.tile_pool(name="slot", bufs=1))
        qz_s = [slot.tile([68, n], bf16, tag=f"qz{i}", name=f"qz{i}") for i in range(2)]
        kp_s = [slot.tile([68, n + 2 * BS], bf16, tag=f"kp{i}", name=f"kp{i}") for i in range(2)]
        va_s = [slot.tile([P, nkc, BS + 1], bf16, tag=f"va{i}", name=f"va{i}") for i in range(2)]
        kg_s = [slot.tile([68, P], bf16, tag=f"kg{i}", name=f"kg{i}") for i in range(2)]
        vg_s = [slot.tile([P, BS + 1], bf16, tag=f"vg{i}", name=f"vg{i}") for i in range(2)]
        qg_s = [slot.tile([68, P], bf16, tag=f"qg{i}", name=f"qg{i}") for i in range(2)]
        for qz in qz_s:
            nc.sync.dma_start(qz[64:68, :], qm_d[:, :])
        for kp in kp_s:
            nc.sync.dma_start(kp[64:68, :], km_d[:, :])
        for kg in kg_s:
            nc.gpsimd.memset(kg[64:68, :], 0.0)
        for qg in qg_s:
            nc.gpsimd.memset(qg[64:68, :], 0.0)
        for va in va_s:
            nc.gpsimd.memset(va[:, :, BS : BS + 1], 1.0)
        for vg in vg_s:
            nc.gpsimd.memset(vg[:, BS : BS + 1], 1.0)

        # ---------------- pass A: projections ----------------
        with ExitStack() as ps:
            wpool = ps.enter_context(tc.tile_pool(name="wpool", bufs=1))
            wq_sb = wpool.tile([P, ndc, dl], bf16)
            wk_sb = wpool.tile([P, ndc, dl], bf16)
            wv_sb = wpool.tile([P, ndc, dl], bf16)
            for a in range(ndc):
                nc.sync.dma_start(wq_sb[:, a, :], wq_d[a * P : (a + 1) * P, :])
                nc.sync.dma_start(wk_sb[:, a, :], wk_d[a * P : (a + 1) * P, :])
                nc.sync.dma_start(wv_sb[:, a, :], wv_d[a * P : (a + 1) * P, :])
            bq_sb = wpool.tile([P, njt], f32)
            bk_sb = wpool.tile([P, njt], f32)
            nc.sync.dma_start(bq_sb, bq_d.rearrange("(a p) -> p a", p=P))
            nc.sync.dma_start(bk_sb, bk_d.rearrange("(a p) -> p a", p=P))

            psA = ps.enter_context(tc.tile_pool(name="psA", bufs=4, space="PSUM"))
            xtpool = ps.enter_context(tc.tile_pool(name="xtpool", bufs=3))
            aout = ps.enter_context(tc.tile_pool(name="aout", bufs=4))

            for ch in range(nch):
                n0 = ch * 512
                xT = xtpool.tile([P, ndc, 512], bf16, tag="xT")
                for dc in range(ndc):
                    nc.sync.dma_start(
                        xT[:, dc, :], xT_d[dc * P : (dc + 1) * P, n0 : n0 + 512]
                    )
                # qT / kT (transposed outputs, bias per-partition)
                for w_sb, b_sb, dst in ((wq_sb, bq_sb, qT_d), (wk_sb, bk_sb, kT_d)):
                    for jt in range(njt):
                        pp = psA.tile([P, 512], f32, tag="ps_a")
                        for dc in range(ndc):
                            nc.tensor.matmul(
                                pp,
                                w_sb[:, dc, jt * P : (jt + 1) * P],
                                xT[:, dc, :],
                                start=(dc == 0),
                                stop=(dc == ndc - 1),
                            )
                        ot = aout.tile([P, 512], bf16, tag="aout")
                        nc.scalar.activation(
                            ot, pp, AF.Identity, bias=b_sb[:, jt : jt + 1]
                        )
                        nc.scalar.dma_start(
                            dst[jt * P : (jt + 1) * P, n0 : n0 + 512], ot
                        )
                # v (natural layout, no bias -- folded to host)
                for ns in range(4):
                    pp = psA.tile([P, dl], f32, tag="ps_a")
                    for dc in range(ndc):
                        nc.tensor.matmul(
                            pp,
                            xT[:, dc, ns * P : (ns + 1) * P],
                            wv_sb[:, dc, :],
                            start=(dc == 0),
                            stop=(dc == ndc - 1),
                        )
                    ot = aout.tile([P, dl], bf16, tag="aout_v")
                    nc.scalar.copy(ot, pp)
                    nc.scalar.dma_start(v_d[n0 + ns * P : n0 + (ns + 1) * P, :], ot)

        # ---------------- pass B: attention ----------------
        with ExitStack() as ps:
            apool = ps.enter_context(tc.tile_pool(name="apool", bufs=4))
            agp = ps.enter_context(tc.tile_pool(name="agp", bufs=2))
            agr = ps.enter_context(tc.tile_pool(name="agr", bufs=2))
            stat = ps.enter_context(tc.tile_pool(name="stat", bufs=4))
            tgp = ps.enter_context(tc.tile_pool(name="tgp", bufs=4))
            psS = ps.enter_context(tc.tile_pool(name="psS", bufs=3, space="PSUM"))
            psC = ps.enter_context(tc.tile_pool(name="psC", bufs=3, space="PSUM"))
            psQ = ps.enter_context(tc.tile_pool(name="psQ", bufs=1, space="PSUM"))

            p0s = (g0 % 2) * BS
            p1s = (g1 % 2) * BS

            for h in range(hpc):
                r0 = h * BS
                qz, kp, va = qz_s[h % 2], kp_s[h % 2], va_s[h % 2]
                kg, vg, qg = kg_s[h % 2], vg_s[h % 2], qg_s[h % 2]

                # -- per-head DMAs (overlap previous head's compute) --
                nc.sync.dma_start(qz[0:64, :], qT_d[r0 : r0 + BS, :])
                nc.sync.dma_start(kp[0:64, BS : BS + n], kT_d[r0 : r0 + BS, :])
                nc.sync.dma_start(kp[0:64, 0:BS], kT_d[r0 : r0 + BS, n - BS : n])
                nc.sync.dma_start(kp[0:64, BS + n :], kT_d[r0 : r0 + BS, 0:BS])
                vs = v_d[:, r0 : r0 + BS]
                nc.sync.dma_start(va[0:BS, 0, 0:BS], vs[n - BS : n, :])
                nc.sync.dma_start(va[BS:P, 0, 0:BS], vs[0:BS, :])
                nc.sync.dma_start(
                    va[:, 1 : nkc - 1, 0:BS],
                    vs[BS : n - BS, :].rearrange("(a p) c -> p a c", p=P),
                )
                nc.sync.dma_start(va[0:BS, nkc - 1, 0:BS], vs[n - BS : n, :])
                nc.sync.dma_start(va[BS:P, nkc - 1, 0:BS], vs[0:BS, :])
                for gi, gv in enumerate((g0, g1)):
                    nc.sync.dma_start(
                        kg[0:64, gi * BS : (gi + 1) * BS],
                        kT_d[r0 : r0 + BS, gv * BS : (gv + 1) * BS],
                    )
                    nc.sync.dma_start(
                        vg[gi * BS : (gi + 1) * BS, 0:BS],
                        vs[gv * BS : (gv + 1) * BS, :],
                    )
                    nc.sync.dma_start(
                        qg[0:64, gi * BS : (gi + 1) * BS],
                        qT_d[r0 : r0 + BS, gv * BS : (gv + 1) * BS],
                    )

                # -- local + global-col scores (S^T layout), exp, AV --
                def sc_pair(pr):
                    """scores+exp for padded key chunks 2pr, 2pr+1 (batched)."""
                    sps = psS.tile([P, 2, 256], f32, tag="sps")
                    a_sb = apool.tile([P, 2, 256], bf16, tag="a")
                    nws = []
                    for i in (0, 1):
                        c = 2 * pr + i
                        if c > nt:
                            continue
                        lo = max(0, (c - 1)) * P
                        hi = min(nt, c + 1) * P
                        nws.append(hi - lo)
                        nc.tensor.matmul(
                            sps[:, i, 0 : hi - lo],
                            kp[:, c * P : (c + 1) * P],
                            qz[:, lo:hi],
                            start=True,
                            stop=True,
                        )
                    if nws == [256, 256]:
                        nc.scalar.activation(a_sb, sps, AF.Exp)
                    else:
                        for i, nw in enumerate(nws):
                            nc.scalar.activation(
                                a_sb[:, i, 0:nw], sps[:, i, 0:nw], AF.Exp
                            )
                    return a_sb

                def gc_group(j):
                    spg = psS.tile([P, 512], f32, tag="sps")
                    nc.tensor.matmul(
                        spg, kg, qz[:, j * 512 : (j + 1) * 512], start=True, stop=True
                    )
                    ag = agp.tile([P, 512], bf16, tag="ag")
                    nc.scalar.activation(ag, spg, AF.Exp)
                    return ag

                ag_cur = gc_group(0)
                a_pair = {0: sc_pair(0), 1: sc_pair(1)}
                for t in range(nt):
                    if t % 4 == 0 and t > 0:
                        ag_cur = gc_group(t // 4)
                    want = min(nt // 2, t // 2 + 2)
                    if want not in a_pair:
                        a_pair[want] = sc_pair(want)
                        a_pair.pop(want - 3, None)
                    a_lo = a_pair[t // 2][:, t % 2, :]
                    off = 0 if t == 0 else P
                    a_up = a_pair[(t + 1) // 2][:, (t + 1) % 2, :]
                    cps = psC.tile([P, 130], f32, tag="cps")
                    nc.tensor.matmul(
                        cps[:, 0:65],
                        a_lo[:, off : off + P],
                        va[:, t, :],
                        start=True,
                        stop=False,
                    )
                    nc.tensor.matmul(
                        cps[:, 0:65],
                        a_up[:, 0:P],
                        va[:, t + 1, :],
                        start=False,
                        stop=True,
                    )
                    nc.tensor.matmul(
                        cps[:, 65:130],
                        ag_cur[:, (t % 4) * P : (t % 4 + 1) * P],
                        vg,
                        start=True,
                        stop=True,
                    )
                    # normalization: per-partition (per-query) reciprocals
                    r2 = stat.tile([P, 2], f32, tag="r2")
                    nc.vector.reciprocal(r2, cps[:, 64:130:65])
                    tg = tgp.tile([P, BS], f32, tag="tg")
                    nc.vector.tensor_scalar_mul(tg, cps[:, 65:129], r2[:, 1:2])
                    nc.vector.scalar_tensor_tensor(
                        ctx_nat[:, t, r0 : r0 + BS],
                        cps[:, 0:64],
                        r2[:, 0:1],
                        tg,
                        OP.mult,
                        OP.add,
                    )

                # -- global rows: full attention for the 2 global q blocks --
                cpr0 = psQ.tile([P, 65], f32, tag="cpr0")
                cpr1 = psQ.tile([P, 65], f32, tag="cpr1")
                for j in range(8):
                    spr = psS.tile([P, 4, P], f32, tag="sps")
                    for i in range(4):
                        c = 1 + 4 * j + i
                        nc.tensor.matmul(
                            spr[:, i, :],
                            kp[:, c * P : (c + 1) * P],
                            qg,
                            start=True,
                            stop=True,
                        )
                    ar = agr.tile([P, 4, P], bf16, tag="ar")
                    nc.scalar.activation(ar, spr, AF.Exp)
                    for i in range(4):
                        c = 1 + 4 * j + i
                        nc.tensor.matmul(
                            cpr0[p0s : p0s + BS, :],
                            ar[:, i, 0:BS],
                            va[:, c, :],
                            start=(c == 1),
                            stop=(c == nkc - 1),
                        )
                        nc.tensor.matmul(
                            cpr1[p1s : p1s + BS, :],
                            ar[:, i, BS:P],
                            va[:, c, :],
                            start=(c == 1),
                            stop=(c == nkc - 1),
                        )
                for gi, gv, pb, cpr in ((0, g0, p0s, cpr0), (1, g1, p1s, cpr1)):
                    rg = stat.tile([P, 1], f32, tag=f"rg{gi}")
                    nc.vector.reciprocal(
                        rg[pb : pb + BS, :], cpr[pb : pb + BS, 64:65]
                    )
                    nc.vector.tensor_scalar_mul(
                        ctx_nat[pb : pb + BS, gv // 2, r0 : r0 + BS],
                        cpr[pb : pb + BS, 0:64],
                        rg[pb : pb + BS, :],
                    )

        # ---------------- pass C: output projection ----------------
        with ExitStack() as ps:
            wop = ps.enter_context(tc.tile_pool(name="wop", bufs=1))
            wo_sb = wop.tile([P, ndc2, dmodel], bf16)
            nc.sync.dma_start(wo_sb, wo_d.rearrange("(a p) o -> p a o", p=P))
            ctp = ps.enter_context(tc.tile_pool(name="ctp", bufs=2))
            copool = ps.enter_context(tc.tile_pool(name="co", bufs=4))
            psT = ps.enter_context(tc.tile_pool(name="psT", bufs=4, space="PSUM"))
            psO = ps.enter_context(tc.tile_pool(name="psO", bufs=4, space="PSUM"))
            for ncc in range(nch):
                ctxT = ctp.tile([P, ndc2, 512], bf16, tag="ctxT")
                for ti in range(4):
                    t = ncc * 4 + ti
                    for dc in range(ndc2):
                        tp = psT.tile([P, P], bf16, tag="tp")
                        nc.tensor.transpose(
                            tp, ctx_nat[:, t, dc * P : (dc + 1) * P], ident
                        )
                        nc.scalar.copy(ctxT[:, dc, ti * P : (ti + 1) * P], tp)
                for ot in range(dmodel // P):
                    pp = psO.tile([P, 512], f32, tag="pso")
                    for dc in range(ndc2):
                        nc.tensor.matmul(
                            pp,
                            wo_sb[:, dc, ot * P : (ot + 1) * P],
                            ctxT[:, dc, :],
                            start=(dc == 0),
                            stop=(dc == ndc2 - 1),
                        )
                    ob = copool.tile([P, 512], bf16, tag="ob")
                    nc.vector.tensor_copy(ob, pp)
                    nc.sync.dma_start(
                        out_d[ot * P : (ot + 1) * P, ncc * 512 : (ncc + 1) * 512], ob
                    )

        if dbg:
            nc.sync.dma_start(qTo_d[:, :], qT_d)
            nc.sync.dma_start(kTo_d[:, :], kT_d)
            nc.sync.dma_start(vo_d[:, :], v_d)
            nc.sync.dma_start(ctxo_d[:, :, :], ctx_nat)

    nc.finalize()
    return nc


@functools.lru_cache(maxsize=8)
def _get(n, dmodel, dl, g0, g1):
    return _build(n, dmodel, dl, g0, g1)


def _prepare(inputs):
    """Build (nc, in_maps, meta) for the SPMD run from full unsharded inputs."""
    x = np.asarray(inputs["x"], np.float32)
    Wq = np.asarray(inputs["Wq"], np.float32)
    Wk = np.asarray(inputs["Wk"], np.float32)
    Wv = np.asarray(inputs["Wv"], np.float32)
    Wo = np.asarray(inputs["Wo"], np.float32)
    bq = np.asarray(inputs["bq"], np.float32)
    bk = np.asarray(inputs["bk"], np.float32)
    bv = np.asarray(inputs["bv"], np.float32)
    bo = np.asarray(inputs["bo"], np.float32)
    gi = np.asarray(inputs["global_indices"]).astype(np.int64)
    g0, g1 = int(gi[0]), int(gi[1])

    b_, n_, d_ = x.shape
    dl = d_ // 2
    scale = 1.0 / np.sqrt(np.float32(64.0)).astype(np.float32)

    nc = _get(n_, d_, dl, g0, g1)

    import ml_dtypes

    bf = ml_dtypes.bfloat16
    # mask pattern rows (periodic in the column index, see pass B docstring)
    NEGf = np.float32(-1e9)
    j = np.arange(n_) % 256
    qmask = np.zeros((4, n_), np.float32)
    qmask[0, (j >= 128) & (j < 192)] = 1.0  # w1e
    qmask[1, (j >= 64) & (j < 128)] = 1.0   # w2e
    qmask[2, j < 64] = 1.0                  # w1o
    qmask[3, j >= 192] = 1.0                # w2o
    qmask = np.ascontiguousarray(qmask).astype(bf)
    jk = np.arange(n_ + 128) % 256
    kmask = np.zeros((4, n_ + 128), np.float32)
    kmask[0, (jk >= 64) & (jk < 128)] = NEGf   # u1e
    kmask[1, jk < 64] = NEGf                   # u2e
    kmask[2, jk >= 192] = NEGf                 # u1o
    kmask[3, (jk >= 128) & (jk < 192)] = NEGf  # u2o
    kmask = np.ascontiguousarray(kmask).astype(bf)
    in_maps = []
    for c in range(8):
        b, hg = divmod(c, 2)
        S = slice(hg * dl, (hg + 1) * dl)
        in_maps.append(
            {
                "xT": np.ascontiguousarray(x[b].T).astype(bf),
                "qmask": qmask,
                "kmask": kmask,
                "wqT": np.ascontiguousarray((Wq[S, :] * scale).T).astype(bf),
                "wkT": np.ascontiguousarray(Wk[S, :].T).astype(bf),
                "wvT": np.ascontiguousarray(Wv[S, :].T).astype(bf),
                "woT": np.ascontiguousarray(Wo[:, S].T).astype(bf),
                "bq": np.ascontiguousarray(bq[S] * scale),
                "bk": np.ascontiguousarray(bk[S]),
            }
        )

    # host-side bv correction: out[q] += c(q) * bv @ Wo.T, c(q)=1 on global
    # blocks (overwritten by full-attention rows), else 2.
    bvWo = bv @ Wo.T  # [d_model]
    coef = np.full((n_, 1), 2.0, np.float32)
    bs = 64
    coef[g0 * bs : (g0 + 1) * bs] = 1.0
    coef[g1 * bs : (g1 + 1) * bs] = 1.0
    corr = (coef * bvWo[None, :] + bo[None, :]).astype(np.float32)

    return nc, in_maps, (b_, n_, d_, corr)


def _combine(res, meta):
    b_, n_, d_, corr = meta
    out = np.empty((b_, n_, d_), np.float32)
    for b in range(b_):
        out[b] = (
            res[2 * b]["outT"].T.astype(np.float32)
            + res[2 * b + 1]["outT"].T.astype(np.float32)
            + corr
        )
    return out


def kernel(**inputs):
    _ensure_path()
    from concourse.bass_utils import run_bass_kernel_spmd

    nc, in_maps, meta = _prepare(inputs)
    res = run_bass_kernel_spmd(nc, in_maps, list(range(8))).results
    return _combine(res, meta)


# revision 18
# speedup vs baseline: 2.1750x; 1.0155x over previous
"""BigBird attention (B=4, N=4096, D=1024, H=16, BS=64) on 8 TRN2 NeuronCores.

Sharding: batch (4-way) x head-group (2-way).  Core c handles batch c//2 and
heads [hg*8, hg*8+8) where hg = c%2 (d_model slice [hg*512, hg*512+512)).

Per core:
  pass A: QKV projections.  x.T tiles produced with DMA transposes; q/k
          emitted transposed (qT/kT: [dl, n], head dim on partitions), v
          natural.  score scale folded into Wq/bq on the host; bv dropped
          entirely (attention is affine in v: host adds c(q)*bv@Wo.T).
  pass B: per-head BigBird attention, all scores computed transposed
          (S^T = K_chunk^T Q, keys on partitions) so probabilities feed the
          AV matmuls directly as stationary operands -- no PE transposes.
          The sliding-window mask is folded into 4 extra contraction rows
          (rank-2 outer product of periodic 0/1 q-patterns and -1e9
          k-patterns), so exp() yields exact zeros in the masked corners.
          No max subtraction (scores bounded ~|3|).  V carries a ones
          column so each AV matmul also emits the softmax denominator
          per-partition; normalization is a per-partition reciprocal.
  pass C: transpose ctx with the PE, then row-parallel output projection
          -> partial outT [d_model, n] (f32).
Host combines: out[b] = outT(core 2b).T + outT(core 2b+1).T + bo + c(q)*bv@Wo.T
with c(q) = 1 for rows in global blocks else 2.

The kernel is specialized (compiled) per global_indices value.
"""

import functools
import sys

import numpy as np

P = 128
BS = 64
NEG = -1e9


def _ensure_path():
    try:
        import concourse.bass  # noqa: F401
    except ImportError:
        sys.path.insert(0, "/opt/trn_rl_repo")


def _build(n, dmodel, dl, g0, g1, dbg=0):
    """Build the per-core Bass program.

    n: sequence length per core, dmodel: model dim, dl: local head dims =
    hpc*64.  g0, g1: global block indices (compile-time constants).
    """
    _ensure_path()
    from contextlib import ExitStack

    import concourse.bass as bass  # noqa: F401
    import concourse.tile as tile
    from concourse import bacc, mybir
    from concourse.masks import make_identity

    f32 = mybir.dt.float32
    bf16 = mybir.dt.bfloat16
    AF = mybir.ActivationFunctionType
    OP = mybir.AluOpType

    nch = n // 512     # 512-column chunks of the sequence
    ndc = dmodel // P  # contraction chunks for QKV proj
    njt = dl // P      # row tiles of qT/kT
    hpc = dl // BS     # heads per core
    nt = n // P        # query tiles (2 blocks each)
    nkc = nt + 1       # padded key chunks (128 keys each, shifted by -BS)
    ndc2 = dl // P     # contraction chunks for out proj

    nc = bacc.Bacc(None, target_bir_lowering=False, debug=False)

    xT_d = nc.dram_tensor("xT", [dmodel, n], bf16, kind="ExternalInput")
    wq_d = nc.dram_tensor("wqT", [dmodel, dl], bf16, kind="ExternalInput")
    wk_d = nc.dram_tensor("wkT", [dmodel, dl], bf16, kind="ExternalInput")
    wv_d = nc.dram_tensor("wvT", [dmodel, dl], bf16, kind="ExternalInput")
    wo_d = nc.dram_tensor("woT", [dl, dmodel], bf16, kind="ExternalInput")
    bq_d = nc.dram_tensor("bq", [dl], f32, kind="ExternalInput")
    bk_d = nc.dram_tensor("bk", [dl], f32, kind="ExternalInput")
    qm_d = nc.dram_tensor("qmask", [4, n], bf16, kind="ExternalInput")
    km_d = nc.dram_tensor("kmask", [4, n + 2 * BS], bf16, kind="ExternalInput")
    out_d = nc.dram_tensor("outT", [dmodel, n], bf16, kind="ExternalOutput")
    if dbg:
        qTo_d = nc.dram_tensor("qTo", [dl, n], bf16, kind="ExternalOutput")
        kTo_d = nc.dram_tensor("kTo", [dl, n], bf16, kind="ExternalOutput")
        vo_d = nc.dram_tensor("vo", [n, dl], bf16, kind="ExternalOutput")
        ctxo_d = nc.dram_tensor("ctxo", [P, n // P, dl], bf16, kind="ExternalOutput")

    with tile.TileContext(nc) as tc, ExitStack() as top:
        dram = top.enter_context(tc.tile_pool(name="dram", bufs=1, space="DRAM"))
        qT_d = dram.tile([dl, n], bf16)
        kT_d = dram.tile([dl, n], bf16)
        v_d = dram.tile([n, dl], bf16)

        const = top.enter_context(tc.tile_pool(name="const", bufs=1))
        ident = const.tile([P, P], bf16)
        make_identity(nc, ident)

        # ctx natural accumulator: [q mod 128, tile, head*64+dh], SBUF-resident
        ctx_pool = top.enter_context(tc.tile_pool(name="ctx", bufs=1))
        ctx_nat = ctx_pool.tile([P, nt, dl], bf16)

        # pass-B per-head slots (manual ping-pong).  Allocated at top level so
        # their memory is disjoint from the pass-A pools: the constant regions
        # (mask rows, ones columns) are written once, up front.
        slot = top.enter_context(tc.tile_pool(name="slot", bufs=1))
        qz_s = [slot.tile([68, n], bf16, tag=f"qz{i}", name=f"qz{i}") for i in range(2)]
        kp_s = [slot.tile([68, n + 2 * BS], bf16, tag=f"kp{i}", name=f"kp{i}") for i in range(2)]
        va_s = [slot.tile([P, nkc, BS + 1], bf16, tag=f"va{i}", name=f"va{i}") for i in range(2)]
        kg_s = [slot.tile([68, P], bf16, tag=f"kg{i}", name=f"kg{i}") for i in range(2)]
        vg_s = [slot.tile([P, BS + 1], bf16, tag=f"vg{i}", name=f"vg{i}") for i in range(2)]
        qg_s = [slot.tile([68, P], bf16, tag=f"qg{i}", name=f"qg{i}") for i in range(2)]
        for qz in qz_s:
            nc.sync.dma_start(qz[64:68, :], qm_d[:, :])
        for kp in kp_s:
            nc.sync.dma_start(kp[64:68, :], km_d[:, :])
        for kg in kg_s:
            nc.gpsimd.memset(kg[64:68, :], 0.0)
        for qg in qg_s:
            nc.gpsimd.memset(qg[64:68, :], 0.0)
        for va in va_s:
            nc.gpsimd.memset(va[:, :, BS : BS + 1], 1.0)
        for vg in vg_s:
            nc.gpsimd.memset(vg[:, BS : BS + 1], 1.0)

        # ---------------- pass A: projections ----------------
        with ExitStack() as ps:
            wpool = ps.enter_context(tc.tile_pool(name="wpool", bufs=1))
            wq_sb = wpool.tile([P, ndc, dl], bf16)
            wk_sb = wpool.tile([P, ndc, dl], bf16)
            wv_sb = wpool.tile([P, ndc, dl], bf16)
            psA = ps.enter_context(tc.tile_pool(name="psA", bufs=4, space="PSUM"))
            xtpool = ps.enter_context(tc.tile_pool(name="xtpool", bufs=3))
            aout = ps.enter_context(tc.tile_pool(name="aout", bufs=4))

            def load_xt(ch):
                n0 = ch * 512
                xT = xtpool.tile([P, ndc, 512], bf16, tag="xT", name="xT")
                for dc in range(ndc):
                    nc.sync.dma_start(
                        xT[:, dc, :], xT_d[dc * P : (dc + 1) * P, n0 : n0 + 512]
                    )
                return xT

            # first x chunk ahead of the (big) weight loads: the sync queue is
            # in-order, and the first matmuls need xT(ch0) + wq[dc0] only.
            xt_next = load_xt(0)
            for a in range(ndc):
                nc.sync.dma_start(wq_sb[:, a, :], wq_d[a * P : (a + 1) * P, :])
                nc.sync.dma_start(wk_sb[:, a, :], wk_d[a * P : (a + 1) * P, :])
                nc.sync.dma_start(wv_sb[:, a, :], wv_d[a * P : (a + 1) * P, :])
            bq_sb = wpool.tile([P, njt], f32)
            bk_sb = wpool.tile([P, njt], f32)
            nc.scalar.dma_start(bq_sb, bq_d.rearrange("(a p) -> p a", p=P))
            nc.scalar.dma_start(bk_sb, bk_d.rearrange("(a p) -> p a", p=P))

            for ch in range(nch):
                n0 = ch * 512
                xT = xt_next
                if ch + 1 < nch:
                    xt_next = load_xt(ch + 1)
                # qT / kT (transposed outputs, bias per-partition)
                for w_sb, b_sb, dst in ((wq_sb, bq_sb, qT_d), (wk_sb, bk_sb, kT_d)):
                    for jt in range(njt):
                        pp = psA.tile([P, 512], f32, tag="ps_a")
                        for dc in range(ndc):
                            nc.tensor.matmul(
                                pp,
                                w_sb[:, dc, jt * P : (jt + 1) * P],
                                xT[:, dc, :],
                                start=(dc == 0),
                                stop=(dc == ndc - 1),
                            )
                        ot = aout.tile([P, 512], bf16, tag="aout")
                        nc.scalar.activation(
                            ot, pp, AF.Identity, bias=b_sb[:, jt : jt + 1]
                        )
                        nc.scalar.dma_start(
                            dst[jt * P : (jt + 1) * P, n0 : n0 + 512], ot
                        )
                # v (natural layout, no bias -- folded to host)
                for ns in range(4):
                    pp = psA.tile([P, dl], f32, tag="ps_a")
                    for dc in range(ndc):
                        nc.tensor.matmul(
                            pp,
                            xT[:, dc, ns * P : (ns + 1) * P],
                            wv_sb[:, dc, :],
                            start=(dc == 0),
                            stop=(dc == ndc - 1),
                        )
                    ot = aout.tile([P, dl], bf16, tag="aout_v")
                    nc.scalar.copy(ot, pp)
                    nc.scalar.dma_start(v_d[n0 + ns * P : n0 + (ns + 1) * P, :], ot)

        # ---------------- pass B: attention ----------------
        with ExitStack() as ps:
            apool = ps.enter_context(tc.tile_pool(name="apool", bufs=4))
            agp = ps.enter_context(tc.tile_pool(name="agp", bufs=2))
            agr = ps.enter_context(tc.tile_pool(name="agr", bufs=2))
            stat = ps.enter_context(tc.tile_pool(name="stat", bufs=4))
            tgp = ps.enter_context(tc.tile_pool(name="tgp", bufs=4))
            psS = ps.enter_context(tc.tile_pool(name="psS", bufs=3, space="PSUM"))
            psC = ps.enter_context(tc.tile_pool(name="psC", bufs=3, space="PSUM"))
            psQ = ps.enter_context(tc.tile_pool(name="psQ", bufs=1, space="PSUM"))

            p0s = (g0 % 2) * BS
            p1s = (g1 % 2) * BS

            for h in range(hpc):
                r0 = h * BS
                qz, kp, va = qz_s[h % 2], kp_s[h % 2], va_s[h % 2]
                kg, vg, qg = kg_s[h % 2], vg_s[h % 2], qg_s[h % 2]

                # -- per-head DMAs (overlap previous head's compute) --
                nc.sync.dma_start(qz[0:64, :], qT_d[r0 : r0 + BS, :])
                nc.sync.dma_start(kp[0:64, BS : BS + n], kT_d[r0 : r0 + BS, :])
                nc.sync.dma_start(kp[0:64, 0:BS], kT_d[r0 : r0 + BS, n - BS : n])
                nc.sync.dma_start(kp[0:64, BS + n :], kT_d[r0 : r0 + BS, 0:BS])
                vs = v_d[:, r0 : r0 + BS]
                nc.sync.dma_start(va[0:BS, 0, 0:BS], vs[n - BS : n, :])
                nc.sync.dma_start(va[BS:P, 0, 0:BS], vs[0:BS, :])
                nc.sync.dma_start(
                    va[:, 1 : nkc - 1, 0:BS],
                    vs[BS : n - BS, :].rearrange("(a p) c -> p a c", p=P),
                )
                nc.sync.dma_start(va[0:BS, nkc - 1, 0:BS], vs[n - BS : n, :])
                nc.sync.dma_start(va[BS:P, nkc - 1, 0:BS], vs[0:BS, :])
                for gi, gv in enumerate((g0, g1)):
                    nc.sync.dma_start(
                        kg[0:64, gi * BS : (gi + 1) * BS],
                        kT_d[r0 : r0 + BS, gv * BS : (gv + 1) * BS],
                    )
                    nc.sync.dma_start(
                        vg[gi * BS : (gi + 1) * BS, 0:BS],
                        vs[gv * BS : (gv + 1) * BS, :],
                    )
                    nc.sync.dma_start(
                        qg[0:64, gi * BS : (gi + 1) * BS],
                        qT_d[r0 : r0 + BS, gv * BS : (gv + 1) * BS],
                    )

                # -- local + global-col scores (S^T layout), exp, AV --
                def sc_pair(pr):
                    """scores+exp for padded key chunks 2pr, 2pr+1 (batched)."""
                    sps = psS.tile([P, 2, 256], f32, tag="sps")
                    a_sb = apool.tile([P, 2, 256], bf16, tag="a")
                    nws = []
                    for i in (0, 1):
                        c = 2 * pr + i
                        if c > nt:
                            continue
                        lo = max(0, (c - 1)) * P
                        hi = min(nt, c + 1) * P
                        nws.append(hi - lo)
                        nc.tensor.matmul(
                            sps[:, i, 0 : hi - lo],
                            kp[:, c * P : (c + 1) * P],
                            qz[:, lo:hi],
                            start=True,
                            stop=True,
                        )
                    if nws == [256, 256]:
                        nc.scalar.activation(a_sb, sps, AF.Exp)
                    else:
                        for i, nw in enumerate(nws):
                            nc.scalar.activation(
                                a_sb[:, i, 0:nw], sps[:, i, 0:nw], AF.Exp
                            )
                    return a_sb

                def gc_group(j):
                    spg = psS.tile([P, 512], f32, tag="sps")
                    nc.tensor.matmul(
                        spg, kg, qz[:, j * 512 : (j + 1) * 512], start=True, stop=True
                    )
                    ag = agp.tile([P, 512], bf16, tag="ag")
                    nc.scalar.activation(ag, spg, AF.Exp)
                    return ag

                ag_cur = gc_group(0)
                a_pair = {0: sc_pair(0), 1: sc_pair(1)}
                for t in range(nt):
                    if t % 4 == 0 and t > 0:
                        ag_cur = gc_group(t // 4)
                    want = min(nt // 2, t // 2 + 2)
                    if want not in a_pair:
                        a_pair[want] = sc_pair(want)
                        a_pair.pop(want - 3, None)
                    a_lo = a_pair[t // 2][:, t % 2, :]
                    off = 0 if t == 0 else P
                    a_up = a_pair[(t + 1) // 2][:, (t + 1) % 2, :]
                    cps = psC.tile([P, 130], f32, tag="cps")
                    nc.tensor.matmul(
                        cps[:, 0:65],
                        a_lo[:, off : off + P],
                        va[:, t, :],
                        start=True,
                        stop=False,
                    )
                    nc.tensor.matmul(
                        cps[:, 0:65],
                        a_up[:, 0:P],
                        va[:, t + 1, :],
                        start=False,
                        stop=True,
                    )
                    nc.tensor.matmul(
                        cps[:, 65:130],
                        ag_cur[:, (t % 4) * P : (t % 4 + 1) * P],
                        vg,
                        start=True,
                        stop=True,
                    )
                    # normalization: per-partition (per-query) reciprocals
                    r2 = stat.tile([P, 2], f32, tag="r2")
                    nc.vector.reciprocal(r2, cps[:, 64:130:65])
                    tg = tgp.tile([P, BS], f32, tag="tg")
                    nc.vector.tensor_scalar_mul(tg, cps[:, 65:129], r2[:, 1:2])
                    nc.vector.scalar_tensor_tensor(
                        ctx_nat[:, t, r0 : r0 + BS],
                        cps[:, 0:64],
                        r2[:, 0:1],
                        tg,
                        OP.mult,
                        OP.add,
                    )

                # -- global rows: full attention for the 2 global q blocks --
                cpr0 = psQ.tile([P, 65], f32, tag="cpr0")
                cpr1 = psQ.tile([P, 65], f32, tag="cpr1")
                for j in range(8):
                    spr = psS.tile([P, 4, P], f32, tag="sps")
                    for i in range(4):
                        c = 1 + 4 * j + i
                        nc.tensor.matmul(
                            spr[:, i, :],
                            kp[:, c * P : (c + 1) * P],
                            qg,
                            start=True,
                            stop=True,
                        )
                    ar = agr.tile([P, 4, P], bf16, tag="ar")
                    nc.scalar.activation(ar, spr, AF.Exp)
                    for i in range(4):
                        c = 1 + 4 * j + i
                        nc.tensor.matmul(
                            cpr0[p0s : p0s + BS, :],
                            ar[:, i, 0:BS],
                            va[:, c, :],
                            start=(c == 1),
                            stop=(c == nkc - 1),
                        )
                        nc.tensor.matmul(
                            cpr1[p1s : p1s + BS, :],
                            ar[:, i, BS:P],
                            va[:, c, :],
                            start=(c == 1),
                            stop=(c == nkc - 1),
                        )
                for gi, gv, pb, cpr in ((0, g0, p0s, cpr0), (1, g1, p1s, cpr1)):
                    rg = stat.tile([P, 1], f32, tag=f"rg{gi}")
                    nc.vector.reciprocal(
                        rg[pb : pb + BS, :], cpr[pb : pb + BS, 64:65]
                    )
                    nc.vector.tensor_scalar_mul(
                        ctx_nat[pb : pb + BS, gv // 2, r0 : r0 + BS],
                        cpr[pb : pb + BS, 0:64],
                        rg[pb : pb + BS, :],
                    )

        # ---------------- pass C: output projection ----------------
        with ExitStack() as ps:
            wop = ps.enter_context(tc.tile_pool(name="wop", bufs=1))
            wo_sb = wop.tile([P, ndc2, dmodel], bf16)
            nc.sync.dma_start(wo_sb, wo_d.rearrange("(a p) o -> p a o", p=P))
            ctp = ps.enter_context(tc.tile_pool(name="ctp", bufs=2))
            copool = ps.enter_context(tc.tile_pool(name="co", bufs=4))
            psT = ps.enter_context(tc.tile_pool(name="psT", bufs=4, space="PSUM"))
            psO = ps.enter_context(tc.tile_pool(name="psO", bufs=4, space="PSUM"))
            for ncc in range(nch):
                ctxT = ctp.tile([P, ndc2, 512], bf16, tag="ctxT")
                for ti in range(4):
                    t = ncc * 4 + ti
                    for dc in range(ndc2):
                        tp = psT.tile([P, P], bf16, tag="tp")
                        nc.tensor.transpose(
                            tp, ctx_nat[:, t, dc * P : (dc + 1) * P], ident
                        )
                        nc.scalar.copy(ctxT[:, dc, ti * P : (ti + 1) * P], tp)
                for ot in range(dmodel // P):
                    pp = psO.tile([P, 512], f32, tag="pso")
                    for dc in range(ndc2):
                        nc.tensor.matmul(
                            pp,
                            wo_sb[:, dc, ot * P : (ot + 1) * P],
                            ctxT[:, dc, :],
                            start=(dc == 0),
                            stop=(dc == ndc2 - 1),
                        )
                    ob = copool.tile([P, 512], bf16, tag="ob")
                    if ot % 2 == 0:
                        nc.scalar.copy(ob, pp)
                    else:
                        nc.vector.tensor_copy(ob, pp)
                    nc.sync.dma_start(
                        out_d[ot * P : (ot + 1) * P, ncc * 512 : (ncc + 1) * 512], ob
                    )

        if dbg:
            nc.sync.dma_start(qTo_d[:, :], qT_d)
            nc.sync.dma_start(kTo_d[:, :], kT_d)
            nc.sync.dma_start(vo_d[:, :], v_d)
            nc.sync.dma_start(ctxo_d[:, :, :], ctx_nat)

    nc.finalize()
    return nc


@functools.lru_cache(maxsize=8)
def _get(n, dmodel, dl, g0, g1):
    return _build(n, dmodel, dl, g0, g1)


def _prepare(inputs):
    """Build (nc, in_maps, meta) for the SPMD run from full unsharded inputs."""
    x = np.asarray(inputs["x"], np.float32)
    Wq = np.asarray(inputs["Wq"], np.float32)
    Wk = np.asarray(inputs["Wk"], np.float32)
    Wv = np.asarray(inputs["Wv"], np.float32)
    Wo = np.asarray(inputs["Wo"], np.float32)
    bq = np.asarray(inputs["bq"], np.float32)
    bk = np.asarray(inputs["bk"], np.float32)
    bv = np.asarray(inputs["bv"], np.float32)
    bo = np.asarray(inputs["bo"], np.float32)
    gi = np.asarray(inputs["global_indices"]).astype(np.int64)
    g0, g1 = int(gi[0]), int(gi[1])

    b_, n_, d_ = x.shape
    dl = d_ // 2
    scale = 1.0 / np.sqrt(np.float32(64.0)).astype(np.float32)

    nc = _get(n_, d_, dl, g0, g1)

    import ml_dtypes

    bf = ml_dtypes.bfloat16
    # mask pattern rows (periodic in the column index, see pass B docstring)
    NEGf = np.float32(-1e9)
    j = np.arange(n_) % 256
    qmask = np.zeros((4, n_), np.float32)
    qmask[0, (j >= 128) & (j < 192)] = 1.0  # w1e
    qmask[1, (j >= 64) & (j < 128)] = 1.0   # w2e
    qmask[2, j < 64] = 1.0                  # w1o
    qmask[3, j >= 192] = 1.0                # w2o
    qmask = np.ascontiguousarray(qmask).astype(bf)
    jk = np.arange(n_ + 128) % 256
    kmask = np.zeros((4, n_ + 128), np.float32)
    kmask[0, (jk >= 64) & (jk < 128)] = NEGf   # u1e
    kmask[1, jk < 64] = NEGf                   # u2e
    kmask[2, jk >= 192] = NEGf                 # u1o
    kmask[3, (jk >= 128) & (jk < 192)] = NEGf  # u2o
    kmask = np.ascontiguousarray(kmask).astype(bf)
    in_maps = []
    for c in range(8):
        b, hg = divmod(c, 2)
        S = slice(hg * dl, (hg + 1) * dl)
        in_maps.append(
            {
                "xT": np.ascontiguousarray(x[b].T).astype(bf),
                "qmask": qmask,
                "kmask": kmask,
                "wqT": np.ascontiguousarray((Wq[S, :] * scale).T).astype(bf),
                "wkT": np.ascontiguousarray(Wk[S, :].T).astype(bf),
                "wvT": np.ascontiguousarray(Wv[S, :].T).astype(bf),
                "woT": np.ascontiguousarray(Wo[:, S].T).astype(bf),
                "bq": np.ascontiguousarray(bq[S] * scale),
                "bk": np.ascontiguousarray(bk[S]),
            }
        )

    # host-side bv correction: out[q] += c(q) * bv @ Wo.T, c(q)=1 on global
    # blocks (overwritten by full-attention rows), else 2.
    bvWo = bv @ Wo.T  # [d_model]
    coef = np.full((n_, 1), 2.0, np.float32)
    bs = 64
    coef[g0 * bs : (g0 + 1) * bs] = 1.0
    coef[g1 * bs : (g1 + 1) * bs] = 1.0
    corr = (coef * bvWo[None, :] + bo[None, :]).astype(np.float32)

    return nc, in_maps, (b_, n_, d_, corr)


def _combine(res, meta):
    b_, n_, d_, corr = meta
    out = np.empty((b_, n_, d_), np.float32)
    for b in range(b_):
        out[b] = (
            res[2 * b]["outT"].T.astype(np.float32)
            + res[2 * b + 1]["outT"].T.astype(np.float32)
            + corr
        )
    return out


def kernel(**inputs):
    _ensure_path()
    from concourse.bass_utils import run_bass_kernel_spmd

    nc, in_maps, meta = _prepare(inputs)
    res = run_bass_kernel_spmd(nc, in_maps, list(range(8))).results
    return _combine(res, meta)


# revision 19
# speedup vs baseline: 2.2546x; 1.0366x over previous
"""BigBird attention (B=4, N=4096, D=1024, H=16, BS=64) on 8 TRN2 NeuronCores.

Sharding: batch (4-way) x head-group (2-way).  Core c handles batch c//2 and
heads [hg*8, hg*8+8) where hg = c%2 (d_model slice [hg*512, hg*512+512)).

Per core:
  pass A: QKV projections.  x.T tiles produced with DMA transposes; q/k
          emitted transposed (qT/kT: [dl, n], head dim on partitions), v
          natural.  score scale folded into Wq/bq on the host; bv dropped
          entirely (attention is affine in v: host adds c(q)*bv@Wo.T).
  pass B: per-head BigBird attention, all scores computed transposed
          (S^T = K_chunk^T Q, keys on partitions) so probabilities feed the
          AV matmuls directly as stationary operands -- no PE transposes.
          The sliding-window mask is folded into 4 extra contraction rows
          (rank-2 outer product of periodic 0/1 q-patterns and -1e9
          k-patterns), so exp() yields exact zeros in the masked corners.
          No max subtraction (scores bounded ~|3|).  V carries a ones
          column so each AV matmul also emits the softmax denominator
          per-partition; normalization is a per-partition reciprocal.
  pass C: transpose ctx with the PE, then row-parallel output projection
          -> partial outT [d_model, n] (f32).
Host combines: out[b] = outT(core 2b).T + outT(core 2b+1).T + bo + c(q)*bv@Wo.T
with c(q) = 1 for rows in global blocks else 2.

The kernel is specialized (compiled) per global_indices value.
"""

import functools
import sys

import numpy as np

P = 128
BS = 64
NEG = -1e9


def _ensure_path():
    try:
        import concourse.bass  # noqa: F401
    except ImportError:
        sys.path.insert(0, "/opt/trn_rl_repo")


def _build(n, dmodel, dl, g0, g1, dbg=0):
    """Build the per-core Bass program.

    n: sequence length per core, dmodel: model dim, dl: local head dims =
    hpc*64.  g0, g1: global block indices (compile-time constants).
    """
    _ensure_path()
    from contextlib import ExitStack

    import concourse.bass as bass  # noqa: F401
    import concourse.tile as tile
    from concourse import bacc, mybir
    from concourse.masks import make_identity

    f32 = mybir.dt.float32
    bf16 = mybir.dt.bfloat16
    AF = mybir.ActivationFunctionType
    OP = mybir.AluOpType

    nch = n // 512     # 512-column chunks of the sequence
    ndc = dmodel // P  # contraction chunks for QKV proj
    njt = dl // P      # row tiles of qT/kT
    hpc = dl // BS     # heads per core
    nt = n // P        # query tiles (2 blocks each)
    nkc = nt + 1       # padded key chunks (128 keys each, shifted by -BS)
    ndc2 = dl // P     # contraction chunks for out proj

    nc = bacc.Bacc(None, target_bir_lowering=False, debug=False)

    xT_d = nc.dram_tensor("xT", [dmodel, n], bf16, kind="ExternalInput")
    wq_d = nc.dram_tensor("wqT", [dmodel, dl], bf16, kind="ExternalInput")
    wk_d = nc.dram_tensor("wkT", [dmodel, dl], bf16, kind="ExternalInput")
    wv_d = nc.dram_tensor("wvT", [dmodel, dl], bf16, kind="ExternalInput")
    wo_d = nc.dram_tensor("woT", [dl, dmodel], bf16, kind="ExternalInput")
    bq_d = nc.dram_tensor("bq", [dl], f32, kind="ExternalInput")
    bk_d = nc.dram_tensor("bk", [dl], f32, kind="ExternalInput")
    qm_d = nc.dram_tensor("qmask", [64, n], bf16, kind="ExternalInput")
    km_d = nc.dram_tensor("kmask", [64, n + 2 * BS], bf16, kind="ExternalInput")
    out_d = nc.dram_tensor("outT", [dmodel, n], bf16, kind="ExternalOutput")
    if dbg:
        qTo_d = nc.dram_tensor("qTo", [dl, n], bf16, kind="ExternalOutput")
        kTo_d = nc.dram_tensor("kTo", [dl, n], bf16, kind="ExternalOutput")
        vo_d = nc.dram_tensor("vo", [n, dl], bf16, kind="ExternalOutput")
        ctxo_d = nc.dram_tensor("ctxo", [P, n // P, dl], bf16, kind="ExternalOutput")

    with tile.TileContext(nc) as tc, ExitStack() as top:
        dram = top.enter_context(tc.tile_pool(name="dram", bufs=1, space="DRAM"))
        qT_d = dram.tile([dl, n], bf16)
        kT_d = dram.tile([dl, n], bf16)
        v_d = dram.tile([n, dl], bf16)

        const = top.enter_context(tc.tile_pool(name="const", bufs=1))
        ident = const.tile([P, P], bf16)
        make_identity(nc, ident)

        # ctx natural accumulator: [q mod 128, tile, head*64+dh], SBUF-resident
        ctx_pool = top.enter_context(tc.tile_pool(name="ctx", bufs=1))
        ctx_nat = ctx_pool.tile([P, nt, dl], bf16)

        # pass-B per-head slots (manual ping-pong).  Allocated at top level so
        # their memory is disjoint from the pass-A pools: the constant regions
        # (mask rows, ones columns) are written once, up front.
        slot = top.enter_context(tc.tile_pool(name="slot", bufs=1))
        qz_s = [slot.tile([P, n], bf16, tag=f"qz{i}", name=f"qz{i}") for i in range(2)]
        kp_s = [slot.tile([P, n + 2 * BS], bf16, tag=f"kp{i}", name=f"kp{i}") for i in range(2)]
        va_s = [slot.tile([P, nkc, BS + 1], bf16, tag=f"va{i}", name=f"va{i}") for i in range(2)]
        kg_s = [slot.tile([P, P], bf16, tag=f"kg{i}", name=f"kg{i}") for i in range(2)]
        vg_s = [slot.tile([P, BS + 1], bf16, tag=f"vg{i}", name=f"vg{i}") for i in range(2)]
        qg_s = [slot.tile([P, P], bf16, tag=f"qg{i}", name=f"qg{i}") for i in range(2)]
        for qz in qz_s:
            nc.sync.dma_start(qz[64:P, :], qm_d[:, :])
        for kp in kp_s:
            nc.sync.dma_start(kp[64:P, :], km_d[:, :])
        for kg in kg_s:
            nc.gpsimd.memset(kg[64:P, :], 0.0)
        for qg in qg_s:
            nc.gpsimd.memset(qg[64:P, :], 0.0)
        for va in va_s:
            nc.gpsimd.memset(va[:, :, BS : BS + 1], 1.0)
        for vg in vg_s:
            nc.gpsimd.memset(vg[:, BS : BS + 1], 1.0)

        # ---------------- pass A: projections ----------------
        with ExitStack() as ps:
            wpool = ps.enter_context(tc.tile_pool(name="wpool", bufs=1))
            wq_sb = wpool.tile([P, ndc, dl], bf16)
            wk_sb = wpool.tile([P, ndc, dl], bf16)
            wv_sb = wpool.tile([P, ndc, dl], bf16)
            psA = ps.enter_context(tc.tile_pool(name="psA", bufs=4, space="PSUM"))
            xtpool = ps.enter_context(tc.tile_pool(name="xtpool", bufs=3))
            aout = ps.enter_context(tc.tile_pool(name="aout", bufs=4))

            def load_xt(ch):
                n0 = ch * 512
                xT = xtpool.tile([P, ndc, 512], bf16, tag="xT", name="xT")
                for dc in range(ndc):
                    nc.sync.dma_start(
                        xT[:, dc, :], xT_d[dc * P : (dc + 1) * P, n0 : n0 + 512]
                    )
                return xT

            # first x chunk ahead of the (big) weight loads: the sync queue is
            # in-order, and the first matmuls need xT(ch0) + wq[dc0] only.
            xt_next = load_xt(0)
            for a in range(ndc):
                nc.sync.dma_start(wq_sb[:, a, :], wq_d[a * P : (a + 1) * P, :])
                nc.sync.dma_start(wk_sb[:, a, :], wk_d[a * P : (a + 1) * P, :])
                nc.sync.dma_start(wv_sb[:, a, :], wv_d[a * P : (a + 1) * P, :])
            bq_sb = wpool.tile([P, njt], f32)
            bk_sb = wpool.tile([P, njt], f32)
            nc.scalar.dma_start(bq_sb, bq_d.rearrange("(a p) -> p a", p=P))
            nc.scalar.dma_start(bk_sb, bk_d.rearrange("(a p) -> p a", p=P))

            for ch in range(nch):
                n0 = ch * 512
                xT = xt_next
                if ch + 1 < nch:
                    xt_next = load_xt(ch + 1)
                # qT / kT (transposed outputs, bias per-partition)
                for w_sb, b_sb, dst in ((wq_sb, bq_sb, qT_d), (wk_sb, bk_sb, kT_d)):
                    for jt in range(njt):
                        pp = psA.tile([P, 512], f32, tag="ps_a")
                        for dc in range(ndc):
                            nc.tensor.matmul(
                                pp,
                                w_sb[:, dc, jt * P : (jt + 1) * P],
                                xT[:, dc, :],
                                start=(dc == 0),
                                stop=(dc == ndc - 1),
                            )
                        ot = aout.tile([P, 512], bf16, tag="aout")
                        nc.scalar.activation(
                            ot, pp, AF.Identity, bias=b_sb[:, jt : jt + 1]
                        )
                        nc.scalar.dma_start(
                            dst[jt * P : (jt + 1) * P, n0 : n0 + 512], ot
                        )
                # v (natural layout, no bias -- folded to host)
                for ns in range(4):
                    pp = psA.tile([P, dl], f32, tag="ps_a")
                    for dc in range(ndc):
                        nc.tensor.matmul(
                            pp,
                            xT[:, dc, ns * P : (ns + 1) * P],
                            wv_sb[:, dc, :],
                            start=(dc == 0),
                            stop=(dc == ndc - 1),
                        )
                    ot = aout.tile([P, dl], bf16, tag="aout_v")
                    nc.scalar.copy(ot, pp)
                    nc.scalar.dma_start(v_d[n0 + ns * P : n0 + (ns + 1) * P, :], ot)

        # ---------------- pass B: attention ----------------
        with ExitStack() as ps:
            apool = ps.enter_context(tc.tile_pool(name="apool", bufs=4))
            agp = ps.enter_context(tc.tile_pool(name="agp", bufs=2))
            agr = ps.enter_context(tc.tile_pool(name="agr", bufs=2))
            stat = ps.enter_context(tc.tile_pool(name="stat", bufs=4))
            tgp = ps.enter_context(tc.tile_pool(name="tgp", bufs=4))
            psS = ps.enter_context(tc.tile_pool(name="psS", bufs=3, space="PSUM"))
            psC = ps.enter_context(tc.tile_pool(name="psC", bufs=3, space="PSUM"))
            psQ = ps.enter_context(tc.tile_pool(name="psQ", bufs=1, space="PSUM"))

            p0s = (g0 % 2) * BS
            p1s = (g1 % 2) * BS

            for h in range(hpc):
                r0 = h * BS
                qz, kp, va = qz_s[h % 2], kp_s[h % 2], va_s[h % 2]
                kg, vg, qg = kg_s[h % 2], vg_s[h % 2], qg_s[h % 2]

                # -- per-head DMAs (overlap previous head's compute) --
                nc.sync.dma_start(qz[0:64, :], qT_d[r0 : r0 + BS, :])
                nc.sync.dma_start(kp[0:64, BS : BS + n], kT_d[r0 : r0 + BS, :])
                nc.sync.dma_start(kp[0:64, 0:BS], kT_d[r0 : r0 + BS, n - BS : n])
                nc.sync.dma_start(kp[0:64, BS + n :], kT_d[r0 : r0 + BS, 0:BS])
                vs = v_d[:, r0 : r0 + BS]
                nc.sync.dma_start(va[0:BS, 0, 0:BS], vs[n - BS : n, :])
                nc.sync.dma_start(va[BS:P, 0, 0:BS], vs[0:BS, :])
                nc.sync.dma_start(
                    va[:, 1 : nkc - 1, 0:BS],
                    vs[BS : n - BS, :].rearrange("(a p) c -> p a c", p=P),
                )
                nc.sync.dma_start(va[0:BS, nkc - 1, 0:BS], vs[n - BS : n, :])
                nc.sync.dma_start(va[BS:P, nkc - 1, 0:BS], vs[0:BS, :])
                for gi, gv in enumerate((g0, g1)):
                    nc.sync.dma_start(
                        kg[0:64, gi * BS : (gi + 1) * BS],
                        kT_d[r0 : r0 + BS, gv * BS : (gv + 1) * BS],
                    )
                    nc.sync.dma_start(
                        vg[gi * BS : (gi + 1) * BS, 0:BS],
                        vs[gv * BS : (gv + 1) * BS, :],
                    )
                    nc.sync.dma_start(
                        qg[0:64, gi * BS : (gi + 1) * BS],
                        qT_d[r0 : r0 + BS, gv * BS : (gv + 1) * BS],
                    )

                # -- local + global-col scores (S^T layout), exp, AV --
                def sc_pair(pr):
                    """scores+exp for padded key chunks 2pr, 2pr+1 (batched)."""
                    sps = psS.tile([P, 2, 256], f32, tag="sps")
                    a_sb = apool.tile([P, 2, 256], bf16, tag="a")
                    nws = []
                    for i in (0, 1):
                        c = 2 * pr + i
                        if c > nt:
                            continue
                        lo = max(0, (c - 1)) * P
                        hi = min(nt, c + 1) * P
                        nws.append(hi - lo)
                        nc.tensor.matmul(
                            sps[:, i, 0 : hi - lo],
                            kp[:, c * P : (c + 1) * P],
                            qz[:, lo:hi],
                            start=True,
                            stop=True,
                        )
                    if nws == [256, 256]:
                        nc.scalar.activation(a_sb, sps, AF.Exp)
                    else:
                        for i, nw in enumerate(nws):
                            nc.scalar.activation(
                                a_sb[:, i, 0:nw], sps[:, i, 0:nw], AF.Exp
                            )
                    return a_sb

                def gc_group(j):
                    spg = psS.tile([P, 512], f32, tag="sps")
                    nc.tensor.matmul(
                        spg, kg, qz[:, j * 512 : (j + 1) * 512], start=True, stop=True
                    )
                    ag = agp.tile([P, 512], bf16, tag="ag")
                    nc.scalar.activation(ag, spg, AF.Exp)
                    return ag

                ag_cur = gc_group(0)
                a_pair = {0: sc_pair(0), 1: sc_pair(1)}
                for t in range(nt):
                    if t % 4 == 0 and t > 0:
                        ag_cur = gc_group(t // 4)
                    want = min(nt // 2, t // 2 + 2)
                    if want not in a_pair:
                        a_pair[want] = sc_pair(want)
                        a_pair.pop(want - 3, None)
                    a_lo = a_pair[t // 2][:, t % 2, :]
                    off = 0 if t == 0 else P
                    a_up = a_pair[(t + 1) // 2][:, (t + 1) % 2, :]
                    cps = psC.tile([P, 130], f32, tag="cps")
                    nc.tensor.matmul(
                        cps[:, 0:65],
                        a_lo[:, off : off + P],
                        va[:, t, :],
                        start=True,
                        stop=False,
                    )
                    nc.tensor.matmul(
                        cps[:, 0:65],
                        a_up[:, 0:P],
                        va[:, t + 1, :],
                        start=False,
                        stop=True,
                    )
                    nc.tensor.matmul(
                        cps[:, 65:130],
                        ag_cur[:, (t % 4) * P : (t % 4 + 1) * P],
                        vg,
                        start=True,
                        stop=True,
                    )
                    # normalization: per-partition (per-query) reciprocals
                    r2 = stat.tile([P, 2], f32, tag="r2")
                    nc.vector.reciprocal(r2, cps[:, 64:130:65])
                    tg = tgp.tile([P, BS], f32, tag="tg")
                    nc.vector.tensor_scalar_mul(tg, cps[:, 65:129], r2[:, 1:2])
                    nc.vector.scalar_tensor_tensor(
                        ctx_nat[:, t, r0 : r0 + BS],
                        cps[:, 0:64],
                        r2[:, 0:1],
                        tg,
                        OP.mult,
                        OP.add,
                    )

                # -- global rows: full attention for the 2 global q blocks --
                cpr0 = psQ.tile([P, 65], f32, tag="cpr0")
                cpr1 = psQ.tile([P, 65], f32, tag="cpr1")
                for j in range(8):
                    spr = psS.tile([P, 4, P], f32, tag="sps")
                    for i in range(4):
                        c = 1 + 4 * j + i
                        nc.tensor.matmul(
                            spr[:, i, :],
                            kp[:, c * P : (c + 1) * P],
                            qg,
                            start=True,
                            stop=True,
                        )
                    ar = agr.tile([P, 4, P], bf16, tag="ar")
                    nc.scalar.activation(ar, spr, AF.Exp)
                    for i in range(4):
                        c = 1 + 4 * j + i
                        nc.tensor.matmul(
                            cpr0[p0s : p0s + BS, :],
                            ar[:, i, 0:BS],
                            va[:, c, :],
                            start=(c == 1),
                            stop=(c == nkc - 1),
                        )
                        nc.tensor.matmul(
                            cpr1[p1s : p1s + BS, :],
                            ar[:, i, BS:P],
                            va[:, c, :],
                            start=(c == 1),
                            stop=(c == nkc - 1),
                        )
                for gi, gv, pb, cpr in ((0, g0, p0s, cpr0), (1, g1, p1s, cpr1)):
                    rg = stat.tile([P, 1], f32, tag=f"rg{gi}")
                    nc.vector.reciprocal(
                        rg[pb : pb + BS, :], cpr[pb : pb + BS, 64:65]
                    )
                    nc.vector.tensor_scalar_mul(
                        ctx_nat[pb : pb + BS, gv // 2, r0 : r0 + BS],
                        cpr[pb : pb + BS, 0:64],
                        rg[pb : pb + BS, :],
                    )

        # ---------------- pass C: output projection ----------------
        with ExitStack() as ps:
            wop = ps.enter_context(tc.tile_pool(name="wop", bufs=1))
            wo_sb = wop.tile([P, ndc2, dmodel], bf16)
            nc.sync.dma_start(wo_sb, wo_d.rearrange("(a p) o -> p a o", p=P))
            ctp = ps.enter_context(tc.tile_pool(name="ctp", bufs=2))
            copool = ps.enter_context(tc.tile_pool(name="co", bufs=4))
            psT = ps.enter_context(tc.tile_pool(name="psT", bufs=4, space="PSUM"))
            psO = ps.enter_context(tc.tile_pool(name="psO", bufs=4, space="PSUM"))
            for ncc in range(nch):
                ctxT = ctp.tile([P, ndc2, 512], bf16, tag="ctxT")
                for ti in range(4):
                    t = ncc * 4 + ti
                    for dc in range(ndc2):
                        tp = psT.tile([P, P], bf16, tag="tp")
                        nc.tensor.transpose(
                            tp, ctx_nat[:, t, dc * P : (dc + 1) * P], ident
                        )
                        nc.scalar.copy(ctxT[:, dc, ti * P : (ti + 1) * P], tp)
                for ot in range(dmodel // P):
                    pp = psO.tile([P, 512], f32, tag="pso")
                    for dc in range(ndc2):
                        nc.tensor.matmul(
                            pp,
                            wo_sb[:, dc, ot * P : (ot + 1) * P],
                            ctxT[:, dc, :],
                            start=(dc == 0),
                            stop=(dc == ndc2 - 1),
                        )
                    ob = copool.tile([P, 512], bf16, tag="ob")
                    if ot % 2 == 0:
                        nc.scalar.copy(ob, pp)
                    else:
                        nc.vector.tensor_copy(ob, pp)
                    nc.sync.dma_start(
                        out_d[ot * P : (ot + 1) * P, ncc * 512 : (ncc + 1) * 512], ob
                    )

        if dbg:
            nc.sync.dma_start(qTo_d[:, :], qT_d)
            nc.sync.dma_start(kTo_d[:, :], kT_d)
            nc.sync.dma_start(vo_d[:, :], v_d)
            nc.sync.dma_start(ctxo_d[:, :, :], ctx_nat)

    nc.finalize()
    return nc


@functools.lru_cache(maxsize=8)
def _get(n, dmodel, dl, g0, g1):
    return _build(n, dmodel, dl, g0, g1)


def _prepare(inputs):
    """Build (nc, in_maps, meta) for the SPMD run from full unsharded inputs."""
    x = np.asarray(inputs["x"], np.float32)
    Wq = np.asarray(inputs["Wq"], np.float32)
    Wk = np.asarray(inputs["Wk"], np.float32)
    Wv = np.asarray(inputs["Wv"], np.float32)
    Wo = np.asarray(inputs["Wo"], np.float32)
    bq = np.asarray(inputs["bq"], np.float32)
    bk = np.asarray(inputs["bk"], np.float32)
    bv = np.asarray(inputs["bv"], np.float32)
    bo = np.asarray(inputs["bo"], np.float32)
    gi = np.asarray(inputs["global_indices"]).astype(np.int64)
    g0, g1 = int(gi[0]), int(gi[1])

    b_, n_, d_ = x.shape
    dl = d_ // 2
    scale = 1.0 / np.sqrt(np.float32(64.0)).astype(np.float32)

    nc = _get(n_, d_, dl, g0, g1)

    import ml_dtypes

    bf = ml_dtypes.bfloat16
    # mask pattern rows (periodic in the column index, see pass B docstring)
    NEGf = np.float32(-1e9)
    j = np.arange(n_) % 256
    qmask = np.zeros((64, n_), np.float32)
    qmask[0, (j >= 128) & (j < 192)] = 1.0  # w1e
    qmask[1, (j >= 64) & (j < 128)] = 1.0   # w2e
    qmask[2, j < 64] = 1.0                  # w1o
    qmask[3, j >= 192] = 1.0                # w2o
    qmask = np.ascontiguousarray(qmask).astype(bf)
    jk = np.arange(n_ + 128) % 256
    kmask = np.zeros((64, n_ + 128), np.float32)
    kmask[0, (jk >= 64) & (jk < 128)] = NEGf   # u1e
    kmask[1, jk < 64] = NEGf                   # u2e
    kmask[2, jk >= 192] = NEGf                 # u1o
    kmask[3, (jk >= 128) & (jk < 192)] = NEGf  # u2o
    kmask = np.ascontiguousarray(kmask).astype(bf)
    in_maps = []
    for c in range(8):
        b, hg = divmod(c, 2)
        S = slice(hg * dl, (hg + 1) * dl)
        in_maps.append(
            {
                "xT": np.ascontiguousarray(x[b].T).astype(bf),
                "qmask": qmask,
                "kmask": kmask,
                "wqT": np.ascontiguousarray((Wq[S, :] * scale).T).astype(bf),
                "wkT": np.ascontiguousarray(Wk[S, :].T).astype(bf),
                "wvT": np.ascontiguousarray(Wv[S, :].T).astype(bf),
                "woT": np.ascontiguousarray(Wo[:, S].T).astype(bf),
                "bq": np.ascontiguousarray(bq[S] * scale),
                "bk": np.ascontiguousarray(bk[S]),
            }
        )

    # host-side bv correction: out[q] += c(q) * bv @ Wo.T, c(q)=1 on global
    # blocks (overwritten by full-attention rows), else 2.
    bvWo = bv @ Wo.T  # [d_model]
    coef = np.full((n_, 1), 2.0, np.float32)
    bs = 64
    coef[g0 * bs : (g0 + 1) * bs] = 1.0
    coef[g1 * bs : (g1 + 1) * bs] = 1.0
    corr = (coef * bvWo[None, :] + bo[None, :]).astype(np.float32)

    return nc, in_maps, (b_, n_, d_, corr)


def _combine(res, meta):
    b_, n_, d_, corr = meta
    out = np.empty((b_, n_, d_), np.float32)
    for b in range(b_):
        out[b] = (
            res[2 * b]["outT"].T.astype(np.float32)
            + res[2 * b + 1]["outT"].T.astype(np.float32)
            + corr
        )
    return out


def kernel(**inputs):
    _ensure_path()
    from concourse.bass_utils import run_bass_kernel_spmd

    nc, in_maps, meta = _prepare(inputs)
    res = run_bass_kernel_spmd(nc, in_maps, list(range(8))).results
    return _combine(res, meta)


# revision 20
# speedup vs baseline: 2.3174x; 1.0279x over previous
"""BigBird attention (B=4, N=4096, D=1024, H=16, BS=64) on 8 TRN2 NeuronCores.

Sharding: batch (4-way) x head-group (2-way).  Core c handles batch c//2 and
heads [hg*8, hg*8+8) where hg = c%2 (d_model slice [hg*512, hg*512+512)).

Per core:
  pass A: QKV projections.  x.T tiles produced with DMA transposes; q/k
          emitted transposed (qT/kT: [dl, n], head dim on partitions), v
          natural.  score scale folded into Wq/bq on the host; bv dropped
          entirely (attention is affine in v: host adds c(q)*bv@Wo.T).
  pass B: per-head BigBird attention, all scores computed transposed
          (S^T = K_chunk^T Q, keys on partitions) so probabilities feed the
          AV matmuls directly as stationary operands -- no PE transposes.
          The sliding-window mask is folded into 4 extra contraction rows
          (rank-2 outer product of periodic 0/1 q-patterns and -1e9
          k-patterns), so exp() yields exact zeros in the masked corners.
          No max subtraction (scores bounded ~|3|).  V carries a ones
          column so each AV matmul also emits the softmax denominator
          per-partition; normalization is a per-partition reciprocal.
  pass C: transpose ctx with the PE, then row-parallel output projection
          -> partial outT [d_model, n] (f32).
Host combines: out[b] = outT(core 2b).T + outT(core 2b+1).T + bo + c(q)*bv@Wo.T
with c(q) = 1 for rows in global blocks else 2.

The kernel is specialized (compiled) per global_indices value.
"""

import functools
import sys

import numpy as np

P = 128
BS = 64
NEG = -1e9


def _ensure_path():
    try:
        import concourse.bass  # noqa: F401
    except ImportError:
        sys.path.insert(0, "/opt/trn_rl_repo")


def _build(n, dmodel, dl, g0, g1, dbg=0):
    """Build the per-core Bass program.

    n: sequence length per core, dmodel: model dim, dl: local head dims =
    hpc*64.  g0, g1: global block indices (compile-time constants).
    """
    _ensure_path()
    from contextlib import ExitStack

    import concourse.bass as bass  # noqa: F401
    import concourse.tile as tile
    from concourse import bacc, mybir
    from concourse.masks import make_identity

    f32 = mybir.dt.float32
    bf16 = mybir.dt.bfloat16
    AF = mybir.ActivationFunctionType
    OP = mybir.AluOpType

    nch = n // 512     # 512-column chunks of the sequence
    ndc = dmodel // P  # contraction chunks for QKV proj
    njt = dl // P      # row tiles of qT/kT
    hpc = dl // BS     # heads per core
    nt = n // P        # query tiles (2 blocks each)
    nkc = nt + 1       # padded key chunks (128 keys each, shifted by -BS)
    ndc2 = dl // P     # contraction chunks for out proj

    nc = bacc.Bacc(None, target_bir_lowering=False, debug=False)

    xT_d = nc.dram_tensor("xT", [dmodel, n], bf16, kind="ExternalInput")
    wq_d = nc.dram_tensor("wqT", [dmodel, dl], bf16, kind="ExternalInput")
    wk_d = nc.dram_tensor("wkT", [dmodel, dl], bf16, kind="ExternalInput")
    wv_d = nc.dram_tensor("wvT", [dmodel, dl], bf16, kind="ExternalInput")
    wo_d = nc.dram_tensor("woT", [dl, dmodel], bf16, kind="ExternalInput")
    bq_d = nc.dram_tensor("bq", [dl], f32, kind="ExternalInput")
    bk_d = nc.dram_tensor("bk", [dl], f32, kind="ExternalInput")
    qm_d = nc.dram_tensor("qmask", [64, n], bf16, kind="ExternalInput")
    km_d = nc.dram_tensor("kmask", [64, n + 2 * BS], bf16, kind="ExternalInput")
    out_d = nc.dram_tensor("outT", [dmodel, n], bf16, kind="ExternalOutput")
    if dbg:
        qTo_d = nc.dram_tensor("qTo", [dl, n], bf16, kind="ExternalOutput")
        kTo_d = nc.dram_tensor("kTo", [dl, n], bf16, kind="ExternalOutput")
        vo_d = nc.dram_tensor("vo", [n, dl], bf16, kind="ExternalOutput")
        ctxo_d = nc.dram_tensor("ctxo", [P, n // P, dl], bf16, kind="ExternalOutput")

    with tile.TileContext(nc) as tc, ExitStack() as top:
        dram = top.enter_context(tc.tile_pool(name="dram", bufs=1, space="DRAM"))
        qT_d = dram.tile([dl, n], bf16)
        kT_d = dram.tile([dl, n], bf16)
        v_d = dram.tile([n, dl], bf16)

        const = top.enter_context(tc.tile_pool(name="const", bufs=1))
        ident = const.tile([P, P], bf16)
        make_identity(nc, ident)

        # ctx natural accumulator: [q mod 128, tile, head*64+dh], SBUF-resident
        ctx_pool = top.enter_context(tc.tile_pool(name="ctx", bufs=1))
        ctx_nat = ctx_pool.tile([P, nt, dl], bf16)

        # pass-B per-head slots (manual ping-pong).  Allocated at top level so
        # their memory is disjoint from the pass-A pools: the constant regions
        # (mask rows, ones columns) are written once, up front.
        slot = top.enter_context(tc.tile_pool(name="slot", bufs=1))
        qz_s = [slot.tile([P, n], bf16, tag=f"qz{i}", name=f"qz{i}") for i in range(2)]
        kp_s = [slot.tile([P, n + 2 * BS], bf16, tag=f"kp{i}", name=f"kp{i}") for i in range(2)]
        va_s = [slot.tile([P, nkc, BS + 1], bf16, tag=f"va{i}", name=f"va{i}") for i in range(2)]
        kg_s = [slot.tile([P, P], bf16, tag=f"kg{i}", name=f"kg{i}") for i in range(2)]
        vg_s = [slot.tile([P, BS + 1], bf16, tag=f"vg{i}", name=f"vg{i}") for i in range(2)]
        qg_s = [slot.tile([P, P], bf16, tag=f"qg{i}", name=f"qg{i}") for i in range(2)]
        def init_slot_consts():
            for qz in qz_s:
                nc.sync.dma_start(qz[64:P, :], qm_d[:, :])
            for kp in kp_s:
                nc.sync.dma_start(kp[64:P, :], km_d[:, :])
            for kg in kg_s:
                nc.gpsimd.memset(kg[64:P, :], 0.0)
            for qg in qg_s:
                nc.gpsimd.memset(qg[64:P, :], 0.0)
            for va in va_s:
                nc.gpsimd.memset(va[:, :, BS : BS + 1], 1.0)
            for vg in vg_s:
                nc.gpsimd.memset(vg[:, BS : BS + 1], 1.0)

        # ---------------- pass A: projections ----------------
        with ExitStack() as ps:
            wpool = ps.enter_context(tc.tile_pool(name="wpool", bufs=1))
            wq_sb = wpool.tile([P, ndc, dl], bf16)
            wk_sb = wpool.tile([P, ndc, dl], bf16)
            wv_sb = wpool.tile([P, ndc, dl], bf16)
            psA = ps.enter_context(tc.tile_pool(name="psA", bufs=4, space="PSUM"))
            xtpool = ps.enter_context(tc.tile_pool(name="xtpool", bufs=3))
            aout = ps.enter_context(tc.tile_pool(name="aout", bufs=4))

            def load_xt(ch):
                n0 = ch * 512
                xT = xtpool.tile([P, ndc, 512], bf16, tag="xT", name="xT")
                for dc in range(ndc):
                    nc.sync.dma_start(
                        xT[:, dc, :], xT_d[dc * P : (dc + 1) * P, n0 : n0 + 512]
                    )
                return xT

            # first x chunk ahead of the (big) weight loads: the sync queue is
            # in-order, and the first matmuls need xT(ch0) + wq[dc0] only.
            xt_next = load_xt(0)
            nc.sync.dma_start(wq_sb, wq_d.rearrange("(a p) j -> p a j", p=P))
            nc.sync.dma_start(wk_sb, wk_d.rearrange("(a p) j -> p a j", p=P))
            nc.sync.dma_start(wv_sb, wv_d.rearrange("(a p) j -> p a j", p=P))
            bq_sb = wpool.tile([P, njt], f32)
            bk_sb = wpool.tile([P, njt], f32)
            nc.scalar.dma_start(bq_sb, bq_d.rearrange("(a p) -> p a", p=P))
            nc.scalar.dma_start(bk_sb, bk_d.rearrange("(a p) -> p a", p=P))

            for ch in range(nch):
                n0 = ch * 512
                xT = xt_next
                if ch + 1 < nch:
                    xt_next = load_xt(ch + 1)
                if ch == 2:
                    init_slot_consts()
                # qT / kT (transposed outputs, bias per-partition)
                for w_sb, b_sb, dst in ((wq_sb, bq_sb, qT_d), (wk_sb, bk_sb, kT_d)):
                    for jt in range(njt):
                        pp = psA.tile([P, 512], f32, tag="ps_a")
                        for dc in range(ndc):
                            nc.tensor.matmul(
                                pp,
                                w_sb[:, dc, jt * P : (jt + 1) * P],
                                xT[:, dc, :],
                                start=(dc == 0),
                                stop=(dc == ndc - 1),
                            )
                        ot = aout.tile([P, 512], bf16, tag="aout")
                        nc.scalar.activation(
                            ot, pp, AF.Identity, bias=b_sb[:, jt : jt + 1]
                        )
                        nc.scalar.dma_start(
                            dst[jt * P : (jt + 1) * P, n0 : n0 + 512], ot
                        )
                # v (natural layout, no bias -- folded to host)
                for ns in range(4):
                    pp = psA.tile([P, dl], f32, tag="ps_a")
                    for dc in range(ndc):
                        nc.tensor.matmul(
                            pp,
                            xT[:, dc, ns * P : (ns + 1) * P],
                            wv_sb[:, dc, :],
                            start=(dc == 0),
                            stop=(dc == ndc - 1),
                        )
                    ot = aout.tile([P, dl], bf16, tag="aout_v")
                    nc.scalar.copy(ot, pp)
                    nc.scalar.dma_start(v_d[n0 + ns * P : n0 + (ns + 1) * P, :], ot)

        # ---------------- pass B: attention ----------------
        with ExitStack() as ps:
            apool = ps.enter_context(tc.tile_pool(name="apool", bufs=4))
            agp = ps.enter_context(tc.tile_pool(name="agp", bufs=2))
            agr = ps.enter_context(tc.tile_pool(name="agr", bufs=2))
            stat = ps.enter_context(tc.tile_pool(name="stat", bufs=4))
            tgp = ps.enter_context(tc.tile_pool(name="tgp", bufs=4))
            psS = ps.enter_context(tc.tile_pool(name="psS", bufs=3, space="PSUM"))
            psC = ps.enter_context(tc.tile_pool(name="psC", bufs=3, space="PSUM"))
            psQ = ps.enter_context(tc.tile_pool(name="psQ", bufs=1, space="PSUM"))

            p0s = (g0 % 2) * BS
            p1s = (g1 % 2) * BS

            for h in range(hpc):
                r0 = h * BS
                qz, kp, va = qz_s[h % 2], kp_s[h % 2], va_s[h % 2]
                kg, vg, qg = kg_s[h % 2], vg_s[h % 2], qg_s[h % 2]

                # -- per-head DMAs (overlap previous head's compute) --
                nc.sync.dma_start(qz[0:64, :], qT_d[r0 : r0 + BS, :])
                nc.sync.dma_start(kp[0:64, BS : BS + n], kT_d[r0 : r0 + BS, :])
                nc.sync.dma_start(kp[0:64, 0:BS], kT_d[r0 : r0 + BS, n - BS : n])
                nc.sync.dma_start(kp[0:64, BS + n :], kT_d[r0 : r0 + BS, 0:BS])
                vs = v_d[:, r0 : r0 + BS]
                nc.sync.dma_start(va[0:BS, 0, 0:BS], vs[n - BS : n, :])
                nc.sync.dma_start(va[BS:P, 0, 0:BS], vs[0:BS, :])
                nc.sync.dma_start(
                    va[:, 1 : nkc - 1, 0:BS],
                    vs[BS : n - BS, :].rearrange("(a p) c -> p a c", p=P),
                )
                nc.sync.dma_start(va[0:BS, nkc - 1, 0:BS], vs[n - BS : n, :])
                nc.sync.dma_start(va[BS:P, nkc - 1, 0:BS], vs[0:BS, :])
                for gi, gv in enumerate((g0, g1)):
                    nc.sync.dma_start(
                        kg[0:64, gi * BS : (gi + 1) * BS],
                        kT_d[r0 : r0 + BS, gv * BS : (gv + 1) * BS],
                    )
                    nc.sync.dma_start(
                        vg[gi * BS : (gi + 1) * BS, 0:BS],
                        vs[gv * BS : (gv + 1) * BS, :],
                    )
                    nc.sync.dma_start(
                        qg[0:64, gi * BS : (gi + 1) * BS],
                        qT_d[r0 : r0 + BS, gv * BS : (gv + 1) * BS],
                    )

                # -- local + global-col scores (S^T layout), exp, AV --
                def sc_pair(pr):
                    """scores+exp for padded key chunks 2pr, 2pr+1 (batched)."""
                    sps = psS.tile([P, 2, 256], f32, tag="sps")
                    a_sb = apool.tile([P, 2, 256], bf16, tag="a")
                    nws = []
                    for i in (0, 1):
                        c = 2 * pr + i
                        if c > nt:
                            continue
                        lo = max(0, (c - 1)) * P
                        hi = min(nt, c + 1) * P
                        nws.append(hi - lo)
                        nc.tensor.matmul(
                            sps[:, i, 0 : hi - lo],
                            kp[:, c * P : (c + 1) * P],
                            qz[:, lo:hi],
                            start=True,
                            stop=True,
                        )
                    if nws == [256, 256]:
                        nc.scalar.activation(a_sb, sps, AF.Exp)
                    else:
                        for i, nw in enumerate(nws):
                            nc.scalar.activation(
                                a_sb[:, i, 0:nw], sps[:, i, 0:nw], AF.Exp
                            )
                    return a_sb

                def gc_group(j):
                    spg = psS.tile([P, 512], f32, tag="sps")
                    nc.tensor.matmul(
                        spg, kg, qz[:, j * 512 : (j + 1) * 512], start=True, stop=True
                    )
                    ag = agp.tile([P, 512], bf16, tag="ag")
                    nc.scalar.activation(ag, spg, AF.Exp)
                    return ag

                ag_cur = gc_group(0)
                a_pair = {0: sc_pair(0), 1: sc_pair(1)}
                for t in range(nt):
                    if t % 4 == 0 and t > 0:
                        ag_cur = gc_group(t // 4)
                    want = min(nt // 2, t // 2 + 2)
                    if want not in a_pair:
                        a_pair[want] = sc_pair(want)
                        a_pair.pop(want - 3, None)
                    a_lo = a_pair[t // 2][:, t % 2, :]
                    off = 0 if t == 0 else P
                    a_up = a_pair[(t + 1) // 2][:, (t + 1) % 2, :]
                    cps = psC.tile([P, 130], f32, tag="cps")
                    nc.tensor.matmul(
                        cps[:, 0:65],
                        a_lo[:, off : off + P],
                        va[:, t, :],
                        start=True,
                        stop=False,
                    )
                    nc.tensor.matmul(
                        cps[:, 0:65],
                        a_up[:, 0:P],
                        va[:, t + 1, :],
                        start=False,
                        stop=True,
                    )
                    nc.tensor.matmul(
                        cps[:, 65:130],
                        ag_cur[:, (t % 4) * P : (t % 4 + 1) * P],
                        vg,
                        start=True,
                        stop=True,
                    )
                    # normalization: per-partition (per-query) reciprocals
                    r2 = stat.tile([P, 2], f32, tag="r2")
                    nc.vector.reciprocal(r2, cps[:, 64:130:65])
                    tg = tgp.tile([P, BS], f32, tag="tg")
                    nc.vector.tensor_scalar_mul(tg, cps[:, 65:129], r2[:, 1:2])
                    nc.vector.scalar_tensor_tensor(
                        ctx_nat[:, t, r0 : r0 + BS],
                        cps[:, 0:64],
                        r2[:, 0:1],
                        tg,
                        OP.mult,
                        OP.add,
                    )

                # -- global rows: full attention for the 2 global q blocks --
                cpr0 = psQ.tile([P, 65], f32, tag="cpr0")
                cpr1 = psQ.tile([P, 65], f32, tag="cpr1")
                for j in range(8):
                    spr = psS.tile([P, 4, P], f32, tag="sps")
                    for i in range(4):
                        c = 1 + 4 * j + i
                        nc.tensor.matmul(
                            spr[:, i, :],
                            kp[:, c * P : (c + 1) * P],
                            qg,
                            start=True,
                            stop=True,
                        )
                    ar = agr.tile([P, 4, P], bf16, tag="ar")
                    nc.scalar.activation(ar, spr, AF.Exp)
                    for i in range(4):
                        c = 1 + 4 * j + i
                        nc.tensor.matmul(
                            cpr0[p0s : p0s + BS, :],
                            ar[:, i, 0:BS],
                            va[:, c, :],
                            start=(c == 1),
                            stop=(c == nkc - 1),
                        )
                        nc.tensor.matmul(
                            cpr1[p1s : p1s + BS, :],
                            ar[:, i, BS:P],
                            va[:, c, :],
                            start=(c == 1),
                            stop=(c == nkc - 1),
                        )
                for gi, gv, pb, cpr in ((0, g0, p0s, cpr0), (1, g1, p1s, cpr1)):
                    rg = stat.tile([P, 1], f32, tag=f"rg{gi}")
                    nc.vector.reciprocal(
                        rg[pb : pb + BS, :], cpr[pb : pb + BS, 64:65]
                    )
                    nc.vector.tensor_scalar_mul(
                        ctx_nat[pb : pb + BS, gv // 2, r0 : r0 + BS],
                        cpr[pb : pb + BS, 0:64],
                        rg[pb : pb + BS, :],
                    )

        # ---------------- pass C: output projection ----------------
        with ExitStack() as ps:
            wop = ps.enter_context(tc.tile_pool(name="wop", bufs=1))
            wo_sb = wop.tile([P, ndc2, dmodel], bf16)
            nc.sync.dma_start(wo_sb, wo_d.rearrange("(a p) o -> p a o", p=P))
            ctp = ps.enter_context(tc.tile_pool(name="ctp", bufs=2))
            copool = ps.enter_context(tc.tile_pool(name="co", bufs=4))
            psT = ps.enter_context(tc.tile_pool(name="psT", bufs=4, space="PSUM"))
            psO = ps.enter_context(tc.tile_pool(name="psO", bufs=4, space="PSUM"))
            for ncc in range(nch):
                ctxT = ctp.tile([P, ndc2, 512], bf16, tag="ctxT")
                for ti in range(4):
                    t = ncc * 4 + ti
                    for dc in range(ndc2):
                        tp = psT.tile([P, P], bf16, tag="tp")
                        nc.tensor.transpose(
                            tp, ctx_nat[:, t, dc * P : (dc + 1) * P], ident
                        )
                        nc.scalar.copy(ctxT[:, dc, ti * P : (ti + 1) * P], tp)
                for ot in range(dmodel // P):
                    pp = psO.tile([P, 512], f32, tag="pso")
                    for dc in range(ndc2):
                        nc.tensor.matmul(
                            pp,
                            wo_sb[:, dc, ot * P : (ot + 1) * P],
                            ctxT[:, dc, :],
                            start=(dc == 0),
                            stop=(dc == ndc2 - 1),
                        )
                    ob = copool.tile([P, 512], bf16, tag="ob")
                    if ot % 2 == 0:
                        nc.scalar.copy(ob, pp)
                    else:
                        nc.vector.tensor_copy(ob, pp)
                    nc.sync.dma_start(
                        out_d[ot * P : (ot + 1) * P, ncc * 512 : (ncc + 1) * 512], ob
                    )

        if dbg:
            nc.sync.dma_start(qTo_d[:, :], qT_d)
            nc.sync.dma_start(kTo_d[:, :], kT_d)
            nc.sync.dma_start(vo_d[:, :], v_d)
            nc.sync.dma_start(ctxo_d[:, :, :], ctx_nat)

    nc.finalize()
    return nc


@functools.lru_cache(maxsize=8)
def _get(n, dmodel, dl, g0, g1):
    return _build(n, dmodel, dl, g0, g1)


def _prepare(inputs):
    """Build (nc, in_maps, meta) for the SPMD run from full unsharded inputs."""
    x = np.asarray(inputs["x"], np.float32)
    Wq = np.asarray(inputs["Wq"], np.float32)
    Wk = np.asarray(inputs["Wk"], np.float32)
    Wv = np.asarray(inputs["Wv"], np.float32)
    Wo = np.asarray(inputs["Wo"], np.float32)
    bq = np.asarray(inputs["bq"], np.float32)
    bk = np.asarray(inputs["bk"], np.float32)
    bv = np.asarray(inputs["bv"], np.float32)
    bo = np.asarray(inputs["bo"], np.float32)
    gi = np.asarray(inputs["global_indices"]).astype(np.int64)
    g0, g1 = int(gi[0]), int(gi[1])

    b_, n_, d_ = x.shape
    dl = d_ // 2
    scale = 1.0 / np.sqrt(np.float32(64.0)).astype(np.float32)

    nc = _get(n_, d_, dl, g0, g1)

    import ml_dtypes

    bf = ml_dtypes.bfloat16
    # mask pattern rows (periodic in the column index, see pass B docstring)
    NEGf = np.float32(-1e9)
    j = np.arange(n_) % 256
    qmask = np.zeros((64, n_), np.float32)
    qmask[0, (j >= 128) & (j < 192)] = 1.0  # w1e
    qmask[1, (j >= 64) & (j < 128)] = 1.0   # w2e
    qmask[2, j < 64] = 1.0                  # w1o
    qmask[3, j >= 192] = 1.0                # w2o
    qmask = np.ascontiguousarray(qmask).astype(bf)
    jk = np.arange(n_ + 128) % 256
    kmask = np.zeros((64, n_ + 128), np.float32)
    kmask[0, (jk >= 64) & (jk < 128)] = NEGf   # u1e
    kmask[1, jk < 64] = NEGf                   # u2e
    kmask[2, jk >= 192] = NEGf                 # u1o
    kmask[3, (jk >= 128) & (jk < 192)] = NEGf  # u2o
    kmask = np.ascontiguousarray(kmask).astype(bf)
    in_maps = []
    for c in range(8):
        b, hg = divmod(c, 2)
        S = slice(hg * dl, (hg + 1) * dl)
        in_maps.append(
            {
                "xT": np.ascontiguousarray(x[b].T).astype(bf),
                "qmask": qmask,
                "kmask": kmask,
                "wqT": np.ascontiguousarray((Wq[S, :] * scale).T).astype(bf),
                "wkT": np.ascontiguousarray(Wk[S, :].T).astype(bf),
                "wvT": np.ascontiguousarray(Wv[S, :].T).astype(bf),
                "woT": np.ascontiguousarray(Wo[:, S].T).astype(bf),
                "bq": np.ascontiguousarray(bq[S] * scale),
                "bk": np.ascontiguousarray(bk[S]),
            }
        )

    # host-side bv correction: out[q] += c(q) * bv @ Wo.T, c(q)=1 on global
    # blocks (overwritten by full-attention rows), else 2.
    bvWo = bv @ Wo.T  # [d_model]
    coef = np.full((n_, 1), 2.0, np.float32)
    bs = 64
    coef[g0 * bs : (g0 + 1) * bs] = 1.0
    coef[g1 * bs : (g1 + 1) * bs] = 1.0
    corr = (coef * bvWo[None, :] + bo[None, :]).astype(np.float32)

    return nc, in_maps, (b_, n_, d_, corr)


def _combine(res, meta):
    b_, n_, d_, corr = meta
    out = np.empty((b_, n_, d_), np.float32)
    for b in range(b_):
        out[b] = (
            res[2 * b]["outT"].T.astype(np.float32)
            + res[2 * b + 1]["outT"].T.astype(np.float32)
            + corr
        )
    return out


def kernel(**inputs):
    _ensure_path()
    from concourse.bass_utils import run_bass_kernel_spmd

    nc, in_maps, meta = _prepare(inputs)
    res = run_bass_kernel_spmd(nc, in_maps, list(range(8))).results
    return _combine(res, meta)


# revision 21
# speedup vs baseline: 2.3939x; 1.0330x over previous
"""BigBird attention (B=4, N=4096, D=1024, H=16, BS=64) on 8 TRN2 NeuronCores.

Sharding: batch (4-way) x head-group (2-way).  Core c handles batch c//2 and
heads [hg*8, hg*8+8) where hg = c%2 (d_model slice [hg*512, hg*512+512)).

Per core:
  pass A: QKV projections.  x.T tiles produced with DMA transposes; q/k
          emitted transposed (qT/kT: [dl, n], head dim on partitions), v
          natural.  score scale folded into Wq/bq on the host; bv dropped
          entirely (attention is affine in v: host adds c(q)*bv@Wo.T).
  pass B: per-head BigBird attention, all scores computed transposed
          (S^T = K_chunk^T Q, keys on partitions) so probabilities feed the
          AV matmuls directly as stationary operands -- no PE transposes.
          The sliding-window mask is folded into 4 extra contraction rows
          (rank-2 outer product of periodic 0/1 q-patterns and -1e9
          k-patterns), so exp() yields exact zeros in the masked corners.
          No max subtraction (scores bounded ~|3|).  V carries a ones
          column so each AV matmul also emits the softmax denominator
          per-partition; normalization is a per-partition reciprocal.
  pass C: transpose ctx with the PE, then row-parallel output projection
          -> partial outT [d_model, n] (f32).
Host combines: out[b] = outT(core 2b).T + outT(core 2b+1).T + bo + c(q)*bv@Wo.T
with c(q) = 1 for rows in global blocks else 2.

The kernel is specialized (compiled) per global_indices value.
"""

import functools
import sys

import numpy as np

P = 128
BS = 64
NEG = -1e9


def _ensure_path():
    try:
        import concourse.bass  # noqa: F401
    except ImportError:
        sys.path.insert(0, "/opt/trn_rl_repo")


def _build(n, dmodel, dl, g0, g1, dbg=0):
    """Build the per-core Bass program.

    n: sequence length per core, dmodel: model dim, dl: local head dims =
    hpc*64.  g0, g1: global block indices (compile-time constants).
    """
    _ensure_path()
    from contextlib import ExitStack

    import concourse.bass as bass  # noqa: F401
    import concourse.tile as tile
    from concourse import bacc, mybir
    from concourse.masks import make_identity

    f32 = mybir.dt.float32
    bf16 = mybir.dt.bfloat16
    AF = mybir.ActivationFunctionType
    OP = mybir.AluOpType

    nch = n // 512     # 512-column chunks of the sequence
    ndc = dmodel // P  # contraction chunks for QKV proj
    njt = dl // P      # row tiles of qT/kT
    hpc = dl // BS     # heads per core
    nt = n // P        # query tiles (2 blocks each)
    nkc = nt + 1       # padded key chunks (128 keys each, shifted by -BS)
    ndc2 = dl // P     # contraction chunks for out proj

    nc = bacc.Bacc(None, target_bir_lowering=False, debug=False)

    xT_d = nc.dram_tensor("xT", [dmodel, n], bf16, kind="ExternalInput")
    wq_d = nc.dram_tensor("wqT", [dmodel, dl], bf16, kind="ExternalInput")
    wk_d = nc.dram_tensor("wkT", [dmodel, dl], bf16, kind="ExternalInput")
    wv_d = nc.dram_tensor("wvT", [dmodel, dl], bf16, kind="ExternalInput")
    wo_d = nc.dram_tensor("woT", [dl, dmodel], bf16, kind="ExternalInput")
    bq_d = nc.dram_tensor("bq", [dl], f32, kind="ExternalInput")
    bk_d = nc.dram_tensor("bk", [dl], f32, kind="ExternalInput")
    qm_d = nc.dram_tensor("qmask", [64, n], bf16, kind="ExternalInput")
    km_d = nc.dram_tensor("kmask", [64, n + 2 * BS], bf16, kind="ExternalInput")
    out_d = nc.dram_tensor("outT", [dmodel, n], bf16, kind="ExternalOutput")
    if dbg:
        qTo_d = nc.dram_tensor("qTo", [dl, n], bf16, kind="ExternalOutput")
        kTo_d = nc.dram_tensor("kTo", [dl, n], bf16, kind="ExternalOutput")
        vo_d = nc.dram_tensor("vo", [n, dl], bf16, kind="ExternalOutput")
        ctxo_d = nc.dram_tensor("ctxo", [P, n // P, dl], bf16, kind="ExternalOutput")

    with tile.TileContext(nc) as tc, ExitStack() as top:
        dram = top.enter_context(tc.tile_pool(name="dram", bufs=1, space="DRAM"))
        qT_d = dram.tile([dl, n], bf16)
        kT_d = dram.tile([dl, n], bf16)
        v_d = dram.tile([n, dl], bf16)

        const = top.enter_context(tc.tile_pool(name="const", bufs=1))
        ident = const.tile([P, P], bf16)
        make_identity(nc, ident)

        # ctx natural accumulator: [q mod 128, tile, head*64+dh], SBUF-resident
        ctx_pool = top.enter_context(tc.tile_pool(name="ctx", bufs=1))
        ctx_nat = ctx_pool.tile([P, nt, dl], bf16)

        # pass-B per-head slots (manual ping-pong).  Allocated at top level so
        # their memory is disjoint from the pass-A pools: the constant regions
        # (mask rows, ones columns) are written once, up front.
        slot = top.enter_context(tc.tile_pool(name="slot", bufs=1))
        qz_s = [slot.tile([P, n], bf16, tag=f"qz{i}", name=f"qz{i}") for i in range(2)]
        kp_s = [slot.tile([P, n + 2 * BS], bf16, tag=f"kp{i}", name=f"kp{i}") for i in range(2)]
        va_s = [slot.tile([P, nkc, BS + 1], bf16, tag=f"va{i}", name=f"va{i}") for i in range(2)]
        kg_s = [slot.tile([P, P], bf16, tag=f"kg{i}", name=f"kg{i}") for i in range(2)]
        vg_s = [slot.tile([P, BS + 1], bf16, tag=f"vg{i}", name=f"vg{i}") for i in range(2)]
        qg_s = [slot.tile([P, P], bf16, tag=f"qg{i}", name=f"qg{i}") for i in range(2)]
        def init_slot_consts():
            for qz in qz_s:
                nc.sync.dma_start(qz[64:P, :], qm_d[:, :])
            for kp in kp_s:
                nc.sync.dma_start(kp[64:P, :], km_d[:, :])
            for kg in kg_s:
                nc.gpsimd.memset(kg[64:P, :], 0.0)
            for qg in qg_s:
                nc.gpsimd.memset(qg[64:P, :], 0.0)
            for va in va_s:
                nc.gpsimd.memset(va[:, :, BS : BS + 1], 1.0)
            for vg in vg_s:
                nc.gpsimd.memset(vg[:, BS : BS + 1], 1.0)

        # ---------------- pass A: projections ----------------
        with ExitStack() as ps:
            wpool = ps.enter_context(tc.tile_pool(name="wpool", bufs=1))
            wq_sb = wpool.tile([P, ndc, dl], bf16)
            wk_sb = wpool.tile([P, ndc, dl], bf16)
            wv_sb = wpool.tile([P, ndc, dl], bf16)
            psA = ps.enter_context(tc.tile_pool(name="psA", bufs=4, space="PSUM"))
            xtpool = ps.enter_context(tc.tile_pool(name="xtpool", bufs=3))
            aout = ps.enter_context(tc.tile_pool(name="aout", bufs=4))

            def load_xt(ch):
                n0 = ch * 512
                xT = xtpool.tile([P, ndc, 512], bf16, tag="xT", name="xT")
                for dc in range(ndc):
                    nc.sync.dma_start(
                        xT[:, dc, :], xT_d[dc * P : (dc + 1) * P, n0 : n0 + 512]
                    )
                return xT

            # first x chunk ahead of the (big) weight loads: the sync queue is
            # in-order, and the first matmuls need xT(ch0) + wq[dc0] only.
            xt_next = load_xt(0)
            nc.sync.dma_start(wq_sb, wq_d.rearrange("(a p) j -> p a j", p=P))
            nc.sync.dma_start(wk_sb, wk_d.rearrange("(a p) j -> p a j", p=P))
            nc.sync.dma_start(wv_sb, wv_d.rearrange("(a p) j -> p a j", p=P))
            bq_sb = wpool.tile([P, njt], f32)
            bk_sb = wpool.tile([P, njt], f32)
            nc.scalar.dma_start(bq_sb, bq_d.rearrange("(a p) -> p a", p=P))
            nc.scalar.dma_start(bk_sb, bk_d.rearrange("(a p) -> p a", p=P))

            for ch in range(nch):
                n0 = ch * 512
                xT = xt_next
                if ch + 1 < nch:
                    xt_next = load_xt(ch + 1)
                if ch == 2:
                    init_slot_consts()
                # qT / kT (transposed outputs, bias per-partition)
                for w_sb, b_sb, dst in ((wq_sb, bq_sb, qT_d), (wk_sb, bk_sb, kT_d)):
                    for jt in range(njt):
                        pp = psA.tile([P, 512], f32, tag="ps_a")
                        for dc in range(ndc):
                            nc.tensor.matmul(
                                pp,
                                w_sb[:, dc, jt * P : (jt + 1) * P],
                                xT[:, dc, :],
                                start=(dc == 0),
                                stop=(dc == ndc - 1),
                            )
                        ot = aout.tile([P, 512], bf16, tag="aout")
                        nc.scalar.activation(
                            ot, pp, AF.Identity, bias=b_sb[:, jt : jt + 1]
                        )
                        nc.scalar.dma_start(
                            dst[jt * P : (jt + 1) * P, n0 : n0 + 512], ot
                        )
                # v (natural layout, no bias -- folded to host)
                for ns in range(4):
                    pp = psA.tile([P, dl], f32, tag="ps_a")
                    for dc in range(ndc):
                        nc.tensor.matmul(
                            pp,
                            xT[:, dc, ns * P : (ns + 1) * P],
                            wv_sb[:, dc, :],
                            start=(dc == 0),
                            stop=(dc == ndc - 1),
                        )
                    ot = aout.tile([P, dl], bf16, tag="aout_v")
                    nc.scalar.copy(ot, pp)
                    nc.scalar.dma_start(v_d[n0 + ns * P : n0 + (ns + 1) * P, :], ot)

        # ---------------- pass B: attention ----------------
        with ExitStack() as ps:
            apool = ps.enter_context(tc.tile_pool(name="apool", bufs=4))
            agp = ps.enter_context(tc.tile_pool(name="agp", bufs=2))
            agr = ps.enter_context(tc.tile_pool(name="agr", bufs=2))
            stat = ps.enter_context(tc.tile_pool(name="stat", bufs=4))
            tgp = ps.enter_context(tc.tile_pool(name="tgp", bufs=4))
            psS = ps.enter_context(tc.tile_pool(name="psS", bufs=3, space="PSUM"))
            psC = ps.enter_context(tc.tile_pool(name="psC", bufs=3, space="PSUM"))
            psQ = ps.enter_context(tc.tile_pool(name="psQ", bufs=1, space="PSUM"))

            p0s = (g0 % 2) * BS
            p1s = (g1 % 2) * BS

            for h in range(hpc):
                r0 = h * BS
                qz, kp, va = qz_s[h % 2], kp_s[h % 2], va_s[h % 2]
                kg, vg, qg = kg_s[h % 2], vg_s[h % 2], qg_s[h % 2]

                # -- per-head DMAs (overlap previous head's compute) --
                nc.sync.dma_start(qz[0:64, :], qT_d[r0 : r0 + BS, :])
                nc.sync.dma_start(kp[0:64, BS : BS + n], kT_d[r0 : r0 + BS, :])
                nc.sync.dma_start(kp[0:64, 0:BS], kT_d[r0 : r0 + BS, n - BS : n])
                nc.sync.dma_start(kp[0:64, BS + n :], kT_d[r0 : r0 + BS, 0:BS])
                vs = v_d[:, r0 : r0 + BS]
                nc.sync.dma_start(va[0:BS, 0, 0:BS], vs[n - BS : n, :])
                nc.sync.dma_start(va[BS:P, 0, 0:BS], vs[0:BS, :])
                nc.sync.dma_start(
                    va[:, 1 : nkc - 1, 0:BS],
                    vs[BS : n - BS, :].rearrange("(a p) c -> p a c", p=P),
                )
                nc.sync.dma_start(va[0:BS, nkc - 1, 0:BS], vs[n - BS : n, :])
                nc.sync.dma_start(va[BS:P, nkc - 1, 0:BS], vs[0:BS, :])
                for gi, gv in enumerate((g0, g1)):
                    nc.sync.dma_start(
                        kg[0:64, gi * BS : (gi + 1) * BS],
                        kT_d[r0 : r0 + BS, gv * BS : (gv + 1) * BS],
                    )
                    nc.sync.dma_start(
                        vg[gi * BS : (gi + 1) * BS, 0:BS],
                        vs[gv * BS : (gv + 1) * BS, :],
                    )
                    nc.sync.dma_start(
                        qg[0:64, gi * BS : (gi + 1) * BS],
                        qT_d[r0 : r0 + BS, gv * BS : (gv + 1) * BS],
                    )

                # -- local + global-col scores (S^T layout), exp, AV --
                def sc_pair(pr):
                    """scores+exp for padded key chunks 2pr, 2pr+1 (batched)."""
                    sps = psS.tile([P, 2, 256], f32, tag="sps")
                    a_sb = apool.tile([P, 2, 256], bf16, tag="a")
                    nws = []
                    for i in (0, 1):
                        c = 2 * pr + i
                        if c > nt:
                            continue
                        lo = max(0, (c - 1)) * P
                        hi = min(nt, c + 1) * P
                        nws.append(hi - lo)
                        nc.tensor.matmul(
                            sps[:, i, 0 : hi - lo],
                            kp[:, c * P : (c + 1) * P],
                            qz[:, lo:hi],
                            start=True,
                            stop=True,
                        )
                    if nws == [256, 256]:
                        nc.scalar.activation(a_sb, sps, AF.Exp)
                    else:
                        for i, nw in enumerate(nws):
                            nc.scalar.activation(
                                a_sb[:, i, 0:nw], sps[:, i, 0:nw], AF.Exp
                            )
                    return a_sb

                def gc_group(j):
                    spg = psS.tile([P, 512], f32, tag="sps")
                    nc.tensor.matmul(
                        spg, kg, qz[:, j * 512 : (j + 1) * 512], start=True, stop=True
                    )
                    ag = agp.tile([P, 512], bf16, tag="ag")
                    nc.scalar.activation(ag, spg, AF.Exp)
                    return ag

                ag_cur = gc_group(0)
                a_pair = {0: sc_pair(0), 1: sc_pair(1)}
                for t in range(nt):
                    if t % 4 == 0 and t > 0:
                        ag_cur = gc_group(t // 4)
                    want = min(nt // 2, t // 2 + 2)
                    if want not in a_pair:
                        a_pair[want] = sc_pair(want)
                        a_pair.pop(want - 3, None)
                    a_lo = a_pair[t // 2][:, t % 2, :]
                    off = 0 if t == 0 else P
                    a_up = a_pair[(t + 1) // 2][:, (t + 1) % 2, :]
                    cps = psC.tile([P, 130], f32, tag="cps")
                    nc.tensor.matmul(
                        cps[:, 0:65],
                        a_lo[:, off : off + P],
                        va[:, t, :],
                        start=True,
                        stop=False,
                    )
                    nc.tensor.matmul(
                        cps[:, 0:65],
                        a_up[:, 0:P],
                        va[:, t + 1, :],
                        start=False,
                        stop=True,
                    )
                    nc.tensor.matmul(
                        cps[:, 65:130],
                        ag_cur[:, (t % 4) * P : (t % 4 + 1) * P],
                        vg,
                        start=True,
                        stop=True,
                    )
                    # normalization: per-partition (per-query) reciprocals
                    r2 = stat.tile([P, 2], f32, tag="r2")
                    nc.vector.reciprocal(r2, cps[:, 64:130:65])
                    tg = tgp.tile([P, BS], f32, tag="tg")
                    nc.vector.tensor_scalar_mul(tg, cps[:, 65:129], r2[:, 1:2])
                    nc.vector.scalar_tensor_tensor(
                        ctx_nat[:, t, r0 : r0 + BS],
                        cps[:, 0:64],
                        r2[:, 0:1],
                        tg,
                        OP.mult,
                        OP.add,
                    )

                # -- global rows: full attention for the 2 global q blocks --
                cpr0 = psQ.tile([P, 65], f32, tag="cpr0")
                cpr1 = psQ.tile([P, 65], f32, tag="cpr1")
                for j in range(8):
                    spr = psS.tile([P, 4, P], f32, tag="sps")
                    for i in range(4):
                        c = 1 + 4 * j + i
                        nc.tensor.matmul(
                            spr[:, i, :],
                            kp[:, c * P : (c + 1) * P],
                            qg,
                            start=True,
                            stop=True,
                        )
                    ar = agr.tile([P, 4, P], bf16, tag="ar")
                    nc.scalar.activation(ar, spr, AF.Exp)
                    for i in range(4):
                        c = 1 + 4 * j + i
                        nc.tensor.matmul(
                            cpr0[p0s : p0s + BS, :],
                            ar[:, i, 0:BS],
                            va[:, c, :],
                            start=(c == 1),
                            stop=(c == nkc - 1),
                        )
                        nc.tensor.matmul(
                            cpr1[p1s : p1s + BS, :],
                            ar[:, i, BS:P],
                            va[:, c, :],
                            start=(c == 1),
                            stop=(c == nkc - 1),
                        )
                for gi, gv, pb, cpr in ((0, g0, p0s, cpr0), (1, g1, p1s, cpr1)):
                    rg = stat.tile([P, 1], f32, tag=f"rg{gi}")
                    nc.vector.reciprocal(
                        rg[pb : pb + BS, :], cpr[pb : pb + BS, 64:65]
                    )
                    nc.vector.tensor_scalar_mul(
                        ctx_nat[pb : pb + BS, gv // 2, r0 : r0 + BS],
                        cpr[pb : pb + BS, 0:64],
                        rg[pb : pb + BS, :],
                    )

        # ---------------- pass C: output projection ----------------
        with ExitStack() as ps:
            wop = ps.enter_context(tc.tile_pool(name="wop", bufs=1))
            wo_sb = wop.tile([P, ndc2, dmodel], bf16)
            nc.sync.dma_start(wo_sb, wo_d.rearrange("(a p) o -> p a o", p=P))
            ctp = ps.enter_context(tc.tile_pool(name="ctp", bufs=2))
            copool = ps.enter_context(tc.tile_pool(name="co", bufs=4))
            psT = ps.enter_context(tc.tile_pool(name="psT", bufs=4, space="PSUM"))
            psO = ps.enter_context(tc.tile_pool(name="psO", bufs=4, space="PSUM"))
            for ncc in range(nch):
                ctxT = ctp.tile([P, ndc2, 512], bf16, tag="ctxT")
                for ti in range(4):
                    t = ncc * 4 + ti
                    for dc in range(ndc2):
                        tp = psT.tile([P, P], bf16, tag="tp")
                        nc.tensor.transpose(
                            tp, ctx_nat[:, t, dc * P : (dc + 1) * P], ident
                        )
                        if dc % 2 == 0:
                            nc.scalar.copy(ctxT[:, dc, ti * P : (ti + 1) * P], tp)
                        else:
                            nc.vector.tensor_copy(ctxT[:, dc, ti * P : (ti + 1) * P], tp)
                for ot in range(dmodel // P):
                    pp = psO.tile([P, 512], f32, tag="pso")
                    for dc in range(ndc2):
                        nc.tensor.matmul(
                            pp,
                            wo_sb[:, dc, ot * P : (ot + 1) * P],
                            ctxT[:, dc, :],
                            start=(dc == 0),
                            stop=(dc == ndc2 - 1),
                        )
                    ob = copool.tile([P, 512], bf16, tag="ob")
                    if ot % 2 == 0:
                        nc.scalar.copy(ob, pp)
                    else:
                        nc.vector.tensor_copy(ob, pp)
                    nc.sync.dma_start(
                        out_d[ot * P : (ot + 1) * P, ncc * 512 : (ncc + 1) * 512], ob
                    )

        if dbg:
            nc.sync.dma_start(qTo_d[:, :], qT_d)
            nc.sync.dma_start(kTo_d[:, :], kT_d)
            nc.sync.dma_start(vo_d[:, :], v_d)
            nc.sync.dma_start(ctxo_d[:, :, :], ctx_nat)

    nc.finalize()
    return nc


@functools.lru_cache(maxsize=8)
def _get(n, dmodel, dl, g0, g1):
    return _build(n, dmodel, dl, g0, g1)


def _prepare(inputs):
    """Build (nc, in_maps, meta) for the SPMD run from full unsharded inputs."""
    x = np.asarray(inputs["x"], np.float32)
    Wq = np.asarray(inputs["Wq"], np.float32)
    Wk = np.asarray(inputs["Wk"], np.float32)
    Wv = np.asarray(inputs["Wv"], np.float32)
    Wo = np.asarray(inputs["Wo"], np.float32)
    bq = np.asarray(inputs["bq"], np.float32)
    bk = np.asarray(inputs["bk"], np.float32)
    bv = np.asarray(inputs["bv"], np.float32)
    bo = np.asarray(inputs["bo"], np.float32)
    gi = np.asarray(inputs["global_indices"]).astype(np.int64)
    g0, g1 = int(gi[0]), int(gi[1])

    b_, n_, d_ = x.shape
    dl = d_ // 2
    scale = 1.0 / np.sqrt(np.float32(64.0)).astype(np.float32)

    nc = _get(n_, d_, dl, g0, g1)

    import ml_dtypes

    bf = ml_dtypes.bfloat16
    # mask pattern rows (periodic in the column index, see pass B docstring)
    NEGf = np.float32(-1e9)
    j = np.arange(n_) % 256
    qmask = np.zeros((64, n_), np.float32)
    qmask[0, (j >= 128) & (j < 192)] = 1.0  # w1e
    qmask[1, (j >= 64) & (j < 128)] = 1.0   # w2e
    qmask[2, j < 64] = 1.0                  # w1o
    qmask[3, j >= 192] = 1.0                # w2o
    qmask = np.ascontiguousarray(qmask).astype(bf)
    jk = np.arange(n_ + 128) % 256
    kmask = np.zeros((64, n_ + 128), np.float32)
    kmask[0, (jk >= 64) & (jk < 128)] = NEGf   # u1e
    kmask[1, jk < 64] = NEGf                   # u2e
    kmask[2, jk >= 192] = NEGf                 # u1o
    kmask[3, (jk >= 128) & (jk < 192)] = NEGf  # u2o
    kmask = np.ascontiguousarray(kmask).astype(bf)
    in_maps = []
    for c in range(8):
        b, hg = divmod(c, 2)
        S = slice(hg * dl, (hg + 1) * dl)
        in_maps.append(
            {
                "xT": np.ascontiguousarray(x[b].T).astype(bf),
                "qmask": qmask,
                "kmask": kmask,
                "wqT": np.ascontiguousarray((Wq[S, :] * scale).T).astype(bf),
                "wkT": np.ascontiguousarray(Wk[S, :].T).astype(bf),
                "wvT": np.ascontiguousarray(Wv[S, :].T).astype(bf),
                "woT": np.ascontiguousarray(Wo[:, S].T).astype(bf),
                "bq": np.ascontiguousarray(bq[S] * scale),
                "bk": np.ascontiguousarray(bk[S]),
            }
        )

    # host-side bv correction: out[q] += c(q) * bv @ Wo.T, c(q)=1 on global
    # blocks (overwritten by full-attention rows), else 2.
    bvWo = bv @ Wo.T  # [d_model]
    coef = np.full((n_, 1), 2.0, np.float32)
    bs = 64
    coef[g0 * bs : (g0 + 1) * bs] = 1.0
    coef[g1 * bs : (g1 + 1) * bs] = 1.0
    corr = (coef * bvWo[None, :] + bo[None, :]).astype(np.float32)

    return nc, in_maps, (b_, n_, d_, corr)


def _combine(res, meta):
    b_, n_, d_, corr = meta
    out = np.empty((b_, n_, d_), np.float32)
    for b in range(b_):
        out[b] = (
            res[2 * b]["outT"].T.astype(np.float32)
            + res[2 * b + 1]["outT"].T.astype(np.float32)
            + corr
        )
    return out


def kernel(**inputs):
    _ensure_path()
    from concourse.bass_utils import run_bass_kernel_spmd

    nc, in_maps, meta = _prepare(inputs)
    res = run_bass_kernel_spmd(nc, in_maps, list(range(8))).results
    return _combine(res, meta)


# revision 22
# speedup vs baseline: 2.4205x; 1.0111x over previous
"""BigBird attention (B=4, N=4096, D=1024, H=16, BS=64) on 8 TRN2 NeuronCores.

Sharding: batch (4-way) x head-group (2-way).  Core c handles batch c//2 and
heads [hg*8, hg*8+8) where hg = c%2 (d_model slice [hg*512, hg*512+512)).

Per core:
  pass A: QKV projections.  x.T tiles produced with DMA transposes; q/k
          emitted transposed (qT/kT: [dl, n], head dim on partitions), v
          natural.  score scale folded into Wq/bq on the host; bv dropped
          entirely (attention is affine in v: host adds c(q)*bv@Wo.T).
  pass B: per-head BigBird attention, all scores computed transposed
          (S^T = K_chunk^T Q, keys on partitions) so probabilities feed the
          AV matmuls directly as stationary operands -- no PE transposes.
          The sliding-window mask is folded into 4 extra contraction rows
          (rank-2 outer product of periodic 0/1 q-patterns and -1e9
          k-patterns), so exp() yields exact zeros in the masked corners.
          No max subtraction (scores bounded ~|3|).  V carries a ones
          column so each AV matmul also emits the softmax denominator
          per-partition; normalization is a per-partition reciprocal.
  pass C: transpose ctx with the PE, then row-parallel output projection
          -> partial outT [d_model, n] (f32).
Host combines: out[b] = outT(core 2b).T + outT(core 2b+1).T + bo + c(q)*bv@Wo.T
with c(q) = 1 for rows in global blocks else 2.

The kernel is specialized (compiled) per global_indices value.
"""

import functools
import sys

import numpy as np

P = 128
BS = 64
NEG = -1e9


def _ensure_path():
    try:
        import concourse.bass  # noqa: F401
    except ImportError:
        sys.path.insert(0, "/opt/trn_rl_repo")


def _build(n, dmodel, dl, g0, g1, dbg=0):
    """Build the per-core Bass program.

    n: sequence length per core, dmodel: model dim, dl: local head dims =
    hpc*64.  g0, g1: global block indices (compile-time constants).
    """
    _ensure_path()
    from contextlib import ExitStack

    import concourse.bass as bass  # noqa: F401
    import concourse.tile as tile
    from concourse import bacc, mybir
    from concourse.masks import make_identity

    f32 = mybir.dt.float32
    bf16 = mybir.dt.bfloat16
    AF = mybir.ActivationFunctionType
    OP = mybir.AluOpType

    nch = n // 512     # 512-column chunks of the sequence
    ndc = dmodel // P  # contraction chunks for QKV proj
    njt = dl // P      # row tiles of qT/kT
    hpc = dl // BS     # heads per core
    nt = n // P        # query tiles (2 blocks each)
    nkc = nt + 1       # padded key chunks (128 keys each, shifted by -BS)
    ndc2 = dl // P     # contraction chunks for out proj

    nc = bacc.Bacc(None, target_bir_lowering=False, debug=False)

    xT_d = nc.dram_tensor("xT", [dmodel, n], bf16, kind="ExternalInput")
    wq_d = nc.dram_tensor("wqT", [dmodel, dl], bf16, kind="ExternalInput")
    wk_d = nc.dram_tensor("wkT", [dmodel, dl], bf16, kind="ExternalInput")
    wv_d = nc.dram_tensor("wvT", [dmodel, dl], bf16, kind="ExternalInput")
    wo_d = nc.dram_tensor("woT", [dl, dmodel], bf16, kind="ExternalInput")
    bq_d = nc.dram_tensor("bq", [dl], f32, kind="ExternalInput")
    bk_d = nc.dram_tensor("bk", [dl], f32, kind="ExternalInput")
    qm_d = nc.dram_tensor("qmask", [64, n], bf16, kind="ExternalInput")
    km_d = nc.dram_tensor("kmask", [64, n + 2 * BS], bf16, kind="ExternalInput")
    out_d = nc.dram_tensor("outT", [dmodel, n], bf16, kind="ExternalOutput")
    if dbg:
        qTo_d = nc.dram_tensor("qTo", [dl, n], bf16, kind="ExternalOutput")
        kTo_d = nc.dram_tensor("kTo", [dl, n], bf16, kind="ExternalOutput")
        vo_d = nc.dram_tensor("vo", [n, dl], bf16, kind="ExternalOutput")
        ctxo_d = nc.dram_tensor("ctxo", [P, n // P, dl], bf16, kind="ExternalOutput")

    with tile.TileContext(nc) as tc, ExitStack() as top:
        dram = top.enter_context(tc.tile_pool(name="dram", bufs=1, space="DRAM"))
        qT_d = dram.tile([dl, n], bf16)
        kT_d = dram.tile([dl, n], bf16)
        v_d = dram.tile([n, dl], bf16)

        const = top.enter_context(tc.tile_pool(name="const", bufs=1))
        ident = const.tile([P, P], bf16)
        make_identity(nc, ident)

        # ctx natural accumulator: [q mod 128, tile, head*64+dh], SBUF-resident
        ctx_pool = top.enter_context(tc.tile_pool(name="ctx", bufs=1))
        ctx_nat = ctx_pool.tile([P, nt, dl], bf16)

        # pass-B per-head slots (manual ping-pong).  Allocated at top level so
        # their memory is disjoint from the pass-A pools: the constant regions
        # (mask rows, ones columns) are written once, up front.
        slot = top.enter_context(tc.tile_pool(name="slot", bufs=1))
        qz_s = [slot.tile([P, n], bf16, tag=f"qz{i}", name=f"qz{i}") for i in range(2)]
        kp_s = [slot.tile([P, n + 2 * BS], bf16, tag=f"kp{i}", name=f"kp{i}") for i in range(2)]
        va_s = [slot.tile([P, nkc, BS + 1], bf16, tag=f"va{i}", name=f"va{i}") for i in range(2)]
        kg_s = [slot.tile([P, P], bf16, tag=f"kg{i}", name=f"kg{i}") for i in range(2)]
        vg_s = [slot.tile([P, BS + 1], bf16, tag=f"vg{i}", name=f"vg{i}") for i in range(2)]
        qg_s = [slot.tile([P, P], bf16, tag=f"qg{i}", name=f"qg{i}") for i in range(2)]
        def init_slot_consts():
            for qz in qz_s:
                nc.sync.dma_start(qz[64:P, :], qm_d[:, :])
            for kp in kp_s:
                nc.sync.dma_start(kp[64:P, :], km_d[:, :])
            for kg in kg_s:
                nc.gpsimd.memset(kg[64:P, :], 0.0)
            for qg in qg_s:
                nc.gpsimd.memset(qg[64:P, :], 0.0)
            for va in va_s:
                nc.gpsimd.memset(va[:, :, BS : BS + 1], 1.0)
            for vg in vg_s:
                nc.gpsimd.memset(vg[:, BS : BS + 1], 1.0)

        # ---------------- pass A: projections ----------------
        with ExitStack() as ps:
            wpool = ps.enter_context(tc.tile_pool(name="wpool", bufs=1))
            wq_sb = wpool.tile([P, ndc, dl], bf16)
            wk_sb = wpool.tile([P, ndc, dl], bf16)
            wv_sb = wpool.tile([P, ndc, dl], bf16)
            psA = ps.enter_context(tc.tile_pool(name="psA", bufs=4, space="PSUM"))
            xtpool = ps.enter_context(tc.tile_pool(name="xtpool", bufs=3))
            aout = ps.enter_context(tc.tile_pool(name="aout", bufs=4))

            def load_xt(ch):
                n0 = ch * 512
                xT = xtpool.tile([P, ndc, 512], bf16, tag="xT", name="xT")
                for dc in range(ndc):
                    nc.sync.dma_start(
                        xT[:, dc, :], xT_d[dc * P : (dc + 1) * P, n0 : n0 + 512]
                    )
                return xT

            # first x chunk ahead of the (big) weight loads: the sync queue is
            # in-order, and the first matmuls need xT(ch0) + wq[dc0] only.
            xt_next = load_xt(0)
            nc.sync.dma_start(wq_sb, wq_d.rearrange("(a p) j -> p a j", p=P))
            nc.sync.dma_start(wk_sb, wk_d.rearrange("(a p) j -> p a j", p=P))
            nc.sync.dma_start(wv_sb, wv_d.rearrange("(a p) j -> p a j", p=P))
            bq_sb = wpool.tile([P, njt], f32)
            bk_sb = wpool.tile([P, njt], f32)
            nc.scalar.dma_start(bq_sb, bq_d.rearrange("(a p) -> p a", p=P))
            nc.scalar.dma_start(bk_sb, bk_d.rearrange("(a p) -> p a", p=P))

            for ch in range(nch):
                n0 = ch * 512
                xT = xt_next
                if ch + 1 < nch:
                    xt_next = load_xt(ch + 1)
                if ch == 2:
                    init_slot_consts()
                # qT / kT (transposed outputs, bias per-partition)
                for w_sb, b_sb, dst in ((wq_sb, bq_sb, qT_d), (wk_sb, bk_sb, kT_d)):
                    for jt in range(njt):
                        pp = psA.tile([P, 512], f32, tag="ps_a")
                        for dc in range(ndc):
                            nc.tensor.matmul(
                                pp,
                                w_sb[:, dc, jt * P : (jt + 1) * P],
                                xT[:, dc, :],
                                start=(dc == 0),
                                stop=(dc == ndc - 1),
                            )
                        ot = aout.tile([P, 512], bf16, tag="aout")
                        nc.scalar.activation(
                            ot, pp, AF.Identity, bias=b_sb[:, jt : jt + 1]
                        )
                        nc.scalar.dma_start(
                            dst[jt * P : (jt + 1) * P, n0 : n0 + 512], ot
                        )
                # v (natural layout, no bias -- folded to host)
                for ns in range(4):
                    pp = psA.tile([P, dl], f32, tag="ps_a")
                    for dc in range(ndc):
                        nc.tensor.matmul(
                            pp,
                            xT[:, dc, ns * P : (ns + 1) * P],
                            wv_sb[:, dc, :],
                            start=(dc == 0),
                            stop=(dc == ndc - 1),
                        )
                    ot = aout.tile([P, dl], bf16, tag="aout_v")
                    nc.scalar.copy(ot, pp)
                    nc.scalar.dma_start(v_d[n0 + ns * P : n0 + (ns + 1) * P, :], ot)

        # ---------------- pass B: attention ----------------
        with ExitStack() as ps:
            apool = ps.enter_context(tc.tile_pool(name="apool", bufs=4))
            agp = ps.enter_context(tc.tile_pool(name="agp", bufs=2))
            agr = ps.enter_context(tc.tile_pool(name="agr", bufs=2))
            stat = ps.enter_context(tc.tile_pool(name="stat", bufs=4))
            tgp = ps.enter_context(tc.tile_pool(name="tgp", bufs=4))
            psS = ps.enter_context(tc.tile_pool(name="psS", bufs=3, space="PSUM"))
            psC = ps.enter_context(tc.tile_pool(name="psC", bufs=3, space="PSUM"))
            psQ = ps.enter_context(tc.tile_pool(name="psQ", bufs=1, space="PSUM"))

            p0s = (g0 % 2) * BS
            p1s = (g1 % 2) * BS

            for h in range(hpc):
                r0 = h * BS
                qz, kp, va = qz_s[h % 2], kp_s[h % 2], va_s[h % 2]
                kg, vg, qg = kg_s[h % 2], vg_s[h % 2], qg_s[h % 2]

                # -- per-head DMAs (overlap previous head's compute) --
                nc.sync.dma_start(qz[0:64, :], qT_d[r0 : r0 + BS, :])
                nc.sync.dma_start(kp[0:64, BS : BS + n], kT_d[r0 : r0 + BS, :])
                nc.sync.dma_start(kp[0:64, 0:BS], kT_d[r0 : r0 + BS, n - BS : n])
                nc.sync.dma_start(kp[0:64, BS + n :], kT_d[r0 : r0 + BS, 0:BS])
                vs = v_d[:, r0 : r0 + BS]
                nc.sync.dma_start(va[0:BS, 0, 0:BS], vs[n - BS : n, :])
                nc.sync.dma_start(va[BS:P, 0, 0:BS], vs[0:BS, :])
                nc.sync.dma_start(
                    va[:, 1 : nkc - 1, 0:BS],
                    vs[BS : n - BS, :].rearrange("(a p) c -> p a c", p=P),
                )
                nc.sync.dma_start(va[0:BS, nkc - 1, 0:BS], vs[n - BS : n, :])
                nc.sync.dma_start(va[BS:P, nkc - 1, 0:BS], vs[0:BS, :])
                for gi, gv in enumerate((g0, g1)):
                    nc.sync.dma_start(
                        kg[0:64, gi * BS : (gi + 1) * BS],
                        kT_d[r0 : r0 + BS, gv * BS : (gv + 1) * BS],
                    )
                    nc.sync.dma_start(
                        vg[gi * BS : (gi + 1) * BS, 0:BS],
                        vs[gv * BS : (gv + 1) * BS, :],
                    )
                    nc.sync.dma_start(
                        qg[0:64, gi * BS : (gi + 1) * BS],
                        qT_d[r0 : r0 + BS, gv * BS : (gv + 1) * BS],
                    )

                # -- local + global-col scores (S^T layout), exp, AV --
                def sc_pair(pr):
                    """scores+exp for padded key chunks 2pr, 2pr+1 (batched)."""
                    sps = psS.tile([P, 2, 256], f32, tag="sps")
                    a_sb = apool.tile([P, 2, 256], bf16, tag="a")
                    nws = []
                    for i in (0, 1):
                        c = 2 * pr + i
                        if c > nt:
                            continue
                        lo = max(0, (c - 1)) * P
                        hi = min(nt, c + 1) * P
                        nws.append(hi - lo)
                        nc.tensor.matmul(
                            sps[:, i, 0 : hi - lo],
                            kp[:, c * P : (c + 1) * P],
                            qz[:, lo:hi],
                            start=True,
                            stop=True,
                        )
                    if nws == [256, 256]:
                        nc.scalar.activation(a_sb, sps, AF.Exp)
                    else:
                        for i, nw in enumerate(nws):
                            nc.scalar.activation(
                                a_sb[:, i, 0:nw], sps[:, i, 0:nw], AF.Exp
                            )
                    return a_sb

                def gc_group(j):
                    spg = psS.tile([P, 512], f32, tag="sps")
                    nc.tensor.matmul(
                        spg, kg, qz[:, j * 512 : (j + 1) * 512], start=True, stop=True
                    )
                    ag = agp.tile([P, 512], bf16, tag="ag")
                    nc.scalar.activation(ag, spg, AF.Exp)
                    return ag

                ag_nxt = gc_group(0)
                a_pair = {0: sc_pair(0), 1: sc_pair(1)}
                ag_cur = None
                for t in range(nt):
                    if t % 4 == 0:
                        ag_cur = ag_nxt
                    if t % 4 == 1 and t // 4 + 1 < 8:
                        ag_nxt = gc_group(t // 4 + 1)
                    want = min(nt // 2, t // 2 + 2)
                    if want not in a_pair:
                        a_pair[want] = sc_pair(want)
                        a_pair.pop(want - 3, None)
                    a_lo = a_pair[t // 2][:, t % 2, :]
                    off = 0 if t == 0 else P
                    a_up = a_pair[(t + 1) // 2][:, (t + 1) % 2, :]
                    cps = psC.tile([P, 130], f32, tag="cps")
                    nc.tensor.matmul(
                        cps[:, 0:65],
                        a_lo[:, off : off + P],
                        va[:, t, :],
                        start=True,
                        stop=False,
                    )
                    nc.tensor.matmul(
                        cps[:, 0:65],
                        a_up[:, 0:P],
                        va[:, t + 1, :],
                        start=False,
                        stop=True,
                    )
                    nc.tensor.matmul(
                        cps[:, 65:130],
                        ag_cur[:, (t % 4) * P : (t % 4 + 1) * P],
                        vg,
                        start=True,
                        stop=True,
                    )
                    # normalization: per-partition (per-query) reciprocals
                    r2 = stat.tile([P, 2], f32, tag="r2")
                    nc.vector.reciprocal(r2, cps[:, 64:130:65])
                    tg = tgp.tile([P, BS], f32, tag="tg")
                    nc.vector.tensor_scalar_mul(tg, cps[:, 65:129], r2[:, 1:2])
                    nc.vector.scalar_tensor_tensor(
                        ctx_nat[:, t, r0 : r0 + BS],
                        cps[:, 0:64],
                        r2[:, 0:1],
                        tg,
                        OP.mult,
                        OP.add,
                    )

                # -- global rows: full attention for the 2 global q blocks --
                cpr0 = psQ.tile([P, 65], f32, tag="cpr0")
                cpr1 = psQ.tile([P, 65], f32, tag="cpr1")

                def grow_scores(j):
                    spr = psS.tile([P, 4, P], f32, tag="sps")
                    for i in range(4):
                        c = 1 + 4 * j + i
                        nc.tensor.matmul(
                            spr[:, i, :],
                            kp[:, c * P : (c + 1) * P],
                            qg,
                            start=True,
                            stop=True,
                        )
                    ar = agr.tile([P, 4, P], bf16, tag="ar")
                    nc.scalar.activation(ar, spr, AF.Exp)
                    return ar

                ar_nxt = grow_scores(0)
                for j in range(8):
                    ar = ar_nxt
                    if j + 1 < 8:
                        ar_nxt = grow_scores(j + 1)
                    for i in range(4):
                        c = 1 + 4 * j + i
                        nc.tensor.matmul(
                            cpr0[p0s : p0s + BS, :],
                            ar[:, i, 0:BS],
                            va[:, c, :],
                            start=(c == 1),
                            stop=(c == nkc - 1),
                        )
                        nc.tensor.matmul(
                            cpr1[p1s : p1s + BS, :],
                            ar[:, i, BS:P],
                            va[:, c, :],
                            start=(c == 1),
                            stop=(c == nkc - 1),
                        )
                for gi, gv, pb, cpr in ((0, g0, p0s, cpr0), (1, g1, p1s, cpr1)):
                    rg = stat.tile([P, 1], f32, tag=f"rg{gi}")
                    nc.vector.reciprocal(
                        rg[pb : pb + BS, :], cpr[pb : pb + BS, 64:65]
                    )
                    nc.vector.tensor_scalar_mul(
                        ctx_nat[pb : pb + BS, gv // 2, r0 : r0 + BS],
                        cpr[pb : pb + BS, 0:64],
                        rg[pb : pb + BS, :],
                    )

        # ---------------- pass C: output projection ----------------
        with ExitStack() as ps:
            wop = ps.enter_context(tc.tile_pool(name="wop", bufs=1))
            wo_sb = wop.tile([P, ndc2, dmodel], bf16)
            nc.sync.dma_start(wo_sb, wo_d.rearrange("(a p) o -> p a o", p=P))
            ctp = ps.enter_context(tc.tile_pool(name="ctp", bufs=2))
            copool = ps.enter_context(tc.tile_pool(name="co", bufs=4))
            psT = ps.enter_context(tc.tile_pool(name="psT", bufs=4, space="PSUM"))
            psO = ps.enter_context(tc.tile_pool(name="psO", bufs=4, space="PSUM"))
            for ncc in range(nch):
                ctxT = ctp.tile([P, ndc2, 512], bf16, tag="ctxT")
                for ti in range(4):
                    t = ncc * 4 + ti
                    for dc in range(ndc2):
                        tp = psT.tile([P, P], bf16, tag="tp")
                        nc.tensor.transpose(
                            tp, ctx_nat[:, t, dc * P : (dc + 1) * P], ident
                        )
                        if dc % 2 == 0:
                            nc.scalar.copy(ctxT[:, dc, ti * P : (ti + 1) * P], tp)
                        else:
                            nc.vector.tensor_copy(ctxT[:, dc, ti * P : (ti + 1) * P], tp)
                for ot in range(dmodel // P):
                    pp = psO.tile([P, 512], f32, tag="pso")
                    for dc in range(ndc2):
                        nc.tensor.matmul(
                            pp,
                            wo_sb[:, dc, ot * P : (ot + 1) * P],
                            ctxT[:, dc, :],
                            start=(dc == 0),
                            stop=(dc == ndc2 - 1),
                        )
                    ob = copool.tile([P, 512], bf16, tag="ob")
                    if ot % 2 == 0:
                        nc.scalar.copy(ob, pp)
                    else:
                        nc.vector.tensor_copy(ob, pp)
                    nc.sync.dma_start(
                        out_d[ot * P : (ot + 1) * P, ncc * 512 : (ncc + 1) * 512], ob
                    )

        if dbg:
            nc.sync.dma_start(qTo_d[:, :], qT_d)
            nc.sync.dma_start(kTo_d[:, :], kT_d)
            nc.sync.dma_start(vo_d[:, :], v_d)
            nc.sync.dma_start(ctxo_d[:, :, :], ctx_nat)

    nc.finalize()
    return nc


@functools.lru_cache(maxsize=8)
def _get(n, dmodel, dl, g0, g1):
    return _build(n, dmodel, dl, g0, g1)


def _prepare(inputs):
    """Build (nc, in_maps, meta) for the SPMD run from full unsharded inputs."""
    x = np.asarray(inputs["x"], np.float32)
    Wq = np.asarray(inputs["Wq"], np.float32)
    Wk = np.asarray(inputs["Wk"], np.float32)
    Wv = np.asarray(inputs["Wv"], np.float32)
    Wo = np.asarray(inputs["Wo"], np.float32)
    bq = np.asarray(inputs["bq"], np.float32)
    bk = np.asarray(inputs["bk"], np.float32)
    bv = np.asarray(inputs["bv"], np.float32)
    bo = np.asarray(inputs["bo"], np.float32)
    gi = np.asarray(inputs["global_indices"]).astype(np.int64)
    g0, g1 = int(gi[0]), int(gi[1])

    b_, n_, d_ = x.shape
    dl = d_ // 2
    scale = 1.0 / np.sqrt(np.float32(64.0)).astype(np.float32)

    nc = _get(n_, d_, dl, g0, g1)

    import ml_dtypes

    bf = ml_dtypes.bfloat16
    # mask pattern rows (periodic in the column index, see pass B docstring)
    NEGf = np.float32(-1e9)
    j = np.arange(n_) % 256
    qmask = np.zeros((64, n_), np.float32)
    qmask[0, (j >= 128) & (j < 192)] = 1.0  # w1e
    qmask[1, (j >= 64) & (j < 128)] = 1.0   # w2e
    qmask[2, j < 64] = 1.0                  # w1o
    qmask[3, j >= 192] = 1.0                # w2o
    qmask = np.ascontiguousarray(qmask).astype(bf)
    jk = np.arange(n_ + 128) % 256
    kmask = np.zeros((64, n_ + 128), np.float32)
    kmask[0, (jk >= 64) & (jk < 128)] = NEGf   # u1e
    kmask[1, jk < 64] = NEGf                   # u2e
    kmask[2, jk >= 192] = NEGf                 # u1o
    kmask[3, (jk >= 128) & (jk < 192)] = NEGf  # u2o
    kmask = np.ascontiguousarray(kmask).astype(bf)
    in_maps = []
    for c in range(8):
        b, hg = divmod(c, 2)
        S = slice(hg * dl, (hg + 1) * dl)
        in_maps.append(
            {
                "xT": np.ascontiguousarray(x[b].T).astype(bf),
                "qmask": qmask,
                "kmask": kmask,
                "wqT": np.ascontiguousarray((Wq[S, :] * scale).T).astype(bf),
                "wkT": np.ascontiguousarray(Wk[S, :].T).astype(bf),
                "wvT": np.ascontiguousarray(Wv[S, :].T).astype(bf),
                "woT": np.ascontiguousarray(Wo[:, S].T).astype(bf),
                "bq": np.ascontiguousarray(bq[S] * scale),
                "bk": np.ascontiguousarray(bk[S]),
            }
        )

    # host-side bv correction: out[q] += c(q) * bv @ Wo.T, c(q)=1 on global
    # blocks (overwritten by full-attention rows), else 2.
    bvWo = bv @ Wo.T  # [d_model]
    coef = np.full((n_, 1), 2.0, np.float32)
    bs = 64
    coef[g0 * bs : (g0 + 1) * bs] = 1.0
    coef[g1 * bs : (g1 + 1) * bs] = 1.0
    corr = (coef * bvWo[None, :] + bo[None, :]).astype(np.float32)

    return nc, in_maps, (b_, n_, d_, corr)


def _combine(res, meta):
    b_, n_, d_, corr = meta
    out = np.empty((b_, n_, d_), np.float32)
    for b in range(b_):
        out[b] = (
            res[2 * b]["outT"].T.astype(np.float32)
            + res[2 * b + 1]["outT"].T.astype(np.float32)
            + corr
        )
    return out


def kernel(**inputs):
    _ensure_path()
    from concourse.bass_utils import run_bass_kernel_spmd

    nc, in_maps, meta = _prepare(inputs)
    res = run_bass_kernel_spmd(nc, in_maps, list(range(8))).results
    return _combine(res, meta)


# revision 23
# speedup vs baseline: 2.4545x; 1.0141x over previous
"""BigBird attention (B=4, N=4096, D=1024, H=16, BS=64) on 8 TRN2 NeuronCores.

Sharding: batch (4-way) x head-group (2-way).  Core c handles batch c//2 and
heads [hg*8, hg*8+8) where hg = c%2 (d_model slice [hg*512, hg*512+512)).

Per core:
  pass A: QKV projections.  x.T tiles produced with DMA transposes; q/k
          emitted transposed (qT/kT: [dl, n], head dim on partitions), v
          natural.  score scale folded into Wq/bq on the host; bv dropped
          entirely (attention is affine in v: host adds c(q)*bv@Wo.T).
  pass B: per-head BigBird attention, all scores computed transposed
          (S^T = K_chunk^T Q, keys on partitions) so probabilities feed the
          AV matmuls directly as stationary operands -- no PE transposes.
          The sliding-window mask is folded into 4 extra contraction rows
          (rank-2 outer product of periodic 0/1 q-patterns and -1e9
          k-patterns), so exp() yields exact zeros in the masked corners.
          No max subtraction (scores bounded ~|3|).  V carries a ones
          column so each AV matmul also emits the softmax denominator
          per-partition; normalization is a per-partition reciprocal.
  pass C: transpose ctx with the PE, then row-parallel output projection
          -> partial outT [d_model, n] (f32).
Host combines: out[b] = outT(core 2b).T + outT(core 2b+1).T + bo + c(q)*bv@Wo.T
with c(q) = 1 for rows in global blocks else 2.

The kernel is specialized (compiled) per global_indices value.
"""

import functools
import sys

import numpy as np

P = 128
BS = 64
NEG = -1e9


def _ensure_path():
    try:
        import concourse.bass  # noqa: F401
    except ImportError:
        sys.path.insert(0, "/opt/trn_rl_repo")


def _build(n, dmodel, dl, g0, g1, dbg=0):
    """Build the per-core Bass program.

    n: sequence length per core, dmodel: model dim, dl: local head dims =
    hpc*64.  g0, g1: global block indices (compile-time constants).
    """
    _ensure_path()
    from contextlib import ExitStack

    import concourse.bass as bass  # noqa: F401
    import concourse.tile as tile
    from concourse import bacc, mybir
    from concourse.masks import make_identity

    f32 = mybir.dt.float32
    bf16 = mybir.dt.bfloat16
    AF = mybir.ActivationFunctionType
    OP = mybir.AluOpType

    nch = n // 512     # 512-column chunks of the sequence
    ndc = dmodel // P  # contraction chunks for QKV proj
    njt = dl // P      # row tiles of qT/kT
    hpc = dl // BS     # heads per core
    nt = n // P        # query tiles (2 blocks each)
    nkc = nt + 1       # padded key chunks (128 keys each, shifted by -BS)
    ndc2 = dl // P     # contraction chunks for out proj

    nc = bacc.Bacc(None, target_bir_lowering=False, debug=False)

    xT_d = nc.dram_tensor("xT", [dmodel, n], bf16, kind="ExternalInput")
    wq_d = nc.dram_tensor("wqT", [dmodel, dl], bf16, kind="ExternalInput")
    wk_d = nc.dram_tensor("wkT", [dmodel, dl], bf16, kind="ExternalInput")
    wv_d = nc.dram_tensor("wvT", [dmodel, dl], bf16, kind="ExternalInput")
    wo_d = nc.dram_tensor("woT", [dl, dmodel], bf16, kind="ExternalInput")
    bq_d = nc.dram_tensor("bq", [dl], f32, kind="ExternalInput")
    bk_d = nc.dram_tensor("bk", [dl], f32, kind="ExternalInput")
    qm_d = nc.dram_tensor("qmask", [64, n], bf16, kind="ExternalInput")
    km_d = nc.dram_tensor("kmask", [64, n + 2 * BS], bf16, kind="ExternalInput")
    out_d = nc.dram_tensor("outT", [dmodel, n], bf16, kind="ExternalOutput")
    if dbg:
        qTo_d = nc.dram_tensor("qTo", [dl, n], bf16, kind="ExternalOutput")
        kTo_d = nc.dram_tensor("kTo", [dl, n], bf16, kind="ExternalOutput")
        vo_d = nc.dram_tensor("vo", [n, dl], bf16, kind="ExternalOutput")
        ctxo_d = nc.dram_tensor("ctxo", [P, n // P, dl], bf16, kind="ExternalOutput")

    with tile.TileContext(nc) as tc, ExitStack() as top:
        dram = top.enter_context(tc.tile_pool(name="dram", bufs=1, space="DRAM"))
        qT_d = dram.tile([dl, n], bf16)
        kT_d = dram.tile([dl, n], bf16)
        v_d = dram.tile([n, dl], bf16)

        const = top.enter_context(tc.tile_pool(name="const", bufs=1))
        ident = const.tile([P, P], bf16)
        make_identity(nc, ident)

        # ctx natural accumulator: [q mod 128, tile, head*64+dh], SBUF-resident
        ctx_pool = top.enter_context(tc.tile_pool(name="ctx", bufs=1))
        ctx_nat = ctx_pool.tile([P, nt, dl], bf16)

        # pass-B per-head slots (manual ping-pong).  Allocated at top level so
        # their memory is disjoint from the pass-A pools: the constant regions
        # (mask rows, ones columns) are written once, up front.
        slot = top.enter_context(tc.tile_pool(name="slot", bufs=1))
        qz_s = [slot.tile([P, n], bf16, tag=f"qz{i}", name=f"qz{i}") for i in range(2)]
        kp_s = [slot.tile([P, n + 2 * BS], bf16, tag=f"kp{i}", name=f"kp{i}") for i in range(2)]
        va_s = [slot.tile([P, nkc, BS + 1], bf16, tag=f"va{i}", name=f"va{i}") for i in range(2)]
        kg_s = [slot.tile([P, P], bf16, tag=f"kg{i}", name=f"kg{i}") for i in range(2)]
        vg_s = [slot.tile([P, BS + 1], bf16, tag=f"vg{i}", name=f"vg{i}") for i in range(2)]
        qg_s = [slot.tile([P, P], bf16, tag=f"qg{i}", name=f"qg{i}") for i in range(2)]
        def init_slot_consts():
            for qz in qz_s:
                nc.sync.dma_start(qz[64:P, :], qm_d[:, :])
            for kp in kp_s:
                nc.sync.dma_start(kp[64:P, :], km_d[:, :])
            for kg in kg_s:
                nc.gpsimd.memset(kg[64:P, :], 0.0)
            for qg in qg_s:
                nc.gpsimd.memset(qg[64:P, :], 0.0)
            for va in va_s:
                nc.gpsimd.memset(va[:, :, BS : BS + 1], 1.0)
            for vg in vg_s:
                nc.gpsimd.memset(vg[:, BS : BS + 1], 1.0)

        # ---------------- pass A: projections ----------------
        with ExitStack() as ps:
            wpool = ps.enter_context(tc.tile_pool(name="wpool", bufs=1))
            wq_sb = wpool.tile([P, ndc, dl], bf16)
            wk_sb = wpool.tile([P, ndc, dl], bf16)
            wv_sb = wpool.tile([P, ndc, dl], bf16)
            psA = ps.enter_context(tc.tile_pool(name="psA", bufs=4, space="PSUM"))
            xtpool = ps.enter_context(tc.tile_pool(name="xtpool", bufs=3))
            aout = ps.enter_context(tc.tile_pool(name="aout", bufs=4))

            def load_xt(ch):
                n0 = ch * 512
                xT = xtpool.tile([P, ndc, 512], bf16, tag="xT", name="xT")
                for dc in range(ndc):
                    nc.sync.dma_start(
                        xT[:, dc, :], xT_d[dc * P : (dc + 1) * P, n0 : n0 + 512]
                    )
                return xT

            # first x chunk ahead of the (big) weight loads: the sync queue is
            # in-order, and the first matmuls need xT(ch0) + wq[dc0] only.
            xt_next = load_xt(0)
            for a in range(ndc):
                nc.sync.dma_start(wq_sb[:, a, :], wq_d[a * P : (a + 1) * P, :])
            nc.sync.dma_start(wk_sb, wk_d.rearrange("(a p) j -> p a j", p=P))
            nc.sync.dma_start(wv_sb, wv_d.rearrange("(a p) j -> p a j", p=P))
            bq_sb = wpool.tile([P, njt], f32)
            bk_sb = wpool.tile([P, njt], f32)
            nc.scalar.dma_start(bq_sb, bq_d.rearrange("(a p) -> p a", p=P))
            nc.scalar.dma_start(bk_sb, bk_d.rearrange("(a p) -> p a", p=P))

            for ch in range(nch):
                n0 = ch * 512
                xT = xt_next
                if ch + 1 < nch:
                    xt_next = load_xt(ch + 1)
                if ch == 2:
                    init_slot_consts()
                # qT / kT (transposed outputs, bias per-partition)
                for w_sb, b_sb, dst in ((wq_sb, bq_sb, qT_d), (wk_sb, bk_sb, kT_d)):
                    for jt in range(njt):
                        pp = psA.tile([P, 512], f32, tag="ps_a")
                        for dc in range(ndc):
                            nc.tensor.matmul(
                                pp,
                                w_sb[:, dc, jt * P : (jt + 1) * P],
                                xT[:, dc, :],
                                start=(dc == 0),
                                stop=(dc == ndc - 1),
                            )
                        ot = aout.tile([P, 512], bf16, tag="aout")
                        nc.scalar.activation(
                            ot, pp, AF.Identity, bias=b_sb[:, jt : jt + 1]
                        )
                        nc.scalar.dma_start(
                            dst[jt * P : (jt + 1) * P, n0 : n0 + 512], ot
                        )
                # v (natural layout, no bias -- folded to host)
                for ns in range(4):
                    pp = psA.tile([P, dl], f32, tag="ps_a")
                    for dc in range(ndc):
                        nc.tensor.matmul(
                            pp,
                            xT[:, dc, ns * P : (ns + 1) * P],
                            wv_sb[:, dc, :],
                            start=(dc == 0),
                            stop=(dc == ndc - 1),
                        )
                    ot = aout.tile([P, dl], bf16, tag="aout_v")
                    nc.scalar.copy(ot, pp)
                    nc.scalar.dma_start(v_d[n0 + ns * P : n0 + (ns + 1) * P, :], ot)

        # ---------------- pass B: attention ----------------
        with ExitStack() as ps:
            apool = ps.enter_context(tc.tile_pool(name="apool", bufs=4))
            agp = ps.enter_context(tc.tile_pool(name="agp", bufs=2))
            agr = ps.enter_context(tc.tile_pool(name="agr", bufs=8))
            stat = ps.enter_context(tc.tile_pool(name="stat", bufs=4))
            tgp = ps.enter_context(tc.tile_pool(name="tgp", bufs=4))
            psS = ps.enter_context(tc.tile_pool(name="psS", bufs=3, space="PSUM"))
            psC = ps.enter_context(tc.tile_pool(name="psC", bufs=4, space="PSUM"))
            psQ = ps.enter_context(tc.tile_pool(name="psQ", bufs=1, space="PSUM"))

            p0s = (g0 % 2) * BS
            p1s = (g1 % 2) * BS

            for h in range(hpc):
                r0 = h * BS
                qz, kp, va = qz_s[h % 2], kp_s[h % 2], va_s[h % 2]
                kg, vg, qg = kg_s[h % 2], vg_s[h % 2], qg_s[h % 2]

                # -- per-head DMAs (overlap previous head's compute) --
                nc.sync.dma_start(qz[0:64, :], qT_d[r0 : r0 + BS, :])
                nc.sync.dma_start(kp[0:64, BS : BS + n], kT_d[r0 : r0 + BS, :])
                nc.sync.dma_start(kp[0:64, 0:BS], kT_d[r0 : r0 + BS, n - BS : n])
                nc.sync.dma_start(kp[0:64, BS + n :], kT_d[r0 : r0 + BS, 0:BS])
                vs = v_d[:, r0 : r0 + BS]
                nc.sync.dma_start(va[0:BS, 0, 0:BS], vs[n - BS : n, :])
                nc.sync.dma_start(va[BS:P, 0, 0:BS], vs[0:BS, :])
                nc.sync.dma_start(
                    va[:, 1 : nkc - 1, 0:BS],
                    vs[BS : n - BS, :].rearrange("(a p) c -> p a c", p=P),
                )
                nc.sync.dma_start(va[0:BS, nkc - 1, 0:BS], vs[n - BS : n, :])
                nc.sync.dma_start(va[BS:P, nkc - 1, 0:BS], vs[0:BS, :])
                for gi, gv in enumerate((g0, g1)):
                    nc.sync.dma_start(
                        kg[0:64, gi * BS : (gi + 1) * BS],
                        kT_d[r0 : r0 + BS, gv * BS : (gv + 1) * BS],
                    )
                    nc.sync.dma_start(
                        vg[gi * BS : (gi + 1) * BS, 0:BS],
                        vs[gv * BS : (gv + 1) * BS, :],
                    )
                    nc.sync.dma_start(
                        qg[0:64, gi * BS : (gi + 1) * BS],
                        qT_d[r0 : r0 + BS, gv * BS : (gv + 1) * BS],
                    )

                # -- local + global-col scores (S^T layout), exp, AV --
                def sc_pair(pr):
                    """scores+exp for padded key chunks 2pr, 2pr+1 (batched)."""
                    sps = psS.tile([P, 2, 256], f32, tag="sps")
                    a_sb = apool.tile([P, 2, 256], bf16, tag="a")
                    nws = []
                    for i in (0, 1):
                        c = 2 * pr + i
                        if c > nt:
                            continue
                        lo = max(0, (c - 1)) * P
                        hi = min(nt, c + 1) * P
                        nws.append(hi - lo)
                        nc.tensor.matmul(
                            sps[:, i, 0 : hi - lo],
                            kp[:, c * P : (c + 1) * P],
                            qz[:, lo:hi],
                            start=True,
                            stop=True,
                        )
                    if nws == [256, 256]:
                        nc.scalar.activation(a_sb, sps, AF.Exp)
                    else:
                        for i, nw in enumerate(nws):
                            nc.scalar.activation(
                                a_sb[:, i, 0:nw], sps[:, i, 0:nw], AF.Exp
                            )
                    return a_sb

                def gc_group(j):
                    spg = psS.tile([P, 512], f32, tag="sps")
                    nc.tensor.matmul(
                        spg, kg, qz[:, j * 512 : (j + 1) * 512], start=True, stop=True
                    )
                    ag = agp.tile([P, 512], bf16, tag="ag")
                    nc.scalar.activation(ag, spg, AF.Exp)
                    return ag

                ag_nxt = gc_group(0)
                a_pair = {0: sc_pair(0), 1: sc_pair(1)}
                ag_cur = None
                for t in range(nt):
                    if t % 4 == 0:
                        ag_cur = ag_nxt
                    if t % 4 == 1 and t // 4 + 1 < 8:
                        ag_nxt = gc_group(t // 4 + 1)
                    want = min(nt // 2, t // 2 + 2)
                    if want not in a_pair:
                        a_pair[want] = sc_pair(want)
                        a_pair.pop(want - 3, None)
                    a_lo = a_pair[t // 2][:, t % 2, :]
                    off = 0 if t == 0 else P
                    a_up = a_pair[(t + 1) // 2][:, (t + 1) % 2, :]
                    cps = psC.tile([P, 130], f32, tag="cps")
                    nc.tensor.matmul(
                        cps[:, 0:65],
                        a_lo[:, off : off + P],
                        va[:, t, :],
                        start=True,
                        stop=False,
                    )
                    nc.tensor.matmul(
                        cps[:, 0:65],
                        a_up[:, 0:P],
                        va[:, t + 1, :],
                        start=False,
                        stop=True,
                    )
                    nc.tensor.matmul(
                        cps[:, 65:130],
                        ag_cur[:, (t % 4) * P : (t % 4 + 1) * P],
                        vg,
                        start=True,
                        stop=True,
                    )
                    # normalization: per-partition (per-query) reciprocals
                    r2 = stat.tile([P, 2], f32, tag="r2")
                    nc.vector.reciprocal(r2, cps[:, 64:130:65])
                    tg = tgp.tile([P, BS], f32, tag="tg")
                    nc.vector.tensor_scalar_mul(tg, cps[:, 65:129], r2[:, 1:2])
                    nc.vector.scalar_tensor_tensor(
                        ctx_nat[:, t, r0 : r0 + BS],
                        cps[:, 0:64],
                        r2[:, 0:1],
                        tg,
                        OP.mult,
                        OP.add,
                    )

                # -- global rows: full attention for the 2 global q blocks --
                cpr0 = psQ.tile([P, 130], f32, tag="cpr0")
                cpr1 = cpr0

                def grow_scores(j):
                    spr = psS.tile([P, 4, P], f32, tag="sps")
                    for i in range(4):
                        c = 1 + 4 * j + i
                        nc.tensor.matmul(
                            spr[:, i, :],
                            kp[:, c * P : (c + 1) * P],
                            qg,
                            start=True,
                            stop=True,
                        )
                    ar = agr.tile([P, 4, P], bf16, tag="ar")
                    nc.scalar.activation(ar, spr, AF.Exp)
                    return ar

                ars = [grow_scores(0)]
                for j in range(8):
                    if j + 1 < 8:
                        ars.append(grow_scores(j + 1))
                    for i in range(4):
                        c = 1 + 4 * j + i
                        nc.tensor.matmul(
                            cpr0[p0s : p0s + BS, 0:65],
                            ars[j][:, i, 0:BS],
                            va[:, c, :],
                            start=(c == 1),
                            stop=(c == nkc - 1),
                        )
                for j in range(8):
                    for i in range(4):
                        c = 1 + 4 * j + i
                        nc.tensor.matmul(
                            cpr1[p1s : p1s + BS, 65:130],
                            ars[j][:, i, BS:P],
                            va[:, c, :],
                            start=(c == 1),
                            stop=(c == nkc - 1),
                        )
                for gi, gv, pb, co in ((0, g0, p0s, 0), (1, g1, p1s, 65)):
                    rg = stat.tile([P, 1], f32, tag=f"rg{gi}")
                    nc.vector.reciprocal(
                        rg[pb : pb + BS, :], cpr0[pb : pb + BS, co + 64 : co + 65]
                    )
                    nc.vector.tensor_scalar_mul(
                        ctx_nat[pb : pb + BS, gv // 2, r0 : r0 + BS],
                        cpr0[pb : pb + BS, co : co + 64],
                        rg[pb : pb + BS, :],
                    )

        # ---------------- pass C: output projection ----------------
        with ExitStack() as ps:
            wop = ps.enter_context(tc.tile_pool(name="wop", bufs=1))
            wo_sb = wop.tile([P, ndc2, dmodel], bf16)
            nc.sync.dma_start(wo_sb, wo_d.rearrange("(a p) o -> p a o", p=P))
            ctp = ps.enter_context(tc.tile_pool(name="ctp", bufs=2))
            copool = ps.enter_context(tc.tile_pool(name="co", bufs=4))
            psT = ps.enter_context(tc.tile_pool(name="psT", bufs=4, space="PSUM"))
            psO = ps.enter_context(tc.tile_pool(name="psO", bufs=4, space="PSUM"))
            for ncc in range(nch):
                ctxT = ctp.tile([P, ndc2, 512], bf16, tag="ctxT")
                for ti in range(4):
                    t = ncc * 4 + ti
                    for dc in range(ndc2):
                        tp = psT.tile([P, P], bf16, tag="tp")
                        nc.tensor.transpose(
                            tp, ctx_nat[:, t, dc * P : (dc + 1) * P], ident
                        )
                        if dc % 2 == 0:
                            nc.scalar.copy(ctxT[:, dc, ti * P : (ti + 1) * P], tp)
                        else:
                            nc.vector.tensor_copy(ctxT[:, dc, ti * P : (ti + 1) * P], tp)
                for ot in range(dmodel // P):
                    pp = psO.tile([P, 512], f32, tag="pso")
                    for dc in range(ndc2):
                        nc.tensor.matmul(
                            pp,
                            wo_sb[:, dc, ot * P : (ot + 1) * P],
                            ctxT[:, dc, :],
                            start=(dc == 0),
                            stop=(dc == ndc2 - 1),
                        )
                    ob = copool.tile([P, 512], bf16, tag="ob")
                    if ot % 2 == 0:
                        nc.scalar.copy(ob, pp)
                    else:
                        nc.vector.tensor_copy(ob, pp)
                    nc.sync.dma_start(
                        out_d[ot * P : (ot + 1) * P, ncc * 512 : (ncc + 1) * 512], ob
                    )

        if dbg:
            nc.sync.dma_start(qTo_d[:, :], qT_d)
            nc.sync.dma_start(kTo_d[:, :], kT_d)
            nc.sync.dma_start(vo_d[:, :], v_d)
            nc.sync.dma_start(ctxo_d[:, :, :], ctx_nat)

    nc.finalize()
    return nc


@functools.lru_cache(maxsize=8)
def _get(n, dmodel, dl, g0, g1):
    return _build(n, dmodel, dl, g0, g1)


def _prepare(inputs):
    """Build (nc, in_maps, meta) for the SPMD run from full unsharded inputs."""
    x = np.asarray(inputs["x"], np.float32)
    Wq = np.asarray(inputs["Wq"], np.float32)
    Wk = np.asarray(inputs["Wk"], np.float32)
    Wv = np.asarray(inputs["Wv"], np.float32)
    Wo = np.asarray(inputs["Wo"], np.float32)
    bq = np.asarray(inputs["bq"], np.float32)
    bk = np.asarray(inputs["bk"], np.float32)
    bv = np.asarray(inputs["bv"], np.float32)
    bo = np.asarray(inputs["bo"], np.float32)
    gi = np.asarray(inputs["global_indices"]).astype(np.int64)
    g0, g1 = int(gi[0]), int(gi[1])

    b_, n_, d_ = x.shape
    dl = d_ // 2
    scale = 1.0 / np.sqrt(np.float32(64.0)).astype(np.float32)

    nc = _get(n_, d_, dl, g0, g1)

    import ml_dtypes

    bf = ml_dtypes.bfloat16
    # mask pattern rows (periodic in the column index, see pass B docstring)
    NEGf = np.float32(-1e9)
    j = np.arange(n_) % 256
    qmask = np.zeros((64, n_), np.float32)
    qmask[0, (j >= 128) & (j < 192)] = 1.0  # w1e
    qmask[1, (j >= 64) & (j < 128)] = 1.0   # w2e
    qmask[2, j < 64] = 1.0                  # w1o
    qmask[3, j >= 192] = 1.0                # w2o
    qmask = np.ascontiguousarray(qmask).astype(bf)
    jk = np.arange(n_ + 128) % 256
    kmask = np.zeros((64, n_ + 128), np.float32)
    kmask[0, (jk >= 64) & (jk < 128)] = NEGf   # u1e
    kmask[1, jk < 64] = NEGf                   # u2e
    kmask[2, jk >= 192] = NEGf                 # u1o
    kmask[3, (jk >= 128) & (jk < 192)] = NEGf  # u2o
    kmask = np.ascontiguousarray(kmask).astype(bf)
    in_maps = []
    for c in range(8):
        b, hg = divmod(c, 2)
        S = slice(hg * dl, (hg + 1) * dl)
        in_maps.append(
            {
                "xT": np.ascontiguousarray(x[b].T).astype(bf),
                "qmask": qmask,
                "kmask": kmask,
                "wqT": np.ascontiguousarray((Wq[S, :] * scale).T).astype(bf),
                "wkT": np.ascontiguousarray(Wk[S, :].T).astype(bf),
                "wvT": np.ascontiguousarray(Wv[S, :].T).astype(bf),
                "woT": np.ascontiguousarray(Wo[:, S].T).astype(bf),
                "bq": np.ascontiguousarray(bq[S] * scale),
                "bk": np.ascontiguousarray(bk[S]),
            }
        )

    # host-side bv correction: out[q] += c(q) * bv @ Wo.T, c(q)=1 on global
    # blocks (overwritten by full-attention rows), else 2.
    bvWo = bv @ Wo.T  # [d_model]
    coef = np.full((n_, 1), 2.0, np.float32)
    bs = 64
    coef[g0 * bs : (g0 + 1) * bs] = 1.0
    coef[g1 * bs : (g1 + 1) * bs] = 1.0
    corr = (coef * bvWo[None, :] + bo[None, :]).astype(np.float32)

    return nc, in_maps, (b_, n_, d_, corr)


def _combine(res, meta):
    b_, n_, d_, corr = meta
    out = np.empty((b_, n_, d_), np.float32)
    for b in range(b_):
        out[b] = (
            res[2 * b]["outT"].T.astype(np.float32)
            + res[2 * b + 1]["outT"].T.astype(np.float32)
            + corr
        )
    return out


def kernel(**inputs):
    _ensure_path()
    from concourse.bass_utils import run_bass_kernel_spmd

    nc, in_maps, meta = _prepare(inputs)
    res = run_bass_kernel_spmd(nc, in_maps, list(range(8))).results
    return _combine(res, meta)


# revision 24
# speedup vs baseline: 2.5279x; 1.0299x over previous
"""BigBird attention (B=4, N=4096, D=1024, H=16, BS=64) on 8 TRN2 NeuronCores.

Sharding: batch (4-way) x head-group (2-way).  Core c handles batch c//2 and
heads [hg*8, hg*8+8) where hg = c%2 (d_model slice [hg*512, hg*512+512)).

Per core:
  pass A: QKV projections.  x.T tiles produced with DMA transposes; q/k
          emitted transposed (qT/kT: [dl, n], head dim on partitions), v
          natural.  score scale folded into Wq/bq on the host; bv dropped
          entirely (attention is affine in v: host adds c(q)*bv@Wo.T).
  pass B: per-head BigBird attention, all scores computed transposed
          (S^T = K_chunk^T Q, keys on partitions) so probabilities feed the
          AV matmuls directly as stationary operands -- no PE transposes.
          The sliding-window mask is folded into 4 extra contraction rows
          (rank-2 outer product of periodic 0/1 q-patterns and -1e9
          k-patterns), so exp() yields exact zeros in the masked corners.
          No max subtraction (scores bounded ~|3|).  V carries a ones
          column so each AV matmul also emits the softmax denominator
          per-partition; normalization is a per-partition reciprocal.
  pass C: transpose ctx with the PE, then row-parallel output projection
          -> partial outT [d_model, n] (f32).
Host combines: out[b] = outT(core 2b).T + outT(core 2b+1).T + bo + c(q)*bv@Wo.T
with c(q) = 1 for rows in global blocks else 2.

The kernel is specialized (compiled) per global_indices value.
"""

import functools
import sys

import numpy as np

P = 128
BS = 64
NEG = -1e9


def _ensure_path():
    try:
        import concourse.bass  # noqa: F401
    except ImportError:
        sys.path.insert(0, "/opt/trn_rl_repo")


def _build(n, dmodel, dl, g0, g1, dbg=0):
    """Build the per-core Bass program.

    n: sequence length per core, dmodel: model dim, dl: local head dims =
    hpc*64.  g0, g1: global block indices (compile-time constants).
    """
    _ensure_path()
    from contextlib import ExitStack

    import concourse.bass as bass  # noqa: F401
    import concourse.tile as tile
    from concourse import bacc, mybir
    from concourse.masks import make_identity

    f32 = mybir.dt.float32
    bf16 = mybir.dt.bfloat16
    AF = mybir.ActivationFunctionType
    OP = mybir.AluOpType

    nch = n // 512     # 512-column chunks of the sequence
    ndc = dmodel // P  # contraction chunks for QKV proj
    njt = dl // P      # row tiles of qT/kT
    hpc = dl // BS     # heads per core
    nt = n // P        # query tiles (2 blocks each)
    nkc = nt + 1       # padded key chunks (128 keys each, shifted by -BS)
    ndc2 = dl // P     # contraction chunks for out proj

    nc = bacc.Bacc(None, target_bir_lowering=False, debug=False)

    xT_d = nc.dram_tensor("xT", [dmodel, n], bf16, kind="ExternalInput")
    wq_d = nc.dram_tensor("wqT", [dmodel, dl], bf16, kind="ExternalInput")
    wk_d = nc.dram_tensor("wkT", [dmodel, dl], bf16, kind="ExternalInput")
    wv_d = nc.dram_tensor("wvT", [dmodel, dl], bf16, kind="ExternalInput")
    wo_d = nc.dram_tensor("woT", [dl, dmodel], bf16, kind="ExternalInput")
    bq_d = nc.dram_tensor("bq", [dl], f32, kind="ExternalInput")
    bk_d = nc.dram_tensor("bk", [dl], f32, kind="ExternalInput")
    qm_d = nc.dram_tensor("qmask", [64, n], bf16, kind="ExternalInput")
    km_d = nc.dram_tensor("kmask", [64, n + 2 * BS], bf16, kind="ExternalInput")
    out_d = nc.dram_tensor("outT", [dmodel, n], bf16, kind="ExternalOutput")
    if dbg:
        qTo_d = nc.dram_tensor("qTo", [dl, n], bf16, kind="ExternalOutput")
        kTo_d = nc.dram_tensor("kTo", [dl, n], bf16, kind="ExternalOutput")
        vo_d = nc.dram_tensor("vo", [n, dl], bf16, kind="ExternalOutput")
        ctxo_d = nc.dram_tensor("ctxo", [P, n // P, dl], bf16, kind="ExternalOutput")

    with tile.TileContext(nc) as tc, ExitStack() as top:
        dram = top.enter_context(tc.tile_pool(name="dram", bufs=1, space="DRAM"))
        qT_d = dram.tile([dl, n], bf16)
        kT_d = dram.tile([dl, n], bf16)
        v_d = dram.tile([n, dl], bf16)

        const = top.enter_context(tc.tile_pool(name="const", bufs=1))
        ident = const.tile([P, P], bf16)
        make_identity(nc, ident)

        # ctx natural accumulator: [q mod 128, tile, head*64+dh], SBUF-resident
        ctx_pool = top.enter_context(tc.tile_pool(name="ctx", bufs=1))
        ctx_nat = ctx_pool.tile([P, nt, dl], bf16)

        # pass-B per-head slots (manual ping-pong).  Allocated at top level so
        # their memory is disjoint from the pass-A pools: the constant regions
        # (mask rows, ones columns) are written once, up front.
        slot = top.enter_context(tc.tile_pool(name="slot", bufs=1))
        qz_s = [slot.tile([P, n], bf16, tag=f"qz{i}", name=f"qz{i}") for i in range(2)]
        kp_s = [slot.tile([P, n + 2 * BS], bf16, tag=f"kp{i}", name=f"kp{i}") for i in range(2)]
        va_s = [slot.tile([P, nkc, BS + 1], bf16, tag=f"va{i}", name=f"va{i}") for i in range(2)]
        kg_s = [slot.tile([P, P], bf16, tag=f"kg{i}", name=f"kg{i}") for i in range(2)]
        vg_s = [slot.tile([P, BS + 1], bf16, tag=f"vg{i}", name=f"vg{i}") for i in range(2)]
        qg_s = [slot.tile([P, P], bf16, tag=f"qg{i}", name=f"qg{i}") for i in range(2)]
        def init_slot_consts():
            for qz in qz_s:
                nc.sync.dma_start(qz[64:P, :], qm_d[:, :])
            for kp in kp_s:
                nc.sync.dma_start(kp[64:P, :], km_d[:, :])
            for kg in kg_s:
                nc.gpsimd.memset(kg[64:P, :], 0.0)
            for qg in qg_s:
                nc.gpsimd.memset(qg[64:P, :], 0.0)
            for va in va_s:
                nc.gpsimd.memset(va[:, :, BS : BS + 1], 1.0)
            for vg in vg_s:
                nc.gpsimd.memset(vg[:, BS : BS + 1], 1.0)

        # ---------------- pass A: projections ----------------
        with ExitStack() as ps:
            wpool = ps.enter_context(tc.tile_pool(name="wpool", bufs=1))
            wq_sb = wpool.tile([P, ndc, dl], bf16)
            wk_sb = wpool.tile([P, ndc, dl], bf16)
            wv_sb = wpool.tile([P, ndc, dl], bf16)
            psA = ps.enter_context(tc.tile_pool(name="psA", bufs=4, space="PSUM"))
            xtpool = ps.enter_context(tc.tile_pool(name="xtpool", bufs=3))
            aout = ps.enter_context(tc.tile_pool(name="aout", bufs=4))

            def load_xt(ch):
                n0 = ch * 512
                xT = xtpool.tile([P, ndc, 512], bf16, tag="xT", name="xT")
                for dc in range(ndc):
                    nc.sync.dma_start(
                        xT[:, dc, :], xT_d[dc * P : (dc + 1) * P, n0 : n0 + 512]
                    )
                return xT

            # first x chunk ahead of the (big) weight loads: the sync queue is
            # in-order, and the first matmuls need xT(ch0) + wq[dc0] only.
            xt_next = load_xt(0)
            for a in range(ndc):
                nc.sync.dma_start(wq_sb[:, a, :], wq_d[a * P : (a + 1) * P, :])
            nc.sync.dma_start(wk_sb, wk_d.rearrange("(a p) j -> p a j", p=P))
            nc.sync.dma_start(wv_sb, wv_d.rearrange("(a p) j -> p a j", p=P))
            bq_sb = wpool.tile([P, njt], f32)
            bk_sb = wpool.tile([P, njt], f32)
            nc.scalar.dma_start(bq_sb, bq_d.rearrange("(a p) -> p a", p=P))
            nc.scalar.dma_start(bk_sb, bk_d.rearrange("(a p) -> p a", p=P))

            for ch in range(nch):
                n0 = ch * 512
                xT = xt_next
                if ch + 1 < nch:
                    xt_next = load_xt(ch + 1)
                if ch == 2:
                    init_slot_consts()
                # qT / kT (transposed outputs, bias per-partition)
                for w_sb, b_sb, dst in ((wq_sb, bq_sb, qT_d), (wk_sb, bk_sb, kT_d)):
                    for jt in range(njt):
                        pp = psA.tile([P, 512], f32, tag="ps_a")
                        for dc in range(ndc):
                            nc.tensor.matmul(
                                pp,
                                w_sb[:, dc, jt * P : (jt + 1) * P],
                                xT[:, dc, :],
                                start=(dc == 0),
                                stop=(dc == ndc - 1),
                            )
                        ot = aout.tile([P, 512], bf16, tag="aout")
                        nc.scalar.activation(
                            ot, pp, AF.Identity, bias=b_sb[:, jt : jt + 1]
                        )
                        nc.scalar.dma_start(
                            dst[jt * P : (jt + 1) * P, n0 : n0 + 512], ot
                        )
                # v (natural layout, no bias -- folded to host)
                for ns in range(4):
                    pp = psA.tile([P, dl], f32, tag="ps_a")
                    for dc in range(ndc):
                        nc.tensor.matmul(
                            pp,
                            xT[:, dc, ns * P : (ns + 1) * P],
                            wv_sb[:, dc, :],
                            start=(dc == 0),
                            stop=(dc == ndc - 1),
                        )
                    ot = aout.tile([P, dl], bf16, tag="aout_v")
                    nc.scalar.copy(ot, pp)
                    nc.scalar.dma_start(v_d[n0 + ns * P : n0 + (ns + 1) * P, :], ot)

        # ---------------- pass B: attention ----------------
        with ExitStack() as ps:
            apool = ps.enter_context(tc.tile_pool(name="apool", bufs=4))
            agp = ps.enter_context(tc.tile_pool(name="agp", bufs=2))
            agr = ps.enter_context(tc.tile_pool(name="agr", bufs=8))
            stat = ps.enter_context(tc.tile_pool(name="stat", bufs=4))
            tgp = ps.enter_context(tc.tile_pool(name="tgp", bufs=4))
            psS = ps.enter_context(tc.tile_pool(name="psS", bufs=3, space="PSUM"))
            psC = ps.enter_context(tc.tile_pool(name="psC", bufs=4, space="PSUM"))
            psQ = ps.enter_context(tc.tile_pool(name="psQ", bufs=1, space="PSUM"))

            p0s = (g0 % 2) * BS
            p1s = (g1 % 2) * BS

            for h in range(hpc):
                r0 = h * BS
                qz, kp, va = qz_s[h % 2], kp_s[h % 2], va_s[h % 2]
                kg, vg, qg = kg_s[h % 2], vg_s[h % 2], qg_s[h % 2]

                # -- per-head DMAs (overlap previous head's compute) --
                nc.sync.dma_start(qz[0:64, :], qT_d[r0 : r0 + BS, :])
                nc.sync.dma_start(kp[0:64, BS : BS + n], kT_d[r0 : r0 + BS, :])
                nc.sync.dma_start(kp[0:64, 0:BS], kT_d[r0 : r0 + BS, n - BS : n])
                nc.sync.dma_start(kp[0:64, BS + n :], kT_d[r0 : r0 + BS, 0:BS])
                vs = v_d[:, r0 : r0 + BS]
                nc.sync.dma_start(va[0:BS, 0, 0:BS], vs[n - BS : n, :])
                nc.sync.dma_start(va[BS:P, 0, 0:BS], vs[0:BS, :])
                nc.sync.dma_start(
                    va[:, 1 : nkc - 1, 0:BS],
                    vs[BS : n - BS, :].rearrange("(a p) c -> p a c", p=P),
                )
                nc.sync.dma_start(va[0:BS, nkc - 1, 0:BS], vs[n - BS : n, :])
                nc.sync.dma_start(va[BS:P, nkc - 1, 0:BS], vs[0:BS, :])
                for gi, gv in enumerate((g0, g1)):
                    nc.sync.dma_start(
                        kg[0:64, gi * BS : (gi + 1) * BS],
                        kT_d[r0 : r0 + BS, gv * BS : (gv + 1) * BS],
                    )
                    nc.sync.dma_start(
                        vg[gi * BS : (gi + 1) * BS, 0:BS],
                        vs[gv * BS : (gv + 1) * BS, :],
                    )
                    nc.sync.dma_start(
                        qg[0:64, gi * BS : (gi + 1) * BS],
                        qT_d[r0 : r0 + BS, gv * BS : (gv + 1) * BS],
                    )

                # -- local + global-col scores (S^T layout), exp, AV --
                def sc_pair(pr):
                    """scores+exp for padded key chunks 2pr, 2pr+1 (batched)."""
                    sps = psS.tile([P, 2, 256], f32, tag="sps")
                    a_sb = apool.tile([P, 2, 256], bf16, tag="a")
                    nws = []
                    for i in (0, 1):
                        c = 2 * pr + i
                        if c > nt:
                            continue
                        lo = max(0, (c - 1)) * P
                        hi = min(nt, c + 1) * P
                        nws.append(hi - lo)
                        nc.tensor.matmul(
                            sps[:, i, 0 : hi - lo],
                            kp[:, c * P : (c + 1) * P],
                            qz[:, lo:hi],
                            start=True,
                            stop=True,
                        )
                    if nws == [256, 256]:
                        nc.scalar.activation(a_sb, sps, AF.Exp)
                    else:
                        for i, nw in enumerate(nws):
                            nc.scalar.activation(
                                a_sb[:, i, 0:nw], sps[:, i, 0:nw], AF.Exp
                            )
                    return a_sb

                def gc_group(j):
                    spg = psS.tile([P, 512], f32, tag="sps")
                    nc.tensor.matmul(
                        spg, kg, qz[:, j * 512 : (j + 1) * 512], start=True, stop=True
                    )
                    ag = agp.tile([P, 512], bf16, tag="ag")
                    nc.scalar.activation(ag, spg, AF.Exp)
                    return ag

                ag_nxt = gc_group(0)
                a_pair = {0: sc_pair(0), 1: sc_pair(1)}
                ag_cur = None
                cps2 = None
                for t in range(nt):
                    if t % 4 == 0:
                        ag_cur = ag_nxt
                    if t % 4 == 1 and t // 4 + 1 < 8:
                        ag_nxt = gc_group(t // 4 + 1)
                    want = min(nt // 2, t // 2 + 2)
                    if want not in a_pair:
                        a_pair[want] = sc_pair(want)
                        a_pair.pop(want - 3, None)
                    a_lo = a_pair[t // 2][:, t % 2, :]
                    off = 0 if t == 0 else P
                    a_up = a_pair[(t + 1) // 2][:, (t + 1) % 2, :]
                    if t % 2 == 0:
                        cps2 = psC.tile([P, 260], f32, tag="cps")
                    co = (t % 2) * 130
                    cps = cps2[:, co : co + 130]
                    nc.tensor.matmul(
                        cps[:, 0:65],
                        a_lo[:, off : off + P],
                        va[:, t, :],
                        start=True,
                        stop=False,
                    )
                    nc.tensor.matmul(
                        cps[:, 0:65],
                        a_up[:, 0:P],
                        va[:, t + 1, :],
                        start=False,
                        stop=True,
                    )
                    nc.tensor.matmul(
                        cps[:, 65:130],
                        ag_cur[:, (t % 4) * P : (t % 4 + 1) * P],
                        vg,
                        start=True,
                        stop=True,
                    )
                    if t % 2 == 0:
                        continue
                    # batched per-partition reciprocals for both tiles
                    r4 = stat.tile([P, 4], f32, tag="r4")
                    nc.vector.reciprocal(r4, cps2[:, 64:260:65])
                    for tt, cc, ri in ((t - 1, 0, 0), (t, 130, 2)):
                        tg = tgp.tile([P, BS], f32, tag="tg")
                        nc.vector.tensor_scalar_mul(
                            tg, cps2[:, cc + 65 : cc + 129], r4[:, ri + 1 : ri + 2]
                        )
                        nc.vector.scalar_tensor_tensor(
                            ctx_nat[:, tt, r0 : r0 + BS],
                            cps2[:, cc : cc + 64],
                            r4[:, ri : ri + 1],
                            tg,
                            OP.mult,
                            OP.add,
                        )

                # -- global rows: full attention for the 2 global q blocks --
                cpr0 = psQ.tile([P, 130], f32, tag="cpr0")
                cpr1 = cpr0

                def grow_scores(j):
                    spr = psS.tile([P, 4, P], f32, tag="sps")
                    for i in range(4):
                        c = 1 + 4 * j + i
                        nc.tensor.matmul(
                            spr[:, i, :],
                            kp[:, c * P : (c + 1) * P],
                            qg,
                            start=True,
                            stop=True,
                        )
                    ar = agr.tile([P, 4, P], bf16, tag="ar")
                    nc.scalar.activation(ar, spr, AF.Exp)
                    return ar

                ars = [grow_scores(0)]
                for j in range(8):
                    if j + 1 < 8:
                        ars.append(grow_scores(j + 1))
                    for i in range(4):
                        c = 1 + 4 * j + i
                        nc.tensor.matmul(
                            cpr0[p0s : p0s + BS, 0:65],
                            ars[j][:, i, 0:BS],
                            va[:, c, :],
                            start=(c == 1),
                            stop=(c == nkc - 1),
                        )
                for j in range(8):
                    for i in range(4):
                        c = 1 + 4 * j + i
                        nc.tensor.matmul(
                            cpr1[p1s : p1s + BS, 65:130],
                            ars[j][:, i, BS:P],
                            va[:, c, :],
                            start=(c == 1),
                            stop=(c == nkc - 1),
                        )
                for gi, gv, pb, co in ((0, g0, p0s, 0), (1, g1, p1s, 65)):
                    rg = stat.tile([P, 1], f32, tag=f"rg{gi}")
                    nc.vector.reciprocal(
                        rg[pb : pb + BS, :], cpr0[pb : pb + BS, co + 64 : co + 65]
                    )
                    nc.vector.tensor_scalar_mul(
                        ctx_nat[pb : pb + BS, gv // 2, r0 : r0 + BS],
                        cpr0[pb : pb + BS, co : co + 64],
                        rg[pb : pb + BS, :],
                    )

        # ---------------- pass C: output projection ----------------
        with ExitStack() as ps:
            wop = ps.enter_context(tc.tile_pool(name="wop", bufs=1))
            wo_sb = wop.tile([P, ndc2, dmodel], bf16)
            nc.sync.dma_start(wo_sb, wo_d.rearrange("(a p) o -> p a o", p=P))
            ctp = ps.enter_context(tc.tile_pool(name="ctp", bufs=2))
            copool = ps.enter_context(tc.tile_pool(name="co", bufs=4))
            psT = ps.enter_context(tc.tile_pool(name="psT", bufs=4, space="PSUM"))
            psO = ps.enter_context(tc.tile_pool(name="psO", bufs=4, space="PSUM"))
            for ncc in range(nch):
                ctxT = ctp.tile([P, ndc2, 512], bf16, tag="ctxT")
                for ti in range(4):
                    t = ncc * 4 + ti
                    for dc in range(ndc2):
                        tp = psT.tile([P, P], bf16, tag="tp")
                        nc.tensor.transpose(
                            tp, ctx_nat[:, t, dc * P : (dc + 1) * P], ident
                        )
                        if dc % 2 == 0:
                            nc.scalar.copy(ctxT[:, dc, ti * P : (ti + 1) * P], tp)
                        else:
                            nc.vector.tensor_copy(ctxT[:, dc, ti * P : (ti + 1) * P], tp)
                for ot in range(dmodel // P):
                    pp = psO.tile([P, 512], f32, tag="pso")
                    for dc in range(ndc2):
                        nc.tensor.matmul(
                            pp,
                            wo_sb[:, dc, ot * P : (ot + 1) * P],
                            ctxT[:, dc, :],
                            start=(dc == 0),
                            stop=(dc == ndc2 - 1),
                        )
                    ob = copool.tile([P, 512], bf16, tag="ob")
                    if ot % 2 == 0:
                        nc.scalar.copy(ob, pp)
                    else:
                        nc.vector.tensor_copy(ob, pp)
                    nc.sync.dma_start(
                        out_d[ot * P : (ot + 1) * P, ncc * 512 : (ncc + 1) * 512], ob
                    )

        if dbg:
            nc.sync.dma_start(qTo_d[:, :], qT_d)
            nc.sync.dma_start(kTo_d[:, :], kT_d)
            nc.sync.dma_start(vo_d[:, :], v_d)
            nc.sync.dma_start(ctxo_d[:, :, :], ctx_nat)

    nc.finalize()
    return nc


@functools.lru_cache(maxsize=8)
def _get(n, dmodel, dl, g0, g1):
    return _build(n, dmodel, dl, g0, g1)


def _prepare(inputs):
    """Build (nc, in_maps, meta) for the SPMD run from full unsharded inputs."""
    x = np.asarray(inputs["x"], np.float32)
    Wq = np.asarray(inputs["Wq"], np.float32)
    Wk = np.asarray(inputs["Wk"], np.float32)
    Wv = np.asarray(inputs["Wv"], np.float32)
    Wo = np.asarray(inputs["Wo"], np.float32)
    bq = np.asarray(inputs["bq"], np.float32)
    bk = np.asarray(inputs["bk"], np.float32)
    bv = np.asarray(inputs["bv"], np.float32)
    bo = np.asarray(inputs["bo"], np.float32)
    gi = np.asarray(inputs["global_indices"]).astype(np.int64)
    g0, g1 = int(gi[0]), int(gi[1])

    b_, n_, d_ = x.shape
    dl = d_ // 2
    scale = 1.0 / np.sqrt(np.float32(64.0)).astype(np.float32)

    nc = _get(n_, d_, dl, g0, g1)

    import ml_dtypes

    bf = ml_dtypes.bfloat16
    # mask pattern rows (periodic in the column index, see pass B docstring)
    NEGf = np.float32(-1e9)
    j = np.arange(n_) % 256
    qmask = np.zeros((64, n_), np.float32)
    qmask[0, (j >= 128) & (j < 192)] = 1.0  # w1e
    qmask[1, (j >= 64) & (j < 128)] = 1.0   # w2e
    qmask[2, j < 64] = 1.0                  # w1o
    qmask[3, j >= 192] = 1.0                # w2o
    qmask = np.ascontiguousarray(qmask).astype(bf)
    jk = np.arange(n_ + 128) % 256
    kmask = np.zeros((64, n_ + 128), np.float32)
    kmask[0, (jk >= 64) & (jk < 128)] = NEGf   # u1e
    kmask[1, jk < 64] = NEGf                   # u2e
    kmask[2, jk >= 192] = NEGf                 # u1o
    kmask[3, (jk >= 128) & (jk < 192)] = NEGf  # u2o
    kmask = np.ascontiguousarray(kmask).astype(bf)
    in_maps = []
    for c in range(8):
        b, hg = divmod(c, 2)
        S = slice(hg * dl, (hg + 1) * dl)
        in_maps.append(
            {
                "xT": np.ascontiguousarray(x[b].T).astype(bf),
                "qmask": qmask,
                "kmask": kmask,
                "wqT": np.ascontiguousarray((Wq[S, :] * scale).T).astype(bf),
                "wkT": np.ascontiguousarray(Wk[S, :].T).astype(bf),
                "wvT": np.ascontiguousarray(Wv[S, :].T).astype(bf),
                "woT": np.ascontiguousarray(Wo[:, S].T).astype(bf),
                "bq": np.ascontiguousarray(bq[S] * scale),
                "bk": np.ascontiguousarray(bk[S]),
            }
        )

    # host-side bv correction: out[q] += c(q) * bv @ Wo.T, c(q)=1 on global
    # blocks (overwritten by full-attention rows), else 2.
    bvWo = bv @ Wo.T  # [d_model]
    coef = np.full((n_, 1), 2.0, np.float32)
    bs = 64
    coef[g0 * bs : (g0 + 1) * bs] = 1.0
    coef[g1 * bs : (g1 + 1) * bs] = 1.0
    corr = (coef * bvWo[None, :] + bo[None, :]).astype(np.float32)

    return nc, in_maps, (b_, n_, d_, corr)


def _combine(res, meta):
    b_, n_, d_, corr = meta
    out = np.empty((b_, n_, d_), np.float32)
    for b in range(b_):
        out[b] = (
            res[2 * b]["outT"].T.astype(np.float32)
            + res[2 * b + 1]["outT"].T.astype(np.float32)
            + corr
        )
    return out


def kernel(**inputs):
    _ensure_path()
    from concourse.bass_utils import run_bass_kernel_spmd

    nc, in_maps, meta = _prepare(inputs)
    res = run_bass_kernel_spmd(nc, in_maps, list(range(8))).results
    return _combine(res, meta)
